# revision 1
# baseline (speedup 1.0000x reference)
"""Allegro-style GNN message passing on 8 TRN2 NeuronCores.

Strategy:
- Host: shard edges by SENDER node range (1024 nodes/core) -> sender
  segment-sums are fully core-local (no cross-core collectives).
- Within a core, group edges by 128-node sender windows; pad each
  (core, window) group to a common K_WIN with dummy edges (d=2 -> u=0 ->
  zero contribution). One-hot matmuls on TensorE do segment-sum
  (scatter) and the gather-back.
- Layer algebra: Y[:,0] == 1, so layer-1 only needs a 16-wide
  segment-sum of w1; W_lsh[1] output is dead; V1 is only needed at
  component 0 => contraction with Ytil = Y * W_lsh[0][:,0].
- Receiver scatter: node id = hi*128+lo; per edge-tile matmul with lo
  one-hot lhsT and (hi one-hot * edge_out) rhs accumulates [128,64]
  partials in PSUM; host sums the 8 per-core partials (the unshard).
- 1/sqrt(AVG_NEIGH) and the 1/sqrt(2) residual scales are folded into
  weights on the host.
"""
import math
import sys

import numpy as np

sys.path.insert(0, "/opt/trn_rl_repo")

import ml_dtypes  # noqa: E402

BF16 = ml_dtypes.bfloat16
SIM_SILU = False   # CoreSim lacks Silu; emulate with Sigmoid*z when set

N, E, MUL, H, F = 8192, 131072, 16, 256, 16
NB = 8
P6 = 6
INV = 1.0 / math.sqrt(16.0)
NC = 8
NPC = N // NC          # nodes per core
WIN = 128
NW = NPC // WIN        # windows per core
RWIN = N // WIN        # 64 receiver windows
SQ = math.sqrt(0.5)


def _host_shard(node_attrs, vectors, senders, receivers):
    """Group edges by (core, sender-window); pad to common K_WIN."""
    core = senders // NPC
    win = (senders % NPC) // WIN
    order = np.argsort(core * NW + win, kind="stable")
    key = (core * NW + win)[order]
    # group boundaries for all NC*NW groups
    counts = np.bincount(key, minlength=NC * NW)
    kwin = int(((counts.max() + 127) // 128) * 128)
    starts = np.zeros(NC * NW + 1, np.int64)
    np.cumsum(counts, out=starts[1:])

    EP = NW * kwin
    shards = []
    for c in range(NC):
        vec = np.zeros((EP, 3), np.float32)
        vec[:, 0] = 2.0
        a2 = np.zeros((EP, 2 * F), np.float32)
        sl = np.zeros(EP, np.int64)    # sender local-in-window
        rg = np.zeros(EP, np.int64)    # receiver global
        for w in range(NW):
            g = c * NW + w
            eid = order[starts[g]:starts[g + 1]]
            o = w * kwin
            n_e = len(eid)
            vec[o:o + n_e] = vectors[eid]
            a2[o:o + n_e, :F] = node_attrs[senders[eid]]
            a2[o:o + n_e, F:] = node_attrs[receivers[eid]]
            sl[o:o + n_e] = senders[eid] - (c * NPC + w * WIN)
            rg[o:o + n_e] = receivers[eid]
        shards.append((vec, a2, sl, rg))
    return kwin, shards


def _pack_core(kwin, vec, a2, sl, rg):
    """Build the per-core device arrays."""
    EP = NW * kwin
    T_ALL = EP // 128
    # plane layout: edge e = t*128 + p  ->  [3, 128, T_ALL]
    vecp = np.ascontiguousarray(
        vec.reshape(T_ALL, 128, 3).transpose(2, 1, 0)).astype(np.float32)
    attrs2 = np.ascontiguousarray(a2.T).astype(BF16)          # [32, EP]
    eye128 = np.eye(128, dtype=BF16)
    ohs = eye128[sl].reshape(T_ALL, 128, 128)                  # [t, e_p, n]
    oh_s = np.ascontiguousarray(ohs.transpose(1, 0, 2))        # [128, T, 128]
    oh_g = np.ascontiguousarray(ohs.transpose(2, 0, 1))        # [n, T, e]
    rql = eye128[rg % 128].reshape(T_ALL, 128, 128)
    rq = np.ascontiguousarray(rql.transpose(1, 0, 2))          # [128, T, 128]
    eye64 = np.eye(RWIN, dtype=BF16)
    rwl = eye64[rg // 128].reshape(T_ALL, 128, RWIN)
    rwin = np.ascontiguousarray(rwl.transpose(1, 0, 2))        # [128, T, 64]
    return dict(vecp=vecp, attrs2=attrs2, oh_s=oh_s, oh_g=oh_g,
                rq=rq, rwin=rwin)


def _prep_weights(i):
    """Fold INV and residual 1/sqrt(2) scales into weights; cast bf16."""
    w = {}
    w["we0"] = i["W_e0"].astype(BF16)                          # [40,256]
    w["we1"] = i["W_e1"].astype(BF16)                          # [256,256]
    w["wv0"] = i["W_v0"].astype(BF16)                          # [256,16]
    w["wlw0"] = (i["W_lw"][0] * INV).astype(BF16)
    w["wlw1"] = (i["W_lw"][1] * INV * SQ).astype(BF16)         # x1 = sq*x1'
    wly1_0 = i["W_ly1"][0].copy()
    wly1_1 = i["W_ly1"][1].copy()
    wly1_1[:H] *= SQ                                           # x rows scaled
    w["wly1_0"] = wly1_0.astype(BF16)
    w["wly1_1"] = wly1_1.astype(BF16)
    w["wly2_0"] = i["W_ly2"][0].astype(BF16)
    w["wly2_1"] = i["W_ly2"][1].astype(BF16)
    w["wout"] = (i["W_out"] * INV * 0.5).astype(BF16)          # x2 = .5*x2'
    w["be0"] = i["b_e0"].reshape(H, 1).astype(np.float32)
    w["be1"] = i["b_e1"].reshape(H, 1).astype(np.float32)
    w["bly1_0"] = i["b_ly1"][0].reshape(H, 1).astype(np.float32)
    w["bly1_1"] = i["b_ly1"][1].reshape(H, 1).astype(np.float32)
    w["bly2_0"] = i["b_ly2"][0].reshape(H, 1).astype(np.float32)
    w["bly2_1"] = i["b_ly2"][1].reshape(H, 1).astype(np.float32)
    w["wcol"] = np.tile(i["W_lsh"][0][:, 0].reshape(1, MUL),
                        (128, 1)).astype(np.float32)           # [128,16]
    w["ones"] = np.ones((1, 128), BF16)
    return w


_CAP_SKIP = {"InstEventSemaphore", "InstBranch", "InstNop",
             "InstCollectiveCompute"}
_CAP_LIMITS = {}


def _split_waits(nc, mybir, mk_carrier, limit=1):
    """Walrus codegen allows only 1 embedded sem-wait on compute
    instructions.  For each instruction with more, strip the extras onto
    freshly created same-engine carrier instructions inserted directly
    before it (engines are in-order, so this preserves semantics)."""
    f = nc.m.functions[0]
    made = 0
    # find blocks that carriers get appended to, to strip later
    for bb in f.blocks:
        insts = list(bb.instructions)
        plan = []          # (index, [carrier insts])
        for i, inst in enumerate(insts):
            tname = type(inst).__name__
            si = inst.sync_info
            nwait = len(si.on_wait) if (si and si.on_wait) else 0
            lim = _CAP_LIMITS.get(tname, limit)
            if tname in _CAP_SKIP or nwait <= lim:
                continue
            waits = list(si.on_wait)
            extras, keep = waits[:-lim], waits[-lim:]
            carriers = []
            for wt in extras:
                ci = mk_carrier(inst.engine)
                if ci is None:
                    keep.insert(0, wt)
                    continue
                ci.sync_info = mybir.SyncInfo(on_wait=[wt], on_update=[])
                carriers.append(ci)
                made += 1
            inst.sync_info = mybir.SyncInfo(on_wait=keep,
                                            on_update=si.on_update)
            if carriers:
                plan.append((i, carriers))
        if plan:
            new = []
            pi = 0
            pmap = dict(plan)
            for i, inst in enumerate(insts):
                if i in pmap:
                    new.extend(pmap[i])
                new.append(inst)
            bb.instructions = new
    return made


def build_graph(kwin):
    from concourse import bass, mybir
    from concourse.masks import make_identity
    from concourse.tile import TileContext

    EP = NW * kwin
    T_ALL = EP // 128
    T_W = kwin // 128
    NCH = (kwin + 511) // 512      # free chunks per window

    f32 = mybir.dt.float32
    bf16 = mybir.dt.bfloat16
    AX = mybir.AxisListType.X
    OP = mybir.AluOpType
    AF = mybir.ActivationFunctionType

    nc = bass.Bass()
    carrier_sem_cm = nc.semaphore("carrier_sem")
    carrier_sem = carrier_sem_cm.__enter__()
    dp = nc.declare_dram_parameter
    d_vecp = dp("vecp", [3, 128, T_ALL], f32, isOutput=False)
    d_attrs = dp("attrs2", [32, EP], bf16, isOutput=False)
    d_ohs = dp("oh_s", [128, T_ALL, 128], bf16, isOutput=False)
    d_ohg = dp("oh_g", [128, T_ALL, 128], bf16, isOutput=False)
    d_rq = dp("rq", [128, T_ALL, 128], bf16, isOutput=False)
    d_rwin = dp("rwin", [128, T_ALL, RWIN], bf16, isOutput=False)
    d_we0 = dp("we0", [40, H], bf16, isOutput=False)
    d_we1 = dp("we1", [H, H], bf16, isOutput=False)
    d_wv0 = dp("wv0", [H, MUL], bf16, isOutput=False)
    d_wlw0 = dp("wlw0", [H, MUL], bf16, isOutput=False)
    d_wlw1 = dp("wlw1", [H, MUL], bf16, isOutput=False)
    d_wly1 = [dp("wly1_0", [H + MUL, H], bf16, isOutput=False),
              dp("wly1_1", [H + MUL, H], bf16, isOutput=False)]
    d_wly2 = [dp("wly2_0", [H, H], bf16, isOutput=False),
              dp("wly2_1", [H, H], bf16, isOutput=False)]
    d_wout = dp("wout", [H, 1], bf16, isOutput=False)
    d_be0 = dp("be0", [H, 1], f32, isOutput=False)
    d_be1 = dp("be1", [H, 1], f32, isOutput=False)
    d_bly1 = [dp("bly1_0", [H, 1], f32, isOutput=False),
              dp("bly1_1", [H, 1], f32, isOutput=False)]
    d_bly2 = [dp("bly2_0", [H, 1], f32, isOutput=False),
              dp("bly2_1", [H, 1], f32, isOutput=False)]
    d_wcol = dp("wcol", [128, MUL], f32, isOutput=False)
    d_ones = dp("ones", [1, 128], bf16, isOutput=False)
    d_out = dp("out", [128, RWIN], f32, isOutput=True)

    with TileContext(nc) as tc:
        with (
            tc.tile_pool(name="glob", bufs=1) as gp,
            tc.tile_pool(name="wgt", bufs=1) as wp,
            tc.tile_pool(name="win", bufs=2) as wnp,
            tc.tile_pool(name="big", bufs=1) as bgp,
            tc.tile_pool(name="sml", bufs=3) as sp,
            tc.tile_pool(name="ps_mlp", bufs=2, space="PSUM") as pmlp,
            tc.tile_pool(name="ps_acc", bufs=1, space="PSUM") as pacc,
            tc.tile_pool(name="ps_gth", bufs=2, space="PSUM") as pgth,
            tc.tile_pool(name="ps_sml", bufs=2, space="PSUM") as psml,
            tc.tile_pool(name="ps_rcv", bufs=1, space="PSUM") as prcv,
        ):
            # ---------------- weights to SBUF ----------------
            def ld(d, shape, dt, tag):
                t = wp.tile(shape, dt, tag=tag)
                nc.sync.dma_start(out=t[:], in_=d[:])
                return t

            def ld2(d, cols, dt, tag, rows=H):
                # [rows, cols] -> [128, rows//128, cols] k-chunked
                nkc = rows // 128
                t = wp.tile([128, nkc, cols], dt, tag=tag)
                for kc in range(nkc):
                    nc.sync.dma_start(out=t[:, kc, :],
                                      in_=d[kc * 128:(kc + 1) * 128, :])
                return t
            we0a = ld(d_we0[0:8, :], [8, H], bf16, "we0a")
            we0b = ld(d_we0[8:40, :], [32, H], bf16, "we0b")
            we1 = ld2(d_we1, H, bf16, "we1")
            wv0 = ld2(d_wv0, MUL, bf16, "wv0")
            wlw0 = ld2(d_wlw0, MUL, bf16, "wlw0")
            wlw1 = ld2(d_wlw1, MUL, bf16, "wlw1")
            wly1 = [ld2(d_wly1[l], H, bf16, f"wly1_{l}") for l in range(2)]
            wly1fb = [ld(d_wly1[l][256:272, :], [MUL, H], bf16,
                         f"wly1fb_{l}") for l in range(2)]
            wly2 = [ld2(d_wly2[l], H, bf16, f"wly2_{l}") for l in range(2)]
            wout = ld2(d_wout, 1, bf16, "wout")
            be0 = ld2(d_be0, 1, f32, "be0")
            be1 = ld2(d_be1, 1, f32, "be1")
            bly1 = [ld2(d_bly1[l], 1, f32, f"bly1_{l}") for l in range(2)]
            bly2 = [ld2(d_bly2[l], 1, f32, f"bly2_{l}") for l in range(2)]
            wcol = ld(d_wcol, [128, MUL], f32, "wcol")
            attrs = ld(d_attrs, [32, EP], bf16, "attrs")
            ident = wp.tile([128, 128], f32, tag="ident")
            make_identity(nc, ident[:])

            ones_bf = ld(d_ones, [1, 128], bf16, "ones")

            # ---------------- edge-scalar stage (planes [128,T_ALL]) ----
            vx = gp.tile([128, T_ALL], f32)
            vy = gp.tile([128, T_ALL], f32)
            vz = gp.tile([128, T_ALL], f32)
            nc.gpsimd.dma_start(out=vx[:], in_=d_vecp[0])
            nc.gpsimd.dma_start(out=vy[:], in_=d_vecp[1])
            nc.gpsimd.dma_start(out=vz[:], in_=d_vecp[2])
            ta = gp.tile([128, T_ALL], f32)
            tb = gp.tile([128, T_ALL], f32)
            tt = nc.vector.tensor_tensor
            ts = nc.vector.tensor_scalar
            act = nc.scalar.activation

            def silu_act(out, ps_in, bias_ap):
                if not SIM_SILU:
                    act(out=out, in_=ps_in, func=AF.Silu, bias=bias_ap)
                else:
                    pp = ps_in.shape[0]
                    sg = bgp.tile([128, 512], f32, tag="simsilu")
                    zz_ = bgp.tile([128, 512], f32, tag="simsilu2")
                    cw_ = ps_in.shape[-1]
                    act(out=sg[:pp, :cw_], in_=ps_in, func=AF.Sigmoid,
                        bias=bias_ap)
                    nc.vector.tensor_scalar(out=zz_[:pp, :cw_], in0=ps_in,
                                            scalar1=bias_ap, scalar2=None,
                                            op0=OP.add)
                    nc.vector.tensor_mul(out=out, in0=sg[:pp, :cw_],
                                         in1=zz_[:pp, :cw_])
            d_pl = gp.tile([128, T_ALL], f32)
            nc.vector.tensor_mul(out=ta[:], in0=vx[:], in1=vx[:])
            nc.vector.tensor_mul(out=tb[:], in0=vy[:], in1=vy[:])
            nc.vector.tensor_add(out=ta[:], in0=ta[:], in1=tb[:])
            nc.vector.tensor_mul(out=tb[:], in0=vz[:], in1=vz[:])
            nc.vector.tensor_add(out=ta[:], in0=ta[:], in1=tb[:])
            act(out=d_pl[:], in_=ta[:], func=AF.Sqrt)
            rinv = gp.tile([128, T_ALL], f32)
            nc.vector.reciprocal(out=rinv[:], in_=d_pl[:])
            ux = gp.tile([128, T_ALL], f32)
            uy = gp.tile([128, T_ALL], f32)
            uz = gp.tile([128, T_ALL], f32)
            nc.vector.tensor_mul(out=ux[:], in0=vx[:], in1=rinv[:])
            nc.vector.tensor_mul(out=uy[:], in0=vy[:], in1=rinv[:])
            nc.vector.tensor_mul(out=uz[:], in0=vz[:], in1=rinv[:])

            # envelope u = 1 + d^6*(-28 + 48d - 21d^2), zero for d >= 1
            u_pl = gp.tile([128, T_ALL], f32)
            nc.vector.tensor_mul(out=ta[:], in0=d_pl[:], in1=d_pl[:])   # d2
            nc.vector.tensor_mul(out=tb[:], in0=ta[:], in1=d_pl[:])     # d3
            nc.vector.tensor_mul(out=tb[:], in0=tb[:], in1=tb[:])       # d6
            ts(out=ta[:], in0=ta[:], scalar1=-21.0, scalar2=None, op0=OP.mult)
            tc_q = gp.tile([128, T_ALL], f32)
            ts(out=tc_q[:], in0=d_pl[:], scalar1=48.0, scalar2=-28.0,
               op0=OP.mult, op1=OP.add)
            nc.vector.tensor_add(out=ta[:], in0=ta[:], in1=tc_q[:])
            nc.vector.tensor_mul(out=tb[:], in0=tb[:], in1=ta[:])
            ts(out=tb[:], in0=tb[:], scalar1=1.0, scalar2=None, op0=OP.add)
            ts(out=ta[:], in0=d_pl[:], scalar1=1.0, scalar2=None,
               op0=OP.is_lt)
            nc.vector.tensor_mul(out=u_pl[:], in0=tb[:], in1=ta[:])

            # spherical harmonics Y [128, T_ALL, 16] f32
            Yt = gp.tile([128, T_ALL, 16], f32)
            s3 = 3.0 ** 0.5; s5 = 5.0 ** 0.5; s15 = 15.0 ** 0.5
            s7 = 7.0 ** 0.5
            c33 = (35.0 / 8.0) ** 0.5; c32 = 105.0 ** 0.5
            c31 = (21.0 / 8.0) ** 0.5
            xx = gp.tile([128, T_ALL], f32)
            yy = gp.tile([128, T_ALL], f32)
            zz = gp.tile([128, T_ALL], f32)
            xy = gp.tile([128, T_ALL], f32)
            nc.vector.tensor_mul(out=xx[:], in0=ux[:], in1=ux[:])
            nc.vector.tensor_mul(out=yy[:], in0=uy[:], in1=uy[:])
            nc.vector.tensor_mul(out=zz[:], in0=uz[:], in1=uz[:])
            nc.vector.tensor_mul(out=xy[:], in0=ux[:], in1=uy[:])
            ts(out=Yt[:, :, 0], in0=ux[:], scalar1=0.0, scalar2=1.0,
               op0=OP.mult, op1=OP.add)
            ts(out=Yt[:, :, 1], in0=ux[:], scalar1=s3, scalar2=None,
               op0=OP.mult)
            ts(out=Yt[:, :, 2], in0=uy[:], scalar1=s3, scalar2=None,
               op0=OP.mult)
            ts(out=Yt[:, :, 3], in0=uz[:], scalar1=s3, scalar2=None,
               op0=OP.mult)
            ts(out=Yt[:, :, 4], in0=xy[:], scalar1=s15, scalar2=None,
               op0=OP.mult)
            nc.vector.tensor_mul(out=ta[:], in0=uy[:], in1=uz[:])
            ts(out=Yt[:, :, 5], in0=ta[:], scalar1=s15, scalar2=None,
               op0=OP.mult)
            ts(out=Yt[:, :, 6], in0=zz[:], scalar1=1.5 * s5,
               scalar2=-0.5 * s5, op0=OP.mult, op1=OP.add)
            nc.vector.tensor_mul(out=tb[:], in0=ux[:], in1=uz[:])
            ts(out=Yt[:, :, 7], in0=tb[:], scalar1=s15, scalar2=None,
               op0=OP.mult)
            xmy = gp.tile([128, T_ALL], f32)
            nc.vector.tensor_sub(out=xmy[:], in0=xx[:], in1=yy[:])
            ts(out=Yt[:, :, 8], in0=xmy[:], scalar1=0.5 * s15, scalar2=None,
               op0=OP.mult)
            # Y9 = c33*y*(3xx-yy)
            ts(out=ta[:], in0=xx[:], scalar1=3.0, scalar2=None, op0=OP.mult)
            nc.vector.tensor_sub(out=ta[:], in0=ta[:], in1=yy[:])
            nc.vector.tensor_mul(out=ta[:], in0=ta[:], in1=uy[:])
            ts(out=Yt[:, :, 9], in0=ta[:], scalar1=c33, scalar2=None,
               op0=OP.mult)
            # Y10 = c32*x*y*z
            nc.vector.tensor_mul(out=ta[:], in0=xy[:], in1=uz[:])
            ts(out=Yt[:, :, 10], in0=ta[:], scalar1=c32, scalar2=None,
               op0=OP.mult)
            # Y11/Y13: c31*{y,x}*(5zz-1)
            ts(out=ta[:], in0=zz[:], scalar1=5.0, scalar2=-1.0,
               op0=OP.mult, op1=OP.add)
            nc.vector.tensor_mul(out=tb[:], in0=ta[:], in1=uy[:])
            ts(out=Yt[:, :, 11], in0=tb[:], scalar1=c31, scalar2=None,
               op0=OP.mult)
            nc.vector.tensor_mul(out=tb[:], in0=ta[:], in1=ux[:])
            ts(out=Yt[:, :, 13], in0=tb[:], scalar1=c31, scalar2=None,
               op0=OP.mult)
            # Y12 = 2.5*s7*z^3 - 1.5*s7*z
            nc.vector.tensor_mul(out=ta[:], in0=zz[:], in1=uz[:])
            ts(out=ta[:], in0=ta[:], scalar1=2.5 * s7, scalar2=None,
               op0=OP.mult)
            ts(out=tb[:], in0=uz[:], scalar1=1.5 * s7, scalar2=None,
               op0=OP.mult)
            nc.vector.tensor_sub(out=Yt[:, :, 12], in0=ta[:], in1=tb[:])
            # Y14 = 0.5*c32*z*(xx-yy)
            nc.vector.tensor_mul(out=ta[:], in0=xmy[:], in1=uz[:])
            ts(out=Yt[:, :, 14], in0=ta[:], scalar1=0.5 * c32, scalar2=None,
               op0=OP.mult)
            # Y15 = c33*x*(xx-3yy)
            ts(out=ta[:], in0=yy[:], scalar1=3.0, scalar2=None, op0=OP.mult)
            nc.vector.tensor_sub(out=ta[:], in0=xx[:], in1=ta[:])
            nc.vector.tensor_mul(out=ta[:], in0=ta[:], in1=ux[:])
            ts(out=Yt[:, :, 15], in0=ta[:], scalar1=c33, scalar2=None,
               op0=OP.mult)

            # bessel (range-reduced): besu [128, T_ALL, 9]; col 8 = u
            besu = gp.tile([128, T_ALL, 8], f32)
            rs = gp.tile([128, T_ALL], f32)
            ts(out=rs[:], in0=rinv[:], scalar1=math.sqrt(2.0), scalar2=None,
               op0=OP.mult)
            mi = gp.tile([128, T_ALL], mybir.dt.int32)
            for k in range(1, NB + 1):
                ts(out=ta[:], in0=d_pl[:], scalar1=0.5 * k, scalar2=None,
                   op0=OP.mult)
                nc.vector.tensor_copy(out=mi[:], in_=ta[:])
                nc.vector.tensor_copy(out=tb[:], in_=mi[:])
                nc.vector.tensor_sub(out=ta[:], in0=ta[:], in1=tb[:])
                # ta = frac in (-0.5, 1) whether the cast rounds or truncates
                ts(out=tb[:], in0=ta[:], scalar1=0.5, scalar2=None,
                   op0=OP.is_gt)
                nc.vector.tensor_sub(out=ta[:], in0=ta[:], in1=tb[:])
                act(out=ta[:], in_=ta[:], func=AF.Sin, scale=2.0 * math.pi)
                nc.vector.tensor_mul(out=besu[:, :, k - 1], in0=ta[:],
                                      in1=rs[:])


            # ---------------- persistent receiver accumulator ----------
            ps_rcv = prcv.tile([128, RWIN], f32, space="PSUM")

            # ---------------- window loop ----------------
            for w in range(NW):
                t0 = w * T_W
                wsl = slice(w * kwin, (w + 1) * kwin)
                ohs = wnp.tile([128, T_W, 128], bf16)
                ohg = wnp.tile([128, T_W, 128], bf16)
                rqt = wnp.tile([128, T_W, 128], bf16)
                rwt = wnp.tile([128, T_W, RWIN], bf16)
                nc.sync.dma_start(out=ohs[:], in_=d_ohs[:, t0:t0 + T_W, :])
                nc.sync.dma_start(out=ohg[:], in_=d_ohg[:, t0:t0 + T_W, :])
                nc.sync.dma_start(out=rqt[:], in_=d_rq[:, t0:t0 + T_W, :])
                nc.sync.dma_start(out=rwt[:], in_=d_rwin[:, t0:t0 + T_W, :])

                # per-window feature-major bes/u rows via PE transpose
                besfm = wnp.tile([8, kwin], bf16)
                ufm = wnp.tile([1, kwin], bf16)
                for t in range(T_W):
                    pst = psml.tile([16, 128], f32, space="PSUM", tag="sml")
                    nc.tensor.transpose(out=pst[0:8, :],
                                        in_=besu[:, t0 + t, :],
                                        identity=ident[:])
                    nc.vector.tensor_copy(out=besfm[:, t * 128:(t + 1) * 128],
                                          in_=pst[0:8, :])
                    psu1 = psml.tile([16, 128], f32, space="PSUM", tag="sml")
                    nc.tensor.transpose(out=psu1[0:1, :],
                                        in_=u_pl[:, t0 + t, None],
                                        identity=ident[:])
                    nc.vector.tensor_copy(out=ufm[:, t * 128:(t + 1) * 128],
                                          in_=psu1[0:1, :])
                # broadcast u row -> [128, kwin] bf16
                ubc = bgp.tile([128, kwin], bf16)
                for ch in range(NCH):
                    c0 = ch * 512
                    c1 = min(kwin, c0 + 512)
                    psu = pmlp.tile([128, 512], f32, space="PSUM", tag="mlp")
                    nc.tensor.matmul(out=psu[:, :c1 - c0], lhsT=ones_bf[:],
                                     rhs=ufm[:, c0:c1],
                                     start=True, stop=True)
                    nc.vector.tensor_copy(out=ubc[:, c0:c1],
                                          in_=psu[:, :c1 - c0])

                # ---- edge MLP: x0 = u*silu(e1(silu(e0(bes,attrs)))) ----
                x0 = bgp.tile([128, 2, kwin], bf16)
                th = bgp.tile([128, 2, kwin], bf16)
                for ch in range(NCH):
                    c0 = ch * 512
                    c1 = min(kwin, c0 + 512)
                    cw = c1 - c0
                    for hc in range(2):
                        hs = slice(hc * 128, (hc + 1) * 128)
                        ps = pmlp.tile([128, 512], f32, space="PSUM", tag="mlp")
                        nc.tensor.matmul(out=ps[:, :cw], lhsT=we0a[:, hs],
                                         rhs=besfm[0:8, c0:c1],
                                         start=True, stop=False)
                        nc.tensor.matmul(out=ps[:, :cw], lhsT=we0b[:, hs],
                                         rhs=attrs[:, wsl][:, c0:c1],
                                         start=False, stop=True)
                        silu_act(th[:, hc, c0:c1], ps[:, :cw], be0[:, hc, :])
                for ch in range(NCH):
                    c0 = ch * 512
                    c1 = min(kwin, c0 + 512)
                    cw = c1 - c0
                    for hc in range(2):
                        hs = slice(hc * 128, (hc + 1) * 128)
                        ps = pmlp.tile([128, 512], f32, space="PSUM", tag="mlp")
                        for kc in range(2):
                            ks = slice(kc * 128, (kc + 1) * 128)
                            nc.tensor.matmul(out=ps[:, :cw],
                                             lhsT=we1[:, kc, hs],
                                             rhs=th[:, kc, c0:c1],
                                             start=(kc == 0), stop=(kc == 1))
                        silu_act(x0[:, hc, c0:c1], ps[:, :cw], be1[:, hc, :])
                for hc in range(2):
                    nc.vector.tensor_mul(out=x0[:, hc, :], in0=x0[:, hc, :],
                                          in1=ubc[:])

                # ---- xv, w0 (edge-major [128,16] per tile) ----
                xv = wnp.tile([128, T_W, MUL], f32)
                w0 = wnp.tile([128, T_W, MUL], bf16)
                for t in range(T_W):
                    tsl = slice(t * 128, (t + 1) * 128)
                    p1 = psml.tile([128, MUL], f32, space="PSUM", tag="sml")
                    p2 = psml.tile([128, MUL], f32, space="PSUM", tag="sml")
                    for kc in range(2):
                        ks = slice(kc * 128, (kc + 1) * 128)
                        nc.tensor.matmul(out=p1[:], lhsT=x0[:, kc, tsl],
                                         rhs=wv0[:, kc, :],
                                         start=(kc == 0), stop=(kc == 1))
                        nc.tensor.matmul(out=p2[:], lhsT=x0[:, kc, tsl],
                                         rhs=wlw0[:, kc, :],
                                         start=(kc == 0), stop=(kc == 1))
                    nc.vector.tensor_copy(out=xv[:, t, :], in_=p1[:])
                    nc.vector.tensor_copy(out=w0[:, t, :], in_=p2[:])

                # ---- layer-0 scatter: wY[n, m*16+i] ----
                ps_acc = pacc.tile([128, 256], f32, space="PSUM", tag="acc")
                val = wnp.tile([128, MUL, 16], bf16)
                for t in range(T_W):
                    v2 = sp.tile([128, MUL, 16], bf16)
                    nc.vector.tensor_mul(
                        out=v2[:],
                        in0=w0[:, t, :, None].to_broadcast([128, MUL, 16]),
                        in1=Yt[:, t0 + t, None, :].to_broadcast(
                            [128, MUL, 16]))
                    nc.tensor.matmul(
                        out=ps_acc[:],
                        lhsT=ohs[:, t, :],
                        rhs=v2[:].rearrange("p a b -> p (a b)"),
                        start=(t == 0), stop=(t == T_W - 1))
                wY = wnp.tile([128, 256], bf16)
                nc.vector.tensor_copy(out=wY[:], in_=ps_acc[:])

                # ---- gather + Ytil contraction + feedback ----
                V10 = wnp.tile([128, T_W, MUL], f32)
                fbfm = wnp.tile([MUL, kwin], bf16)
                prod = wnp.tile([128, MUL, 16], f32)
                ytil = wnp.tile([128, MUL], f32)
                Ssb = wnp.tile([128, MUL], f32)
                fb = wnp.tile([128, MUL], f32)
                for t in range(T_W):
                    pg = pgth.tile([128, 256], f32, space="PSUM", tag="gth")
                    nc.tensor.matmul(out=pg[:], lhsT=ohg[:, t, :], rhs=wY[:],
                                     start=True, stop=True)
                    pg3 = pg[:].rearrange("p (a b) -> p a b", b=16)
                    nc.vector.tensor_mul(out=ytil[:], in0=Yt[:, t0 + t, :],
                                          in1=wcol[:])
                    nc.vector.tensor_mul(
                        out=prod[:], in0=pg3,
                        in1=ytil[:, None, :].to_broadcast([128, MUL, 16]))
                    nc.vector.reduce_sum(out=Ssb[:, :, None], in_=prod[:],
                                         axis=AX)
                    nc.vector.tensor_mul(out=V10[:, t, :], in0=Ssb[:],
                                          in1=xv[:, t, :])
                    nc.vector.tensor_mul(out=fb[:], in0=pg3[:, :, 0],
                                          in1=xv[:, t, :])
                    pst = psml.tile([16, 128], f32, space="PSUM", tag="sml")
                    nc.tensor.transpose(out=pst[:], in_=fb[:],
                                        identity=ident[:])
                    nc.vector.tensor_copy(out=fbfm[:, t * 128:(t + 1) * 128],
                                          in_=pst[:])

                # ---- layer-0 ly1/ly2 + residual -> x1 ----
                x1 = bgp.tile([128, 2, kwin], bf16)

                def mlp_block(xin, xout, wl1, wl1fb, bl1, wl2, bl2, fbrow, resid_sq2):
                    ty = bgp.tile([128, 2, kwin], bf16)
                    for ch in range(NCH):
                        c0 = ch * 512
                        c1 = min(kwin, c0 + 512)
                        cw = c1 - c0
                        for hc in range(2):
                            hs = slice(hc * 128, (hc + 1) * 128)
                            ps = pmlp.tile([128, 512], f32, space="PSUM", tag="mlp")
                            for kc in range(2):
                                ks = slice(kc * 128, (kc + 1) * 128)
                                nc.tensor.matmul(out=ps[:, :cw],
                                                 lhsT=wl1[:, kc, hs],
                                                 rhs=xin[:, kc, c0:c1],
                                                 start=(kc == 0), stop=False)
                            nc.tensor.matmul(out=ps[:, :cw],
                                             lhsT=wl1fb[:, hs],
                                             rhs=fbrow[:, c0:c1],
                                             start=False, stop=True)
                            silu_act(ty[:, hc, c0:c1], ps[:, :cw], bl1[:, hc, :])
                    ty2 = bgp.tile([128, 2, kwin], bf16)
                    for ch in range(NCH):
                        c0 = ch * 512
                        c1 = min(kwin, c0 + 512)
                        cw = c1 - c0
                        for hc in range(2):
                            hs = slice(hc * 128, (hc + 1) * 128)
                            ps = pmlp.tile([128, 512], f32, space="PSUM", tag="mlp")
                            for kc in range(2):
                                ks = slice(kc * 128, (kc + 1) * 128)
                                nc.tensor.matmul(out=ps[:, :cw],
                                                 lhsT=wl2[:, kc, hs],
                                                 rhs=ty[:, kc, c0:c1],
                                                 start=(kc == 0),
                                                 stop=(kc == 1))
                            silu_act(ty2[:, hc, c0:c1], ps[:, :cw], bl2[:, hc, :])
                    # x_out' = x_in' + s * u * y   (s = 1 or sqrt(2))
                    for hc in range(2):
                        nc.vector.tensor_mul(out=ty2[:, hc, :],
                                              in0=ty2[:, hc, :], in1=ubc[:])
                        if resid_sq2:
                            ts(out=ty2[:, hc, :], in0=ty2[:, hc, :],
                               scalar1=math.sqrt(2.0), scalar2=None,
                               op0=OP.mult)
                        nc.vector.tensor_add(out=xout[:, hc, :],
                                             in0=xin[:, hc, :],
                                             in1=ty2[:, hc, :])

                mlp_block(x0, x1, wly1[0], wly1fb[0], bly1[0], wly2[0], bly2[0],
                          fbfm, False)

                # ---- layer 1: w1, 16-wide scatter/gather, feedback ----
                w1 = wnp.tile([128, T_W, MUL], bf16)
                for t in range(T_W):
                    tsl = slice(t * 128, (t + 1) * 128)
                    p1 = psml.tile([128, MUL], f32, space="PSUM", tag="sml")
                    for kc in range(2):
                        ks = slice(kc * 128, (kc + 1) * 128)
                        nc.tensor.matmul(out=p1[:], lhsT=x1[:, kc, tsl],
                                         rhs=wlw1[:, kc, :],
                                         start=(kc == 0), stop=(kc == 1))
                    nc.vector.tensor_copy(out=w1[:, t, :], in_=p1[:])
                ps_a1 = pacc.tile([128, 256], f32, space="PSUM", tag="acc")
                for t in range(T_W):
                    nc.tensor.matmul(out=ps_a1[:, 0:MUL], lhsT=ohs[:, t, :],
                                     rhs=w1[:, t, :],
                                     start=(t == 0), stop=(t == T_W - 1))
                wY1 = wnp.tile([128, MUL], bf16)
                nc.vector.tensor_copy(out=wY1[:], in_=ps_a1[:, 0:MUL])
                fbfm1 = wnp.tile([MUL, kwin], bf16)
                fb1 = wnp.tile([128, MUL], f32)
                for t in range(T_W):
                    pg = pgth.tile([128, 256], f32, space="PSUM", tag="gth")
                    nc.tensor.matmul(out=pg[:, 0:MUL], lhsT=ohg[:, t, :],
                                     rhs=wY1[:], start=True, stop=True)
                    nc.vector.tensor_mul(out=fb1[:], in0=pg[:, 0:MUL],
                                          in1=V10[:, t, :])
                    pst = psml.tile([16, 128], f32, space="PSUM", tag="sml")
                    nc.tensor.transpose(out=pst[:], in_=fb1[:],
                                        identity=ident[:])
                    nc.vector.tensor_copy(out=fbfm1[:, t * 128:(t + 1) * 128],
                                          in_=pst[:])

                # ---- layer-1 ly1/ly2 + residual -> x2 ----
                x2 = bgp.tile([128, 2, kwin], bf16)
                mlp_block(x1, x2, wly1[1], wly1fb[1], bly1[1], wly2[1], bly2[1],
                          fbfm1, True)

                # ---- edge out + receiver scatter ----
                eo = wnp.tile([128, 1], f32)
                mt = wnp.tile([128, RWIN], bf16)
                for t in range(T_W):
                    tsl = slice(t * 128, (t + 1) * 128)
                    p1 = psml.tile([128, MUL], f32, space="PSUM", tag="sml")
                    for kc in range(2):
                        ks = slice(kc * 128, (kc + 1) * 128)
                        nc.tensor.matmul(out=p1[:, 0:1], lhsT=x2[:, kc, tsl],
                                         rhs=wout[:, kc, :],
                                         start=(kc == 0), stop=(kc == 1))
                    nc.vector.tensor_mul(out=eo[:], in0=p1[:, 0:1],
                                          in1=u_pl[:, t0 + t, None])
                    nc.vector.tensor_mul(
                        out=mt[:], in0=rwt[:, t, :],
                        in1=eo[:].to_broadcast([128, RWIN]))
                    nc.tensor.matmul(out=ps_rcv[:], lhsT=rqt[:, t, :],
                                     rhs=mt[:],
                                     start=(w == 0 and t == 0),
                                     stop=(w == NW - 1 and t == T_W - 1))

            out_sb = gp.tile([128, RWIN], f32)
            nc.vector.tensor_copy(out=out_sb[:], in_=ps_rcv[:])
            nc.sync.dma_start(out=d_out[:], in_=out_sb[:])

    ET = mybir.EngineType
    eng_map = {ET.DVE: nc.vector, ET.Activation: nc.scalar,
               ET.Pool: nc.gpsimd, ET.PE: nc.tensor, ET.SP: nc.sync}

    def mk_carrier(eng):
        be = eng_map.get(eng)
        if be is None:
            return None
        w = be.wait_ge(carrier_sem, 0)
        ci = w.ins if hasattr(w, "ins") else w
        # strip from whatever block it was appended to
        for bb in nc.m.functions[0].blocks:
            il = list(bb.instructions)
            if any(x is ci for x in il):
                bb.instructions = [x for x in il if x is not ci]
                break
        return ci

    made = _split_waits(nc, mybir, mk_carrier)
    print(f"split_waits: carriers={made}", flush=True)
    return nc


def make_in_maps(inputs):
    kwin, shards = _host_shard(inputs["node_attrs"], inputs["vectors"],
                               inputs["senders"], inputs["receivers"])
    w = _prep_weights(inputs)
    in_maps = []
    for c in range(NC):
        m = dict(w)
        m.update(_pack_core(kwin, *shards[c]))
        in_maps.append({k: np.ascontiguousarray(v) for k, v in m.items()})
    return kwin, in_maps


def kernel(**inputs):
    inputs = {k: np.asarray(v) for k, v in inputs.items()}
    kwin, in_maps = make_in_maps(inputs)
    nc = build_graph(kwin)
    from concourse.bass_utils import run_bass_kernel_spmd
    res = run_bass_kernel_spmd(nc, in_maps, core_ids=list(range(NC)))
    out = np.zeros((128, RWIN), np.float64)
    for r in res.results:
        out += np.asarray(r["out"], np.float64)
    # node n = hi*128 + lo stored at [lo, hi]
    return np.ascontiguousarray(out.T.reshape(N, 1)).astype(np.float32)



# revision 15
# speedup vs baseline: 2.9005x; 2.9005x over previous
"""Allegro-style GNN message passing on 8 TRN2 NeuronCores.

Strategy (v2 — minimal host->device bytes):
- Host: shard edges by SENDER node range (1024 nodes/core) -> sender
  segment-sums are fully core-local (no cross-core collectives).
  Within a core, group edges by 128-node sender windows; pad each
  (core, window) group to a common K_WIN with dummy edges (d=2 -> u=0 ->
  zero contribution).
- Inputs per core are just 3 packed blobs (~1.4 MB total): u8 index
  planes (sender-local / receiver-lo / receiver-hi), an f32 blob
  (edge vectors + biases + wcol), and a bf16 blob (node table +
  weights). One-hot scatter/gather matrices and endpoint-attribute
  gathers are built ON DEVICE (iota + is_equal + PE transposes +
  one-hot matmuls) instead of being shipped from the host -- the axon
  PJRT tunnel moves ~40 MB/s, so the previous 17.7 MB/core of host-
  built one-hots dominated wall time.
- Layer algebra: Y[:,0] == 1, so layer-1 only needs a 16-wide
  segment-sum of w1; W_lsh[1] output is dead; V1 is only needed at
  component 0 => contraction with Ytil = Y * W_lsh[0][:,0].
- Receiver scatter: node id = hi*128+lo; per edge-tile matmul with lo
  one-hot lhsT and (hi one-hot * edge_out) rhs accumulates [128,64]
  partials in PSUM; host sums the 8 per-core partials (the unshard).
- 1/sqrt(AVG_NEIGH) and the 1/sqrt(2) residual scales are folded into
  weights on the host.
"""
import math
import sys

import numpy as np

sys.path.insert(0, "/opt/trn_rl_repo")

import ml_dtypes  # noqa: E402

BF16 = ml_dtypes.bfloat16
SIM_SILU = False   # CoreSim lacks Silu; emulate with Sigmoid*z when set

N, E, MUL, H, F = 8192, 131072, 16, 256, 16
NB = 8
INV = 1.0 / math.sqrt(16.0)
NC = 8
NPC = N // NC          # nodes per core
WIN = 128
NW = NPC // WIN        # windows per core
RWIN = N // WIN        # 64 receiver windows
SQ = math.sqrt(0.5)

# ---- bf16 weight-blob column layout [128, CB] ----
OFF_NAT = 0                      # natts [128, 16*64]  cols = f*64 + hi
OFF_SNAT = OFF_NAT + 1024        # snat  [128, 8*16]   cols = w*16 + f
OFF_WE0 = OFF_SNAT + 128         # we0 [40, 256] (rows 0..39)
OFF_WE1 = OFF_WE0 + 256          # we1 2 x [128, 256]
OFF_WV0 = OFF_WE1 + 512          # wv0 2 x [128, 16]
OFF_WLW0 = OFF_WV0 + 32
OFF_WLW1 = OFF_WLW0 + 32
OFF_WLY1 = (OFF_WLW1 + 32, OFF_WLW1 + 32 + 768)
#   per layer: main 2 x [128, 256] (512 cols) then fb [16, 256] (256 cols)
OFF_WLY2 = (OFF_WLY1[1] + 768, OFF_WLY1[1] + 768 + 512)
OFF_WOUT = OFF_WLY2[1] + 512     # wout 2 x [128, 1]
CB = OFF_WOUT + 2


def _host_shard(vectors, senders, receivers):
    """Group edges by (core, sender-window); pad to common K_WIN."""
    core = senders // NPC
    win = (senders % NPC) // WIN
    key = core * NW + win
    order = np.argsort(key, kind="stable")
    counts = np.bincount(key, minlength=NC * NW)
    kwin = int(((counts.max() + 127) // 128) * 128)
    starts = np.zeros(NC * NW + 1, np.int64)
    np.cumsum(counts, out=starts[1:])

    EP = NW * kwin
    shards = []
    for c in range(NC):
        vec = np.zeros((EP, 3), np.float32)
        vec[:, 0] = 2.0
        sl = np.zeros(EP, np.uint8)    # sender local-in-window
        rlo = np.zeros(EP, np.uint8)
        rhi = np.zeros(EP, np.uint8)
        for w in range(NW):
            g = c * NW + w
            eid = order[starts[g]:starts[g + 1]]
            o = w * kwin
            n_e = len(eid)
            vec[o:o + n_e] = vectors[eid]
            sl[o:o + n_e] = (senders[eid] - (c * NPC + w * WIN)).astype(np.uint8)
            rlo[o:o + n_e] = (receivers[eid] % 128).astype(np.uint8)
            rhi[o:o + n_e] = (receivers[eid] // 128).astype(np.uint8)
        shards.append((vec, sl, rlo, rhi))
    return kwin, shards


def _plane(a, T_ALL):
    """[EP] or [EP, k] -> plane layout [128, T_ALL*(k)] with e = t*128+p."""
    if a.ndim == 1:
        return np.ascontiguousarray(a.reshape(T_ALL, 128).T)
    # [EP, k] -> [128, k*T_ALL] with component-major column groups
    k = a.shape[1]
    p = a.reshape(T_ALL, 128, k).transpose(2, 1, 0)     # [k, 128, T_ALL]
    return np.ascontiguousarray(p.reshape(k * 128, T_ALL)).reshape(k, 128, T_ALL)


def _prep_weights(i):
    """Fold INV and residual 1/sqrt(2) scales into weights (f32)."""
    w = {}
    w["we0"] = i["W_e0"]                                       # [40,256]
    w["we1"] = i["W_e1"]
    w["wv0"] = i["W_v0"]
    w["wlw0"] = i["W_lw"][0] * INV
    w["wlw1"] = i["W_lw"][1] * INV * SQ                        # x1 = sq*x1'
    wly1_1 = i["W_ly1"][1].copy()
    wly1_1[:H] *= SQ                                           # x rows scaled
    w["wly1_0"] = i["W_ly1"][0]
    w["wly1_1"] = wly1_1
    w["wly2_0"] = i["W_ly2"][0]
    w["wly2_1"] = i["W_ly2"][1]
    w["wout"] = i["W_out"] * INV * 0.5                         # x2 = .5*x2'
    return w


def _pack_blobw(i, c):
    """Per-core bf16 blob [128, CB]: node table + folded weights."""
    w = _prep_weights(i)
    na = i["node_attrs"]                                       # [N, F]
    blob = np.zeros((128, CB), np.float32)
    # natts[lo, f*64+hi] = na[hi*128+lo, f]
    nat = na.reshape(RWIN, 128, F).transpose(1, 2, 0)          # [lo, f, hi]
    blob[:, OFF_NAT:OFF_NAT + 1024] = nat.reshape(128, F * RWIN)
    # snat[lo, w*16+f] = na[(c*8+w)*128+lo, f]
    sn = na.reshape(RWIN, 128, F)[c * NW:(c + 1) * NW]         # [w, lo, f]
    blob[:, OFF_SNAT:OFF_SNAT + 128] = sn.transpose(1, 0, 2).reshape(128, 128)
    # rhs row order is [snd attrs(16), rcv attrs(16), bessel(8)] so the
    # on-device copies land on legal partition offsets (0 and 32)
    blob[0:40, OFF_WE0:OFF_WE0 + 256] = np.vstack([w["we0"][8:40],
                                                   w["we0"][0:8]])
    for kc in range(2):
        s = slice(kc * 128, (kc + 1) * 128)
        blob[:, OFF_WE1 + kc * 256:OFF_WE1 + (kc + 1) * 256] = w["we1"][s]
        blob[:, OFF_WV0 + kc * 16:OFF_WV0 + (kc + 1) * 16] = w["wv0"][s]
        blob[:, OFF_WLW0 + kc * 16:OFF_WLW0 + (kc + 1) * 16] = w["wlw0"][s]
        blob[:, OFF_WLW1 + kc * 16:OFF_WLW1 + (kc + 1) * 16] = w["wlw1"][s]
        blob[:, OFF_WOUT + kc:OFF_WOUT + kc + 1] = w["wout"][s]
    for l in range(2):
        m = w[f"wly1_{l}"]
        for kc in range(2):
            s = slice(kc * 128, (kc + 1) * 128)
            blob[:, OFF_WLY1[l] + kc * 256:OFF_WLY1[l] + (kc + 1) * 256] = m[s]
            blob[:, OFF_WLY2[l] + kc * 256:OFF_WLY2[l] + (kc + 1) * 256] = \
                w[f"wly2_{l}"][s]
        blob[0:16, OFF_WLY1[l] + 512:OFF_WLY1[l] + 768] = m[256:272]
    return blob.astype(BF16)


def make_in_maps(inputs):
    kwin, shards = _host_shard(inputs["vectors"], inputs["senders"],
                               inputs["receivers"])
    EP = NW * kwin
    T_ALL = EP // 128
    CF = 3 * T_ALL + 28
    bias_list = [inputs["b_e0"], inputs["b_e1"],
                 inputs["b_ly1"][0], inputs["b_ly1"][1],
                 inputs["b_ly2"][0], inputs["b_ly2"][1]]
    wcol = inputs["W_lsh"][0][:, 0]                            # [16]
    blobw0 = _pack_blobw(inputs, 0)      # core-dependent only in SNAT
    in_maps = []
    dbg = []
    for c in range(NC):
        vec, sl, rlo, rhi = shards[c]
        b8 = np.empty((128, 3 * T_ALL), np.uint8)
        b8[:, 0:T_ALL] = _plane(sl, T_ALL)
        b8[:, T_ALL:2 * T_ALL] = _plane(rlo, T_ALL)
        b8[:, 2 * T_ALL:3 * T_ALL] = _plane(rhi, T_ALL)
        bf = np.zeros((128, CF), np.float32)
        vp = _plane(vec, T_ALL)                                # [3,128,T]
        bf[:, 0:T_ALL] = vp[0]
        bf[:, T_ALL:2 * T_ALL] = vp[1]
        bf[:, 2 * T_ALL:3 * T_ALL] = vp[2]
        for i, b in enumerate(bias_list):
            bf[:, 3 * T_ALL + 2 * i] = b[0:128]
            bf[:, 3 * T_ALL + 2 * i + 1] = b[128:256]
        bf[:, 3 * T_ALL + 12:3 * T_ALL + 28] = np.tile(
            wcol.reshape(1, 16), (128, 1))
        bw = _pack_blobw(inputs, c) if c else blobw0
        in_maps.append({"blob8": b8, "blobf": np.ascontiguousarray(bf),
                        "blobw": bw})
        dbg.append(dict(vec=vec, sl=sl, rlo=rlo, rhi=rhi))
    return kwin, in_maps, dbg


_CAP_SKIP = {"InstEventSemaphore", "InstBranch", "InstNop",
             "InstCollectiveCompute"}
_CAP_LIMITS = {}


def _split_waits(nc, mybir, mk_carrier, limit=1):
    """Walrus codegen allows only 1 embedded sem-wait on compute
    instructions.  For each instruction with more, strip the extras onto
    freshly created same-engine carrier instructions inserted directly
    before it (engines are in-order, so this preserves semantics)."""
    f = nc.m.functions[0]
    made = 0
    for bb in f.blocks:
        insts = list(bb.instructions)
        plan = []          # (index, [carrier insts])
        for i, inst in enumerate(insts):
            tname = type(inst).__name__
            si = inst.sync_info
            nwait = len(si.on_wait) if (si and si.on_wait) else 0
            lim = _CAP_LIMITS.get(tname, limit)
            if tname in _CAP_SKIP or nwait <= lim:
                continue
            waits = list(si.on_wait)
            extras, keep = waits[:-lim], waits[-lim:]
            carriers = []
            for wt in extras:
                ci = mk_carrier(inst.engine)
                if ci is None:
                    keep.insert(0, wt)
                    continue
                ci.sync_info = mybir.SyncInfo(on_wait=[wt], on_update=[])
                carriers.append(ci)
                made += 1
            inst.sync_info = mybir.SyncInfo(on_wait=keep,
                                            on_update=si.on_update)
            if carriers:
                plan.append((i, carriers))
        if plan:
            new = []
            pmap = dict(plan)
            for i, inst in enumerate(insts):
                if i in pmap:
                    new.extend(pmap[i])
                new.append(inst)
            bb.instructions = new
    return made


def build_graph(kwin):
    from concourse import bass, mybir
    from concourse.masks import make_identity
    from concourse.tile import TileContext

    EP = NW * kwin
    T_ALL = EP // 128
    T_W = kwin // 128
    NCH = (kwin + 511) // 512      # free chunks per window
    CF = 3 * T_ALL + 28

    f32 = mybir.dt.float32
    bf16 = mybir.dt.bfloat16
    i32 = mybir.dt.int32
    u8 = mybir.dt.uint8
    AX = mybir.AxisListType.X
    OP = mybir.AluOpType
    AF = mybir.ActivationFunctionType

    nc = bass.Bass()
    carrier_sem_cm = nc.semaphore("carrier_sem")
    carrier_sem = carrier_sem_cm.__enter__()
    dp = nc.declare_dram_parameter
    d_b8 = dp("blob8", [128, 3 * T_ALL], u8, isOutput=False)
    d_bf = dp("blobf", [128, CF], f32, isOutput=False)
    d_bw = dp("blobw", [128, CB], bf16, isOutput=False)
    d_out = dp("out", [128, RWIN], f32, isOutput=True)

    with TileContext(nc) as tc:
        with (
            tc.tile_pool(name="glob", bufs=1) as gp,
            tc.tile_pool(name="wgt", bufs=1) as wp,
            tc.tile_pool(name="win", bufs=2) as wnp,
            tc.tile_pool(name="big", bufs=1) as bgp,
            tc.tile_pool(name="sml", bufs=3) as sp,
            tc.tile_pool(name="ps_mlp", bufs=2, space="PSUM") as pmlp,
            tc.tile_pool(name="ps_acc", bufs=1, space="PSUM") as pacc,
            tc.tile_pool(name="ps_gth", bufs=1, space="PSUM") as pgth,
            tc.tile_pool(name="ps_sml", bufs=1, space="PSUM") as psml,
            tc.tile_pool(name="ps_rcv", bufs=1, space="PSUM") as prcv,
        ):
            # ---------------- blobs to SBUF ----------------
            wb = wp.tile([128, CB], bf16, tag="wb")
            nc.sync.dma_start(out=wb[:], in_=d_bw[:])
            fbt = wp.tile([128, CF], f32, tag="fbt")
            nc.sync.dma_start(out=fbt[:], in_=d_bf[:])
            i8t = wp.tile([128, 3 * T_ALL], u8, tag="i8t")
            nc.sync.dma_start(out=i8t[:], in_=d_b8[:])
            slf = wp.tile([128, T_ALL], f32, tag="slf")
            rlof = wp.tile([128, T_ALL], f32, tag="rlof")
            rhif = wp.tile([128, T_ALL], f32, tag="rhif")
            nc.vector.tensor_copy(out=slf[:], in_=i8t[:, 0:T_ALL])
            nc.vector.tensor_copy(out=rlof[:], in_=i8t[:, T_ALL:2 * T_ALL])
            nc.vector.tensor_copy(out=rhif[:], in_=i8t[:, 2 * T_ALL:3 * T_ALL])

            ident = wp.tile([128, 128], f32, tag="ident")
            make_identity(nc, ident[:])
            identb = wp.tile([128, 128], bf16, tag="identb")
            make_identity(nc, identb[:])
            it32 = wp.tile([128, 128], i32, tag="it32")
            nc.gpsimd.iota(out=it32[:], pattern=[[1, 128]], base=0,
                           channel_multiplier=0)
            iof = wp.tile([128, 128], f32, tag="iof")
            nc.vector.tensor_copy(out=iof[:], in_=it32[:])
            ones_bf = wp.tile([1, 128], bf16, tag="ones")
            nc.gpsimd.memset(ones_bf[:], 1.0)

            # views into the blobs
            vx = fbt[:, 0:T_ALL]
            vy = fbt[:, T_ALL:2 * T_ALL]
            vz = fbt[:, 2 * T_ALL:3 * T_ALL]
            BIA = 3 * T_ALL

            def bias(i, hc):
                return fbt[:, BIA + 2 * i + hc, None]
            wcol = fbt[:, BIA + 12:BIA + 28]
            natv = wb[:, OFF_NAT:OFF_NAT + 1024]

            # ---------------- edge-scalar stage (planes [128,T_ALL]) ----
            ta = gp.tile([128, T_ALL], f32)
            tb = gp.tile([128, T_ALL], f32)
            ts = nc.vector.tensor_scalar
            act = nc.scalar.activation

            def silu_act(out, ps_in, bias_ap):
                if not SIM_SILU:
                    act(out=out, in_=ps_in, func=AF.Silu, bias=bias_ap)
                else:
                    pp = ps_in.shape[0]
                    sg = bgp.tile([128, 512], f32, tag="simsilu")
                    zz_ = bgp.tile([128, 512], f32, tag="simsilu2")
                    cw_ = ps_in.shape[-1]
                    act(out=sg[:pp, :cw_], in_=ps_in, func=AF.Sigmoid,
                        bias=bias_ap)
                    nc.vector.tensor_scalar(out=zz_[:pp, :cw_], in0=ps_in,
                                            scalar1=bias_ap, scalar2=None,
                                            op0=OP.add)
                    nc.vector.tensor_mul(out=out, in0=sg[:pp, :cw_],
                                         in1=zz_[:pp, :cw_])
            d_pl = gp.tile([128, T_ALL], f32)
            nc.vector.tensor_mul(out=ta[:], in0=vx, in1=vx)
            nc.vector.tensor_mul(out=tb[:], in0=vy, in1=vy)
            nc.vector.tensor_add(out=ta[:], in0=ta[:], in1=tb[:])
            nc.vector.tensor_mul(out=tb[:], in0=vz, in1=vz)
            nc.vector.tensor_add(out=ta[:], in0=ta[:], in1=tb[:])
            act(out=d_pl[:], in_=ta[:], func=AF.Sqrt)
            rinv = gp.tile([128, T_ALL], f32)
            nc.vector.reciprocal(out=rinv[:], in_=d_pl[:])
            ux = gp.tile([128, T_ALL], f32)
            uy = gp.tile([128, T_ALL], f32)
            uz = gp.tile([128, T_ALL], f32)
            nc.vector.tensor_mul(out=ux[:], in0=vx, in1=rinv[:])
            nc.vector.tensor_mul(out=uy[:], in0=vy, in1=rinv[:])
            nc.vector.tensor_mul(out=uz[:], in0=vz, in1=rinv[:])

            # envelope u = 1 + d^6*(-28 + 48d - 21d^2), zero for d >= 1
            u_pl = gp.tile([128, T_ALL], f32)
            nc.vector.tensor_mul(out=ta[:], in0=d_pl[:], in1=d_pl[:])   # d2
            nc.vector.tensor_mul(out=tb[:], in0=ta[:], in1=d_pl[:])     # d3
            nc.vector.tensor_mul(out=tb[:], in0=tb[:], in1=tb[:])       # d6
            ts(out=ta[:], in0=ta[:], scalar1=-21.0, scalar2=None, op0=OP.mult)
            tc_q = gp.tile([128, T_ALL], f32)
            ts(out=tc_q[:], in0=d_pl[:], scalar1=48.0, scalar2=-28.0,
               op0=OP.mult, op1=OP.add)
            nc.vector.tensor_add(out=ta[:], in0=ta[:], in1=tc_q[:])
            nc.vector.tensor_mul(out=tb[:], in0=tb[:], in1=ta[:])
            ts(out=tb[:], in0=tb[:], scalar1=1.0, scalar2=None, op0=OP.add)
            ts(out=ta[:], in0=d_pl[:], scalar1=1.0, scalar2=None,
               op0=OP.is_lt)
            nc.vector.tensor_mul(out=u_pl[:], in0=tb[:], in1=ta[:])

            # spherical harmonics Y [128, T_ALL, 16] f32
            Yt = gp.tile([128, T_ALL, 16], f32)
            s3 = 3.0 ** 0.5; s5 = 5.0 ** 0.5; s15 = 15.0 ** 0.5
            s7 = 7.0 ** 0.5
            c33 = (35.0 / 8.0) ** 0.5; c32 = 105.0 ** 0.5
            c31 = (21.0 / 8.0) ** 0.5
            xx = gp.tile([128, T_ALL], f32)
            yy = gp.tile([128, T_ALL], f32)
            zz = gp.tile([128, T_ALL], f32)
            xy = gp.tile([128, T_ALL], f32)
            nc.vector.tensor_mul(out=xx[:], in0=ux[:], in1=ux[:])
            nc.vector.tensor_mul(out=yy[:], in0=uy[:], in1=uy[:])
            nc.vector.tensor_mul(out=zz[:], in0=uz[:], in1=uz[:])
            nc.vector.tensor_mul(out=xy[:], in0=ux[:], in1=uy[:])
            ts(out=Yt[:, :, 0], in0=ux[:], scalar1=0.0, scalar2=1.0,
               op0=OP.mult, op1=OP.add)
            ts(out=Yt[:, :, 1], in0=ux[:], scalar1=s3, scalar2=None,
               op0=OP.mult)
            ts(out=Yt[:, :, 2], in0=uy[:], scalar1=s3, scalar2=None,
               op0=OP.mult)
            ts(out=Yt[:, :, 3], in0=uz[:], scalar1=s3, scalar2=None,
               op0=OP.mult)
            ts(out=Yt[:, :, 4], in0=xy[:], scalar1=s15, scalar2=None,
               op0=OP.mult)
            nc.vector.tensor_mul(out=ta[:], in0=uy[:], in1=uz[:])
            ts(out=Yt[:, :, 5], in0=ta[:], scalar1=s15, scalar2=None,
               op0=OP.mult)
            ts(out=Yt[:, :, 6], in0=zz[:], scalar1=1.5 * s5,
               scalar2=-0.5 * s5, op0=OP.mult, op1=OP.add)
            nc.vector.tensor_mul(out=tb[:], in0=ux[:], in1=uz[:])
            ts(out=Yt[:, :, 7], in0=tb[:], scalar1=s15, scalar2=None,
               op0=OP.mult)
            xmy = gp.tile([128, T_ALL], f32)
            nc.vector.tensor_sub(out=xmy[:], in0=xx[:], in1=yy[:])
            ts(out=Yt[:, :, 8], in0=xmy[:], scalar1=0.5 * s15, scalar2=None,
               op0=OP.mult)
            ts(out=ta[:], in0=xx[:], scalar1=3.0, scalar2=None, op0=OP.mult)
            nc.vector.tensor_sub(out=ta[:], in0=ta[:], in1=yy[:])
            nc.vector.tensor_mul(out=ta[:], in0=ta[:], in1=uy[:])
            ts(out=Yt[:, :, 9], in0=ta[:], scalar1=c33, scalar2=None,
               op0=OP.mult)
            nc.vector.tensor_mul(out=ta[:], in0=xy[:], in1=uz[:])
            ts(out=Yt[:, :, 10], in0=ta[:], scalar1=c32, scalar2=None,
               op0=OP.mult)
            ts(out=ta[:], in0=zz[:], scalar1=5.0, scalar2=-1.0,
               op0=OP.mult, op1=OP.add)
            nc.vector.tensor_mul(out=tb[:], in0=ta[:], in1=uy[:])
            ts(out=Yt[:, :, 11], in0=tb[:], scalar1=c31, scalar2=None,
               op0=OP.mult)
            nc.vector.tensor_mul(out=tb[:], in0=ta[:], in1=ux[:])
            ts(out=Yt[:, :, 13], in0=tb[:], scalar1=c31, scalar2=None,
               op0=OP.mult)
            nc.vector.tensor_mul(out=ta[:], in0=zz[:], in1=uz[:])
            ts(out=ta[:], in0=ta[:], scalar1=2.5 * s7, scalar2=None,
               op0=OP.mult)
            ts(out=tb[:], in0=uz[:], scalar1=1.5 * s7, scalar2=None,
               op0=OP.mult)
            nc.vector.tensor_sub(out=Yt[:, :, 12], in0=ta[:], in1=tb[:])
            nc.vector.tensor_mul(out=ta[:], in0=xmy[:], in1=uz[:])
            ts(out=Yt[:, :, 14], in0=ta[:], scalar1=0.5 * c32, scalar2=None,
               op0=OP.mult)
            ts(out=ta[:], in0=yy[:], scalar1=3.0, scalar2=None, op0=OP.mult)
            nc.vector.tensor_sub(out=ta[:], in0=xx[:], in1=ta[:])
            nc.vector.tensor_mul(out=ta[:], in0=ta[:], in1=ux[:])
            ts(out=Yt[:, :, 15], in0=ta[:], scalar1=c33, scalar2=None,
               op0=OP.mult)

            # bessel (range-reduced): besu [128, T_ALL, 8]
            besu = gp.tile([128, T_ALL, 8], f32)
            rs = gp.tile([128, T_ALL], f32)
            ts(out=rs[:], in0=rinv[:], scalar1=math.sqrt(2.0), scalar2=None,
               op0=OP.mult)
            mi = gp.tile([128, T_ALL], mybir.dt.int32)
            for k in range(1, NB + 1):
                ts(out=ta[:], in0=d_pl[:], scalar1=0.5 * k, scalar2=None,
                   op0=OP.mult)
                nc.vector.tensor_copy(out=mi[:], in_=ta[:])
                nc.vector.tensor_copy(out=tb[:], in_=mi[:])
                nc.vector.tensor_sub(out=ta[:], in0=ta[:], in1=tb[:])
                # ta = frac in (-0.5, 1) whether the cast rounds or truncates
                ts(out=tb[:], in0=ta[:], scalar1=0.5, scalar2=None,
                   op0=OP.is_gt)
                nc.vector.tensor_sub(out=ta[:], in0=ta[:], in1=tb[:])
                act(out=ta[:], in_=ta[:], func=AF.Sin, scale=2.0 * math.pi)
                nc.vector.tensor_mul(out=besu[:, :, k - 1], in0=ta[:],
                                      in1=rs[:])

            # ---------------- persistent receiver accumulator ----------
            ps_rcv = prcv.tile([128, RWIN], f32, space="PSUM")

            # ---------------- window loop ----------------
            for w in range(NW):
                t0 = w * T_W
                ohs = wnp.tile([128, T_W, 128], bf16)   # [e, n]
                ohg = wnp.tile([128, T_W, 128], bf16)   # [n, e]
                rqs = wnp.tile([128, T_W, 128], bf16)   # [e, lo]
                rqg = wnp.tile([128, T_W, 128], bf16)   # [lo, e]
                rwt = wnp.tile([128, T_W, RWIN], bf16)  # [e, hi]
                xfm = wnp.tile([40, kwin], bf16)        # snd(16)+rcv(16)+bes(8)
                ufm = wnp.tile([1, kwin], bf16)
                tt = nc.vector.tensor_tensor
                for t in range(T_W):
                    tg = t0 + t
                    csl = slice(t * 128, (t + 1) * 128)
                    tt(out=ohs[:, t, :],
                       in0=slf[:, tg, None].to_broadcast([128, 128]),
                       in1=iof[:], op=OP.is_equal)
                    tt(out=rqs[:, t, :],
                       in0=rlof[:, tg, None].to_broadcast([128, 128]),
                       in1=iof[:], op=OP.is_equal)
                    tt(out=rwt[:, t, :],
                       in0=rhif[:, tg, None].to_broadcast([128, RWIN]),
                       in1=iof[:, 0:RWIN], op=OP.is_equal)
                    ptr = psml.tile([128, 128], bf16, space="PSUM", tag="trn")
                    nc.tensor.transpose(out=ptr[:], in_=ohs[:, t, :],
                                        identity=identb[:])
                    nc.vector.tensor_copy(out=ohg[:, t, :], in_=ptr[:])
                    ptr2 = psml.tile([128, 128], bf16, space="PSUM", tag="trn")
                    nc.tensor.transpose(out=ptr2[:], in_=rqs[:, t, :],
                                        identity=identb[:])
                    nc.vector.tensor_copy(out=rqg[:, t, :], in_=ptr2[:])
                    # bessel + u feature-major
                    pst = psml.tile([32, 128], f32, space="PSUM", tag="sml")
                    nc.tensor.transpose(out=pst[0:8, :], in_=besu[:, tg, :],
                                        identity=ident[:])
                    nc.vector.tensor_copy(out=xfm[32:40, csl],
                                          in_=pst[0:8, :])
                    psu1 = psml.tile([32, 128], f32, space="PSUM", tag="sml")
                    nc.tensor.transpose(out=psu1[0:1, :],
                                        in_=u_pl[:, tg, None],
                                        identity=ident[:])
                    nc.vector.tensor_copy(out=ufm[:, csl], in_=psu1[0:1, :])
                    # endpoint-attr gather: sender (window-local one-hot)
                    gcmb = sp.tile([128, 32], f32, tag="gcmb")
                    psn = psml.tile([128, 32], f32, space="PSUM", tag="sm2")
                    nc.tensor.matmul(
                        out=psn[:, 0:16], lhsT=ohg[:, t, :],
                        rhs=wb[:, OFF_SNAT + w * 16:OFF_SNAT + (w + 1) * 16],
                        start=True, stop=True)
                    nc.vector.tensor_copy(out=gcmb[:, 0:16], in_=psn[:, 0:16])
                    # receiver: lo-gather matmul then hi-select
                    for c2 in range(2):
                        prg = pgth.tile([128, 512], f32, space="PSUM",
                                        tag="gth")
                        nc.tensor.matmul(
                            out=prg[:], lhsT=rqg[:, t, :],
                            rhs=natv[:, c2 * 512:(c2 + 1) * 512],
                            start=True, stop=True)
                        prod = sp.tile([128, 8, RWIN], f32, tag="rsel")
                        nc.vector.tensor_mul(
                            out=prod[:],
                            in0=prg[:].rearrange("p (a b) -> p a b", b=RWIN),
                            in1=rwt[:, t, None, :].to_broadcast(
                                [128, 8, RWIN]))
                        nc.vector.reduce_sum(
                            out=gcmb[:, 16 + c2 * 8:16 + (c2 + 1) * 8, None],
                            in_=prod[:], axis=AX)
                    ptg = psml.tile([32, 128], f32, space="PSUM", tag="sml")
                    nc.tensor.transpose(out=ptg[:], in_=gcmb[:],
                                        identity=ident[:])
                    nc.vector.tensor_copy(out=xfm[0:32, csl], in_=ptg[:])

                # broadcast u row -> [128, kwin] bf16
                ubc = bgp.tile([128, kwin], bf16)
                for ch in range(NCH):
                    c0 = ch * 512
                    c1 = min(kwin, c0 + 512)
                    psu = pmlp.tile([128, 512], f32, space="PSUM", tag="mlp")
                    nc.tensor.matmul(out=psu[:, :c1 - c0], lhsT=ones_bf[:],
                                     rhs=ufm[:, c0:c1],
                                     start=True, stop=True)
                    nc.vector.tensor_copy(out=ubc[:, c0:c1],
                                          in_=psu[:, :c1 - c0])

                # ---- edge MLP: x0 = u*silu(e1(silu(e0(bes,attrs)))) ----
                x0 = bgp.tile([128, 2, kwin], bf16)
                th = bgp.tile([128, 2, kwin], bf16)
                for ch in range(NCH):
                    c0 = ch * 512
                    c1 = min(kwin, c0 + 512)
                    cw = c1 - c0
                    for hc in range(2):
                        ps = pmlp.tile([128, 512], f32, space="PSUM", tag="mlp")
                        nc.tensor.matmul(
                            out=ps[:, :cw],
                            lhsT=wb[0:40, OFF_WE0 + hc * 128:
                                    OFF_WE0 + (hc + 1) * 128],
                            rhs=xfm[:, c0:c1], start=True, stop=True)
                        silu_act(th[:, hc, c0:c1], ps[:, :cw], bias(0, hc))
                for ch in range(NCH):
                    c0 = ch * 512
                    c1 = min(kwin, c0 + 512)
                    cw = c1 - c0
                    for hc in range(2):
                        ps = pmlp.tile([128, 512], f32, space="PSUM", tag="mlp")
                        for kc in range(2):
                            nc.tensor.matmul(
                                out=ps[:, :cw],
                                lhsT=wb[:, OFF_WE1 + kc * 256 + hc * 128:
                                        OFF_WE1 + kc * 256 + (hc + 1) * 128],
                                rhs=th[:, kc, c0:c1],
                                start=(kc == 0), stop=(kc == 1))
                        silu_act(x0[:, hc, c0:c1], ps[:, :cw], bias(1, hc))
                for hc in range(2):
                    nc.vector.tensor_mul(out=x0[:, hc, :], in0=x0[:, hc, :],
                                          in1=ubc[:])

                # ---- xv, w0 (edge-major [128,16] per tile) ----
                xv = wnp.tile([128, T_W, MUL], f32)
                w0 = wnp.tile([128, T_W, MUL], bf16)
                for t in range(T_W):
                    tsl = slice(t * 128, (t + 1) * 128)
                    p12 = psml.tile([128, 32], f32, space="PSUM", tag="sm2")
                    for kc in range(2):
                        nc.tensor.matmul(
                            out=p12[:, 0:16], lhsT=x0[:, kc, tsl],
                            rhs=wb[:, OFF_WV0 + kc * 16:OFF_WV0 + (kc + 1) * 16],
                            start=(kc == 0), stop=(kc == 1))
                    for kc in range(2):
                        nc.tensor.matmul(
                            out=p12[:, 16:32], lhsT=x0[:, kc, tsl],
                            rhs=wb[:, OFF_WLW0 + kc * 16:
                                    OFF_WLW0 + (kc + 1) * 16],
                            start=(kc == 0), stop=(kc == 1))
                    nc.vector.tensor_copy(out=xv[:, t, :], in_=p12[:, 0:16])
                    nc.vector.tensor_copy(out=w0[:, t, :], in_=p12[:, 16:32])

                # ---- layer-0 scatter: wY[n, m*16+i] ----
                ps_acc = pacc.tile([128, 256], f32, space="PSUM", tag="acc")
                for t in range(T_W):
                    v2 = sp.tile([128, MUL, 16], bf16, tag="v2")
                    nc.vector.tensor_mul(
                        out=v2[:],
                        in0=w0[:, t, :, None].to_broadcast([128, MUL, 16]),
                        in1=Yt[:, t0 + t, None, :].to_broadcast(
                            [128, MUL, 16]))
                    nc.tensor.matmul(
                        out=ps_acc[:],
                        lhsT=ohs[:, t, :],
                        rhs=v2[:].rearrange("p a b -> p (a b)"),
                        start=(t == 0), stop=(t == T_W - 1))
                wY = wnp.tile([128, 256], bf16)
                nc.vector.tensor_copy(out=wY[:], in_=ps_acc[:])

                # ---- gather + Ytil contraction + feedback ----
                V10 = wnp.tile([128, T_W, MUL], f32)
                fbfm = wnp.tile([MUL, kwin], bf16)
                prod = wnp.tile([128, MUL, 16], f32)
                ytil = wnp.tile([128, MUL], f32)
                Ssb = wnp.tile([128, MUL], f32)
                fb = wnp.tile([128, MUL], f32)
                for t in range(T_W):
                    pgf = pgth.tile([128, 512], f32, space="PSUM", tag="gth")
                    pg = pgf[:, 0:256]
                    nc.tensor.matmul(out=pg, lhsT=ohg[:, t, :], rhs=wY[:],
                                     start=True, stop=True)
                    pg3 = pg.rearrange("p (a b) -> p a b", b=16)
                    nc.vector.tensor_mul(out=ytil[:], in0=Yt[:, t0 + t, :],
                                          in1=wcol)
                    nc.vector.tensor_mul(
                        out=prod[:], in0=pg3,
                        in1=ytil[:, None, :].to_broadcast([128, MUL, 16]))
                    nc.vector.reduce_sum(out=Ssb[:, :, None], in_=prod[:],
                                         axis=AX)
                    nc.vector.tensor_mul(out=V10[:, t, :], in0=Ssb[:],
                                          in1=xv[:, t, :])
                    nc.vector.tensor_mul(out=fb[:], in0=pg3[:, :, 0],
                                          in1=xv[:, t, :])
                    pst = psml.tile([32, 128], f32, space="PSUM", tag="sml")
                    nc.tensor.transpose(out=pst[0:16, :], in_=fb[:],
                                        identity=ident[:])
                    nc.vector.tensor_copy(out=fbfm[:, t * 128:(t + 1) * 128],
                                          in_=pst[0:16, :])

                # ---- layer-0 ly1/ly2 + residual -> x1 ----
                x1 = bgp.tile([128, 2, kwin], bf16)

                def mlp_block(xin, xout, l, fbrow, resid_sq2):
                    b1 = OFF_WLY1[l]
                    bf_ = OFF_WLY1[l] + 512
                    b2 = OFF_WLY2[l]
                    ty = bgp.tile([128, 2, kwin], bf16)
                    for ch in range(NCH):
                        c0 = ch * 512
                        c1 = min(kwin, c0 + 512)
                        cw = c1 - c0
                        for hc in range(2):
                            hs = slice(hc * 128, (hc + 1) * 128)
                            ps = pmlp.tile([128, 512], f32, space="PSUM",
                                           tag="mlp")
                            for kc in range(2):
                                nc.tensor.matmul(
                                    out=ps[:, :cw],
                                    lhsT=wb[:, b1 + kc * 256 + hc * 128:
                                            b1 + kc * 256 + (hc + 1) * 128],
                                    rhs=xin[:, kc, c0:c1],
                                    start=(kc == 0), stop=False)
                            nc.tensor.matmul(
                                out=ps[:, :cw],
                                lhsT=wb[0:16, bf_ + hc * 128:
                                        bf_ + (hc + 1) * 128],
                                rhs=fbrow[:, c0:c1],
                                start=False, stop=True)
                            silu_act(ty[:, hc, c0:c1], ps[:, :cw],
                                     bias(2 + l, hc))
                    ty2 = bgp.tile([128, 2, kwin], bf16)
                    for ch in range(NCH):
                        c0 = ch * 512
                        c1 = min(kwin, c0 + 512)
                        cw = c1 - c0
                        for hc in range(2):
                            ps = pmlp.tile([128, 512], f32, space="PSUM",
                                           tag="mlp")
                            for kc in range(2):
                                nc.tensor.matmul(
                                    out=ps[:, :cw],
                                    lhsT=wb[:, b2 + kc * 256 + hc * 128:
                                            b2 + kc * 256 + (hc + 1) * 128],
                                    rhs=ty[:, kc, c0:c1],
                                    start=(kc == 0), stop=(kc == 1))
                            silu_act(ty2[:, hc, c0:c1], ps[:, :cw],
                                     bias(4 + l, hc))
                    # x_out' = x_in' + s * u * y   (s = 1 or sqrt(2))
                    for hc in range(2):
                        nc.vector.tensor_mul(out=ty2[:, hc, :],
                                              in0=ty2[:, hc, :], in1=ubc[:])
                        if resid_sq2:
                            ts(out=ty2[:, hc, :], in0=ty2[:, hc, :],
                               scalar1=math.sqrt(2.0), scalar2=None,
                               op0=OP.mult)
                        nc.vector.tensor_add(out=xout[:, hc, :],
                                             in0=xin[:, hc, :],
                                             in1=ty2[:, hc, :])

                mlp_block(x0, x1, 0, fbfm, False)

                # ---- layer 1: w1, 16-wide scatter/gather, feedback ----
                w1 = wnp.tile([128, T_W, MUL], bf16)
                for t in range(T_W):
                    tsl = slice(t * 128, (t + 1) * 128)
                    p1 = psml.tile([128, 32], f32, space="PSUM", tag="sm2")
                    for kc in range(2):
                        nc.tensor.matmul(
                            out=p1[:, 0:MUL], lhsT=x1[:, kc, tsl],
                            rhs=wb[:, OFF_WLW1 + kc * 16:
                                    OFF_WLW1 + (kc + 1) * 16],
                            start=(kc == 0), stop=(kc == 1))
                    nc.vector.tensor_copy(out=w1[:, t, :], in_=p1[:, 0:MUL])
                ps_a1 = pacc.tile([128, 256], f32, space="PSUM", tag="acc")
                for t in range(T_W):
                    nc.tensor.matmul(out=ps_a1[:, 0:MUL], lhsT=ohs[:, t, :],
                                     rhs=w1[:, t, :],
                                     start=(t == 0), stop=(t == T_W - 1))
                wY1 = wnp.tile([128, MUL], bf16)
                nc.vector.tensor_copy(out=wY1[:], in_=ps_a1[:, 0:MUL])
                fbfm1 = wnp.tile([MUL, kwin], bf16)
                fb1 = wnp.tile([128, MUL], f32)
                for t in range(T_W):
                    pg = pgth.tile([128, 512], f32, space="PSUM", tag="gth")
                    nc.tensor.matmul(out=pg[:, 0:MUL], lhsT=ohg[:, t, :],
                                     rhs=wY1[:], start=True, stop=True)
                    nc.vector.tensor_mul(out=fb1[:], in0=pg[:, 0:MUL],
                                          in1=V10[:, t, :])
                    pst = psml.tile([32, 128], f32, space="PSUM", tag="sml")
                    nc.tensor.transpose(out=pst[0:16, :], in_=fb1[:],
                                        identity=ident[:])
                    nc.vector.tensor_copy(out=fbfm1[:, t * 128:(t + 1) * 128],
                                          in_=pst[0:16, :])

                # ---- layer-1 ly1/ly2 + residual -> x2 ----
                x2 = bgp.tile([128, 2, kwin], bf16)
                mlp_block(x1, x2, 1, fbfm1, True)

                # ---- edge out + receiver scatter ----
                eo = wnp.tile([128, 1], f32)
                mt = wnp.tile([128, RWIN], bf16)
                for t in range(T_W):
                    tsl = slice(t * 128, (t + 1) * 128)
                    p1 = psml.tile([128, 32], f32, space="PSUM", tag="sm2")
                    for kc in range(2):
                        nc.tensor.matmul(
                            out=p1[:, 0:1], lhsT=x2[:, kc, tsl],
                            rhs=wb[:, OFF_WOUT + kc:OFF_WOUT + kc + 1],
                            start=(kc == 0), stop=(kc == 1))
                    nc.vector.tensor_mul(out=eo[:], in0=p1[:, 0:1],
                                          in1=u_pl[:, t0 + t, None])
                    nc.vector.tensor_mul(
                        out=mt[:], in0=rwt[:, t, :],
                        in1=eo[:].to_broadcast([128, RWIN]))
                    nc.tensor.matmul(out=ps_rcv[:], lhsT=rqs[:, t, :],
                                     rhs=mt[:],
                                     start=(w == 0 and t == 0),
                                     stop=(w == NW - 1 and t == T_W - 1))

            out_sb = gp.tile([128, RWIN], f32)
            nc.vector.tensor_copy(out=out_sb[:], in_=ps_rcv[:])
            nc.sync.dma_start(out=d_out[:], in_=out_sb[:])

    ET = mybir.EngineType
    eng_map = {ET.DVE: nc.vector, ET.Activation: nc.scalar,
               ET.Pool: nc.gpsimd, ET.PE: nc.tensor, ET.SP: nc.sync}

    def mk_carrier(eng):
        be = eng_map.get(eng)
        if be is None:
            return None
        w = be.wait_ge(carrier_sem, 0)
        ci = w.ins if hasattr(w, "ins") else w
        for bb in nc.m.functions[0].blocks:
            il = list(bb.instructions)
            if any(x is ci for x in il):
                bb.instructions = [x for x in il if x is not ci]
                break
        return ci

    made = _split_waits(nc, mybir, mk_carrier)
    print(f"split_waits: carriers={made}", flush=True)
    return nc


def kernel(**inputs):
    inputs = {k: np.asarray(v) for k, v in inputs.items()}
    kwin, in_maps, _ = make_in_maps(inputs)
    nc = build_graph(kwin)
    from concourse.bass_utils import run_bass_kernel_spmd
    res = run_bass_kernel_spmd(nc, in_maps, core_ids=list(range(NC)))
    out = np.zeros((128, RWIN), np.float64)
    for r in res.results:
        out += np.asarray(r["out"], np.float64)
    # node n = hi*128 + lo stored at [lo, hi]
    return np.ascontiguousarray(out.T.reshape(N, 1)).astype(np.float32)


# revision 32
# speedup vs baseline: 5.8845x; 2.0287x over previous
"""Allegro-style GNN message passing on 8 TRN2 NeuronCores.

Strategy (v2 — minimal host->device bytes):
- Host: shard edges by SENDER node range (1024 nodes/core) -> sender
  segment-sums are fully core-local (no cross-core collectives).
  Within a core, group edges by 128-node sender windows; pad each
  (core, window) group to a common K_WIN with dummy edges (d=2 -> u=0 ->
  zero contribution).
- Inputs per core are just 3 packed blobs (~1.4 MB total): u8 index
  planes (sender-local / receiver-lo / receiver-hi), an f32 blob
  (edge vectors + biases + wcol), and a bf16 blob (node table +
  weights). One-hot scatter/gather matrices and endpoint-attribute
  gathers are built ON DEVICE (iota + is_equal + PE transposes +
  one-hot matmuls) instead of being shipped from the host -- the axon
  PJRT tunnel moves ~40 MB/s, so the previous 17.7 MB/core of host-
  built one-hots dominated wall time.
- Layer algebra: Y[:,0] == 1, so layer-1 only needs a 16-wide
  segment-sum of w1; W_lsh[1] output is dead; V1 is only needed at
  component 0 => contraction with Ytil = Y * W_lsh[0][:,0].
- Receiver scatter: node id = hi*128+lo; per edge-tile matmul with lo
  one-hot lhsT and (hi one-hot * edge_out) rhs accumulates [128,64]
  partials in PSUM; host sums the 8 per-core partials (the unshard).
- 1/sqrt(AVG_NEIGH) and the 1/sqrt(2) residual scales are folded into
  weights on the host.
"""
import math
import sys

import numpy as np

sys.path.insert(0, "/opt/trn_rl_repo")

import ml_dtypes  # noqa: E402

try:
    import jax
    jax.config.update("jax_compilation_cache_dir", "/tmp/jax_pcache")
    jax.config.update("jax_persistent_cache_min_entry_size_bytes", -1)
    jax.config.update("jax_persistent_cache_min_compile_time_secs", 0.0)
except Exception:
    pass

BF16 = ml_dtypes.bfloat16
SIM_SILU = False   # CoreSim lacks Silu; emulate with Sigmoid*z when set

N, E, MUL, H, F = 8192, 131072, 16, 256, 16
NB = 8
INV = 1.0 / math.sqrt(16.0)
NC = 8
NPC = N // NC          # nodes per core
WIN = 128
NW = NPC // WIN        # windows per core
RWIN = N // WIN        # 64 receiver windows
SQ = math.sqrt(0.5)

# ---- bf16 weight-blob column layout [128, CB] ----
OFF_NAT = 0                      # natts [128, 16*64]  cols = f*64 + hi
OFF_SNAT = OFF_NAT + 1024        # snat  [128, 8*16]   cols = w*16 + f
OFF_WE0 = OFF_SNAT + 128         # we0 [40, 256] rows 0..39; the two
#   wly1fb [16, 256] blocks share these cols at rows 64..79 / 96..111
OFF_WE1 = OFF_WE0 + 256          # we1 2 x [128, 256]
OFF_WV0 = OFF_WE1 + 512          # wv0 2 x [128, 16]
OFF_WLW0 = OFF_WV0 + 32
OFF_WLW1 = OFF_WLW0 + 32
OFF_WLY1 = (OFF_WLW1 + 32, OFF_WLW1 + 32 + 512)
OFF_WLY2 = (OFF_WLY1[1] + 512, OFF_WLY1[1] + 512 + 512)
OFF_WOUT = OFF_WLY2[1] + 512     # wout 2 x [128, 1]
OFF_FB1 = OFF_WOUT + 2           # wly1fb_1 [16, 256] (rows 0..15)
CB = OFF_FB1 + 256


def _host_shard(vectors, senders, receivers):
    """Group edges by (core, sender-window); pad to common K_WIN."""
    core = senders // NPC
    win = (senders % NPC) // WIN
    key = core * NW + win
    order = np.argsort(key, kind="stable")
    counts = np.bincount(key, minlength=NC * NW)
    kwin = int(((counts.max() + 127) // 128) * 128)
    starts = np.zeros(NC * NW + 1, np.int64)
    np.cumsum(counts, out=starts[1:])

    EP = NW * kwin
    shards = []
    for c in range(NC):
        vec = np.zeros((EP, 3), np.float32)
        vec[:, 0] = 2.0
        sl = np.zeros(EP, np.uint8)    # sender local-in-window
        rlo = np.zeros(EP, np.uint8)
        rhi = np.zeros(EP, np.uint8)
        for w in range(NW):
            g = c * NW + w
            eid = order[starts[g]:starts[g + 1]]
            o = w * kwin
            n_e = len(eid)
            vec[o:o + n_e] = vectors[eid]
            sl[o:o + n_e] = (senders[eid] - (c * NPC + w * WIN)).astype(np.uint8)
            rlo[o:o + n_e] = (receivers[eid] % 128).astype(np.uint8)
            rhi[o:o + n_e] = (receivers[eid] // 128).astype(np.uint8)
        shards.append((vec, sl, rlo, rhi))
    return kwin, shards


def _plane(a, T_ALL):
    """[EP] or [EP, k] -> plane layout [128, T_ALL*(k)] with e = t*128+p."""
    if a.ndim == 1:
        return np.ascontiguousarray(a.reshape(T_ALL, 128).T)
    # [EP, k] -> [128, k*T_ALL] with component-major column groups
    k = a.shape[1]
    p = a.reshape(T_ALL, 128, k).transpose(2, 1, 0)     # [k, 128, T_ALL]
    return np.ascontiguousarray(p.reshape(k * 128, T_ALL)).reshape(k, 128, T_ALL)


def _prep_weights(i):
    """Fold INV and residual 1/sqrt(2) scales into weights (f32)."""
    w = {}
    w["we0"] = i["W_e0"]                                       # [40,256]
    w["we1"] = i["W_e1"]
    w["wv0"] = i["W_v0"]
    w["wlw0"] = i["W_lw"][0] * INV
    w["wlw1"] = i["W_lw"][1] * INV * SQ                        # x1 = sq*x1'
    wly1_1 = i["W_ly1"][1].copy()
    wly1_1[:H] *= SQ                                           # x rows scaled
    w["wly1_0"] = i["W_ly1"][0]
    w["wly1_1"] = wly1_1
    w["wly2_0"] = i["W_ly2"][0]
    w["wly2_1"] = i["W_ly2"][1]
    w["wout"] = i["W_out"] * INV * 0.5                         # x2 = .5*x2'
    return w


def _pack_blobw(i, c):
    """Per-core bf16 blob [128, CB]: node table + folded weights."""
    w = _prep_weights(i)
    na = i["node_attrs"]                                       # [N, F]
    blob = np.zeros((128, CB), np.float32)
    # natts[lo, f*64+hi] = na[hi*128+lo, f]
    nat = na.reshape(RWIN, 128, F).transpose(1, 2, 0)          # [lo, f, hi]
    blob[:, OFF_NAT:OFF_NAT + 1024] = nat.reshape(128, F * RWIN)
    # snat[lo, w*16+f] = na[(c*8+w)*128+lo, f]
    sn = na.reshape(RWIN, 128, F)[c * NW:(c + 1) * NW]         # [w, lo, f]
    blob[:, OFF_SNAT:OFF_SNAT + 128] = sn.transpose(1, 0, 2).reshape(128, 128)
    # rhs row order is [snd attrs(16), rcv attrs(16), bessel(8)] so the
    # on-device copies land on legal partition offsets (0 and 32)
    blob[0:40, OFF_WE0:OFF_WE0 + 256] = np.vstack([w["we0"][8:40],
                                                   w["we0"][0:8]])
    blob[64:80, OFF_WE0:OFF_WE0 + 256] = w["wly1_0"][256:272]
    blob[0:16, OFF_FB1:OFF_FB1 + 256] = w["wly1_1"][256:272]
    for kc in range(2):
        s = slice(kc * 128, (kc + 1) * 128)
        blob[:, OFF_WE1 + kc * 256:OFF_WE1 + (kc + 1) * 256] = w["we1"][s]
        blob[:, OFF_WV0 + kc * 16:OFF_WV0 + (kc + 1) * 16] = w["wv0"][s]
        blob[:, OFF_WLW0 + kc * 16:OFF_WLW0 + (kc + 1) * 16] = w["wlw0"][s]
        blob[:, OFF_WLW1 + kc * 16:OFF_WLW1 + (kc + 1) * 16] = w["wlw1"][s]
        blob[:, OFF_WOUT + kc:OFF_WOUT + kc + 1] = w["wout"][s]
    for l in range(2):
        m = w[f"wly1_{l}"]
        for kc in range(2):
            s = slice(kc * 128, (kc + 1) * 128)
            blob[:, OFF_WLY1[l] + kc * 256:OFF_WLY1[l] + (kc + 1) * 256] = m[s]
            blob[:, OFF_WLY2[l] + kc * 256:OFF_WLY2[l] + (kc + 1) * 256] = \
                w[f"wly2_{l}"][s]
    return blob.astype(BF16)


def make_in_maps(inputs):
    kwin, shards = _host_shard(inputs["vectors"], inputs["senders"],
                               inputs["receivers"])
    EP = NW * kwin
    T_ALL = EP // 128
    CF = 3 * T_ALL + 28
    bias_list = [inputs["b_e0"], inputs["b_e1"],
                 inputs["b_ly1"][0], inputs["b_ly1"][1],
                 inputs["b_ly2"][0], inputs["b_ly2"][1]]
    wcol = inputs["W_lsh"][0][:, 0]                            # [16]
    blobw0 = _pack_blobw(inputs, 0)      # core-dependent only in SNAT
    in_maps = []
    dbg = []
    for c in range(NC):
        vec, sl, rlo, rhi = shards[c]
        b8 = np.empty((128, 3 * T_ALL), np.uint8)
        b8[:, 0:T_ALL] = _plane(sl, T_ALL)
        b8[:, T_ALL:2 * T_ALL] = _plane(rlo, T_ALL)
        b8[:, 2 * T_ALL:3 * T_ALL] = _plane(rhi, T_ALL)
        bf = np.zeros((128, CF), np.float32)
        vp = _plane(vec, T_ALL)                                # [3,128,T]
        bf[:, 0:T_ALL] = vp[0]
        bf[:, T_ALL:2 * T_ALL] = vp[1]
        bf[:, 2 * T_ALL:3 * T_ALL] = vp[2]
        for i, b in enumerate(bias_list):
            bf[:, 3 * T_ALL + 2 * i] = b[0:128]
            bf[:, 3 * T_ALL + 2 * i + 1] = b[128:256]
        bf[:, 3 * T_ALL + 12:3 * T_ALL + 28] = np.tile(
            wcol.reshape(1, 16), (128, 1))
        bw = _pack_blobw(inputs, c) if c else blobw0
        in_maps.append({"blob8": b8, "blobf": np.ascontiguousarray(bf),
                        "blobw": bw})
        dbg.append(dict(vec=vec, sl=sl, rlo=rlo, rhi=rhi))
    return kwin, in_maps, dbg


_CAP_SKIP = {"InstEventSemaphore", "InstBranch", "InstNop",
             "InstCollectiveCompute"}
_CAP_LIMITS = {}


def _split_waits(nc, mybir, mk_carrier, limit=1):
    """Walrus codegen allows only 1 embedded sem-wait on compute
    instructions.  For each instruction with more, strip the extras onto
    freshly created same-engine carrier instructions inserted directly
    before it (engines are in-order, so this preserves semantics)."""
    f = nc.m.functions[0]
    made = 0
    for bb in f.blocks:
        insts = list(bb.instructions)
        plan = []          # (index, [carrier insts])
        for i, inst in enumerate(insts):
            tname = type(inst).__name__
            si = inst.sync_info
            nwait = len(si.on_wait) if (si and si.on_wait) else 0
            lim = _CAP_LIMITS.get(tname, limit)
            if tname in _CAP_SKIP or nwait <= lim:
                continue
            waits = list(si.on_wait)
            extras, keep = waits[:-lim], waits[-lim:]
            carriers = []
            for wt in extras:
                ci = mk_carrier(inst.engine)
                if ci is None:
                    keep.insert(0, wt)
                    continue
                ci.sync_info = mybir.SyncInfo(on_wait=[wt], on_update=[])
                carriers.append(ci)
                made += 1
            inst.sync_info = mybir.SyncInfo(on_wait=keep,
                                            on_update=si.on_update)
            if carriers:
                plan.append((i, carriers))
        if plan:
            new = []
            pmap = dict(plan)
            for i, inst in enumerate(insts):
                if i in pmap:
                    new.extend(pmap[i])
                new.append(inst)
            bb.instructions = new
    return made


def build_graph(kwin):
    from concourse import bass, mybir
    from concourse.masks import make_identity
    from concourse.tile import TileContext

    EP = NW * kwin
    T_ALL = EP // 128
    T_W = kwin // 128
    NCH = (kwin + 511) // 512      # free chunks per window
    CF = 3 * T_ALL + 28

    f32 = mybir.dt.float32
    bf16 = mybir.dt.bfloat16
    i32 = mybir.dt.int32
    u8 = mybir.dt.uint8
    AX = mybir.AxisListType.X
    OP = mybir.AluOpType
    AF = mybir.ActivationFunctionType

    nc = bass.Bass()
    carrier_sem_cm = nc.semaphore("carrier_sem")
    carrier_sem = carrier_sem_cm.__enter__()
    dp = nc.declare_dram_parameter
    d_b8 = dp("blob8", [128, 3 * T_ALL], u8, isOutput=False)
    d_bf = dp("blobf", [128, CF], f32, isOutput=False)
    d_bw = dp("blobw", [128, CB], bf16, isOutput=False)
    d_out = dp("out", [128, RWIN], f32, isOutput=True)

    with TileContext(nc) as tc:
        with (
            tc.tile_pool(name="glob", bufs=1) as gp,
            tc.tile_pool(name="wgt", bufs=1) as wp,
            tc.tile_pool(name="win", bufs=2) as wnp,
            tc.tile_pool(name="big", bufs=1) as bgp,
            tc.tile_pool(name="sml", bufs=3) as sp,
            tc.tile_pool(name="ps_mlp", bufs=2, space="PSUM") as pmlp,
            tc.tile_pool(name="ps_acc", bufs=1, space="PSUM") as pacc,
            tc.tile_pool(name="ps_gth", bufs=1, space="PSUM") as pgth,
            tc.tile_pool(name="ps_sml", bufs=1, space="PSUM") as psml,
            tc.tile_pool(name="ps_rcv", bufs=1, space="PSUM") as prcv,
        ):
            # ---------------- blobs to SBUF ----------------
            wb = wp.tile([128, CB], bf16, tag="wb")
            nc.sync.dma_start(out=wb[:], in_=d_bw[:])
            fbt = wp.tile([128, CF], f32, tag="fbt")
            nc.sync.dma_start(out=fbt[:], in_=d_bf[:])
            i8t = wp.tile([128, 3 * T_ALL], u8, tag="i8t")
            nc.sync.dma_start(out=i8t[:], in_=d_b8[:])
            slf = wp.tile([128, T_ALL], f32, tag="slf")
            rlof = wp.tile([128, T_ALL], f32, tag="rlof")
            rhif = wp.tile([128, T_ALL], f32, tag="rhif")
            nc.vector.tensor_copy(out=slf[:], in_=i8t[:, 0:T_ALL])
            nc.vector.tensor_copy(out=rlof[:], in_=i8t[:, T_ALL:2 * T_ALL])
            nc.vector.tensor_copy(out=rhif[:], in_=i8t[:, 2 * T_ALL:3 * T_ALL])

            ident = wp.tile([128, 128], f32, tag="ident")
            make_identity(nc, ident[:])
            identb = wp.tile([128, 128], bf16, tag="identb")
            make_identity(nc, identb[:])
            it32 = wp.tile([128, 128], i32, tag="it32")
            nc.gpsimd.iota(out=it32[:], pattern=[[1, 128]], base=0,
                           channel_multiplier=0)
            iof = wp.tile([128, 128], f32, tag="iof")
            nc.vector.tensor_copy(out=iof[:], in_=it32[:])
            ones_bf = wp.tile([1, 128], bf16, tag="ones")
            nc.gpsimd.memset(ones_bf[:], 1.0)

            # views into the blobs
            vx = fbt[:, 0:T_ALL]
            vy = fbt[:, T_ALL:2 * T_ALL]
            vz = fbt[:, 2 * T_ALL:3 * T_ALL]
            BIA = 3 * T_ALL

            def bias(i, hc):
                return fbt[:, BIA + 2 * i + hc, None]
            wcol = fbt[:, BIA + 12:BIA + 28]
            natv = wb[:, OFF_NAT:OFF_NAT + 1024]

            # ---------------- edge-scalar stage (planes [128,T_ALL]) ----
            ta = gp.tile([128, T_ALL], f32)
            tb = gp.tile([128, T_ALL], f32)
            ts = nc.vector.tensor_scalar
            act = nc.scalar.activation

            def silu_act(out, ps_in, bias_ap):
                if not SIM_SILU:
                    act(out=out, in_=ps_in, func=AF.Silu, bias=bias_ap)
                else:
                    pp = ps_in.shape[0]
                    sg = bgp.tile([128, 512], f32, tag="simsilu")
                    zz_ = bgp.tile([128, 512], f32, tag="simsilu2")
                    cw_ = ps_in.shape[-1]
                    act(out=sg[:pp, :cw_], in_=ps_in, func=AF.Sigmoid,
                        bias=bias_ap)
                    nc.vector.tensor_scalar(out=zz_[:pp, :cw_], in0=ps_in,
                                            scalar1=bias_ap, scalar2=None,
                                            op0=OP.add)
                    nc.vector.tensor_mul(out=out, in0=sg[:pp, :cw_],
                                         in1=zz_[:pp, :cw_])
            d_pl = gp.tile([128, T_ALL], f32)
            nc.vector.tensor_mul(out=ta[:], in0=vx, in1=vx)
            nc.vector.tensor_mul(out=tb[:], in0=vy, in1=vy)
            nc.vector.tensor_add(out=ta[:], in0=ta[:], in1=tb[:])
            nc.vector.tensor_mul(out=tb[:], in0=vz, in1=vz)
            nc.vector.tensor_add(out=ta[:], in0=ta[:], in1=tb[:])
            act(out=d_pl[:], in_=ta[:], func=AF.Sqrt)
            rinv = gp.tile([128, T_ALL], f32)
            nc.vector.reciprocal(out=rinv[:], in_=d_pl[:])
            ux = gp.tile([128, T_ALL], f32)
            uy = gp.tile([128, T_ALL], f32)
            uz = gp.tile([128, T_ALL], f32)
            nc.vector.tensor_mul(out=ux[:], in0=vx, in1=rinv[:])
            nc.vector.tensor_mul(out=uy[:], in0=vy, in1=rinv[:])
            nc.vector.tensor_mul(out=uz[:], in0=vz, in1=rinv[:])

            # envelope u = 1 + d^6*(-28 + 48d - 21d^2), zero for d >= 1
            u_pl = gp.tile([128, T_ALL], f32)
            nc.vector.tensor_mul(out=ta[:], in0=d_pl[:], in1=d_pl[:])   # d2
            nc.vector.tensor_mul(out=tb[:], in0=ta[:], in1=d_pl[:])     # d3
            nc.vector.tensor_mul(out=tb[:], in0=tb[:], in1=tb[:])       # d6
            ts(out=ta[:], in0=ta[:], scalar1=-21.0, scalar2=None, op0=OP.mult)
            tc_q = gp.tile([128, T_ALL], f32)
            ts(out=tc_q[:], in0=d_pl[:], scalar1=48.0, scalar2=-28.0,
               op0=OP.mult, op1=OP.add)
            nc.vector.tensor_add(out=ta[:], in0=ta[:], in1=tc_q[:])
            nc.vector.tensor_mul(out=tb[:], in0=tb[:], in1=ta[:])
            ts(out=tb[:], in0=tb[:], scalar1=1.0, scalar2=None, op0=OP.add)
            ts(out=ta[:], in0=d_pl[:], scalar1=1.0, scalar2=None,
               op0=OP.is_lt)
            nc.vector.tensor_mul(out=u_pl[:], in0=tb[:], in1=ta[:])

            # spherical harmonics Y [128, T_ALL, 16] f32
            Yt = gp.tile([128, T_ALL, 16], f32)
            s3 = 3.0 ** 0.5; s5 = 5.0 ** 0.5; s15 = 15.0 ** 0.5
            s7 = 7.0 ** 0.5
            c33 = (35.0 / 8.0) ** 0.5; c32 = 105.0 ** 0.5
            c31 = (21.0 / 8.0) ** 0.5
            xx = gp.tile([128, T_ALL], f32)
            yy = gp.tile([128, T_ALL], f32)
            zz = gp.tile([128, T_ALL], f32)
            xy = gp.tile([128, T_ALL], f32)
            nc.vector.tensor_mul(out=xx[:], in0=ux[:], in1=ux[:])
            nc.vector.tensor_mul(out=yy[:], in0=uy[:], in1=uy[:])
            nc.vector.tensor_mul(out=zz[:], in0=uz[:], in1=uz[:])
            nc.vector.tensor_mul(out=xy[:], in0=ux[:], in1=uy[:])
            ts(out=Yt[:, :, 0], in0=ux[:], scalar1=0.0, scalar2=1.0,
               op0=OP.mult, op1=OP.add)
            ts(out=Yt[:, :, 1], in0=ux[:], scalar1=s3, scalar2=None,
               op0=OP.mult)
            ts(out=Yt[:, :, 2], in0=uy[:], scalar1=s3, scalar2=None,
               op0=OP.mult)
            ts(out=Yt[:, :, 3], in0=uz[:], scalar1=s3, scalar2=None,
               op0=OP.mult)
            ts(out=Yt[:, :, 4], in0=xy[:], scalar1=s15, scalar2=None,
               op0=OP.mult)
            nc.vector.tensor_mul(out=ta[:], in0=uy[:], in1=uz[:])
            ts(out=Yt[:, :, 5], in0=ta[:], scalar1=s15, scalar2=None,
               op0=OP.mult)
            ts(out=Yt[:, :, 6], in0=zz[:], scalar1=1.5 * s5,
               scalar2=-0.5 * s5, op0=OP.mult, op1=OP.add)
            nc.vector.tensor_mul(out=tb[:], in0=ux[:], in1=uz[:])
            ts(out=Yt[:, :, 7], in0=tb[:], scalar1=s15, scalar2=None,
               op0=OP.mult)
            xmy = gp.tile([128, T_ALL], f32)
            nc.vector.tensor_sub(out=xmy[:], in0=xx[:], in1=yy[:])
            ts(out=Yt[:, :, 8], in0=xmy[:], scalar1=0.5 * s15, scalar2=None,
               op0=OP.mult)
            ts(out=ta[:], in0=xx[:], scalar1=3.0, scalar2=None, op0=OP.mult)
            nc.vector.tensor_sub(out=ta[:], in0=ta[:], in1=yy[:])
            nc.vector.tensor_mul(out=ta[:], in0=ta[:], in1=uy[:])
            ts(out=Yt[:, :, 9], in0=ta[:], scalar1=c33, scalar2=None,
               op0=OP.mult)
            nc.vector.tensor_mul(out=ta[:], in0=xy[:], in1=uz[:])
            ts(out=Yt[:, :, 10], in0=ta[:], scalar1=c32, scalar2=None,
               op0=OP.mult)
            ts(out=ta[:], in0=zz[:], scalar1=5.0, scalar2=-1.0,
               op0=OP.mult, op1=OP.add)
            nc.vector.tensor_mul(out=tb[:], in0=ta[:], in1=uy[:])
            ts(out=Yt[:, :, 11], in0=tb[:], scalar1=c31, scalar2=None,
               op0=OP.mult)
            nc.vector.tensor_mul(out=tb[:], in0=ta[:], in1=ux[:])
            ts(out=Yt[:, :, 13], in0=tb[:], scalar1=c31, scalar2=None,
               op0=OP.mult)
            nc.vector.tensor_mul(out=ta[:], in0=zz[:], in1=uz[:])
            ts(out=ta[:], in0=ta[:], scalar1=2.5 * s7, scalar2=None,
               op0=OP.mult)
            ts(out=tb[:], in0=uz[:], scalar1=1.5 * s7, scalar2=None,
               op0=OP.mult)
            nc.vector.tensor_sub(out=Yt[:, :, 12], in0=ta[:], in1=tb[:])
            nc.vector.tensor_mul(out=ta[:], in0=xmy[:], in1=uz[:])
            ts(out=Yt[:, :, 14], in0=ta[:], scalar1=0.5 * c32, scalar2=None,
               op0=OP.mult)
            ts(out=ta[:], in0=yy[:], scalar1=3.0, scalar2=None, op0=OP.mult)
            nc.vector.tensor_sub(out=ta[:], in0=xx[:], in1=ta[:])
            nc.vector.tensor_mul(out=ta[:], in0=ta[:], in1=ux[:])
            ts(out=Yt[:, :, 15], in0=ta[:], scalar1=c33, scalar2=None,
               op0=OP.mult)

            # bessel (range-reduced): besu [128, T_ALL, 8]
            besu = gp.tile([128, T_ALL, 8], f32)
            rs = gp.tile([128, T_ALL], f32)
            ts(out=rs[:], in0=rinv[:], scalar1=math.sqrt(2.0), scalar2=None,
               op0=OP.mult)
            mi = gp.tile([128, T_ALL], mybir.dt.int32)
            for k in range(1, NB + 1):
                ts(out=ta[:], in0=d_pl[:], scalar1=0.5 * k, scalar2=None,
                   op0=OP.mult)
                nc.vector.tensor_copy(out=mi[:], in_=ta[:])
                nc.vector.tensor_copy(out=tb[:], in_=mi[:])
                nc.vector.tensor_sub(out=ta[:], in0=ta[:], in1=tb[:])
                # ta = frac in (-0.5, 1) whether the cast rounds or truncates
                ts(out=tb[:], in0=ta[:], scalar1=0.5, scalar2=None,
                   op0=OP.is_gt)
                nc.vector.tensor_sub(out=ta[:], in0=ta[:], in1=tb[:])
                act(out=ta[:], in_=ta[:], func=AF.Sin, scale=2.0 * math.pi)
                nc.vector.tensor_mul(out=besu[:, :, k - 1], in0=ta[:],
                                      in1=rs[:])

            # ---------------- persistent receiver accumulator ----------
            ps_rcv = prcv.tile([128, RWIN], f32, space="PSUM")

            # ---------------- window loop ----------------
            for w in range(NW):
                t0 = w * T_W
                ohs = wnp.tile([128, T_W, 128], bf16)   # [e, n]
                ohg = wnp.tile([128, T_W, 128], bf16)   # [n, e]
                rqs = wnp.tile([128, T_W, 128], bf16)   # [e, lo]
                rqg = wnp.tile([128, T_W, 128], bf16)   # [lo, e]
                rwt = wnp.tile([128, T_W, RWIN], bf16)  # [e, hi]
                xfm = wnp.tile([40, kwin], bf16)        # snd(16)+rcv(16)+bes(8)
                ufm = wnp.tile([1, kwin], bf16)
                tt = nc.vector.tensor_tensor
                for t in range(T_W):
                    tg = t0 + t
                    csl = slice(t * 128, (t + 1) * 128)
                    tt(out=ohs[:, t, :],
                       in0=slf[:, tg, None].to_broadcast([128, 128]),
                       in1=iof[:], op=OP.is_equal)
                    tt(out=rqs[:, t, :],
                       in0=rlof[:, tg, None].to_broadcast([128, 128]),
                       in1=iof[:], op=OP.is_equal)
                    tt(out=rwt[:, t, :],
                       in0=rhif[:, tg, None].to_broadcast([128, RWIN]),
                       in1=iof[:, 0:RWIN], op=OP.is_equal)
                    ptr = psml.tile([128, 128], bf16, space="PSUM", tag="trn")
                    nc.tensor.transpose(out=ptr[:], in_=ohs[:, t, :],
                                        identity=identb[:])
                    nc.vector.tensor_copy(out=ohg[:, t, :], in_=ptr[:])
                    ptr2 = psml.tile([128, 128], bf16, space="PSUM", tag="trn")
                    nc.tensor.transpose(out=ptr2[:], in_=rqs[:, t, :],
                                        identity=identb[:])
                    nc.vector.tensor_copy(out=rqg[:, t, :], in_=ptr2[:])
                    # bessel + u feature-major
                    pst = psml.tile([32, 128], f32, space="PSUM", tag="sml")
                    nc.tensor.transpose(out=pst[0:8, :], in_=besu[:, tg, :],
                                        identity=ident[:])
                    nc.vector.tensor_copy(out=xfm[32:40, csl],
                                          in_=pst[0:8, :])
                    psu1 = psml.tile([32, 128], f32, space="PSUM", tag="sml")
                    nc.tensor.transpose(out=psu1[0:1, :],
                                        in_=u_pl[:, tg, None],
                                        identity=ident[:])
                    nc.vector.tensor_copy(out=ufm[:, csl], in_=psu1[0:1, :])
                    # endpoint-attr gather: sender (window-local one-hot)
                    gcmb = sp.tile([128, 32], f32, tag="gcmb")
                    psn = psml.tile([128, 32], f32, space="PSUM", tag="sm2")
                    nc.tensor.matmul(
                        out=psn[:, 0:16], lhsT=ohg[:, t, :],
                        rhs=wb[:, OFF_SNAT + w * 16:OFF_SNAT + (w + 1) * 16],
                        start=True, stop=True)
                    nc.vector.tensor_copy(out=gcmb[:, 0:16], in_=psn[:, 0:16])
                    # receiver: lo-gather matmul then hi-select
                    for c2 in range(2):
                        prg = pgth.tile([128, 512], f32, space="PSUM",
                                        tag="gth")
                        nc.tensor.matmul(
                            out=prg[:], lhsT=rqg[:, t, :],
                            rhs=natv[:, c2 * 512:(c2 + 1) * 512],
                            start=True, stop=True)
                        prod = sp.tile([128, 8, RWIN], f32, tag="rsel")
                        nc.vector.tensor_mul(
                            out=prod[:],
                            in0=prg[:].rearrange("p (a b) -> p a b", b=RWIN),
                            in1=rwt[:, t, None, :].to_broadcast(
                                [128, 8, RWIN]))
                        nc.vector.reduce_sum(
                            out=gcmb[:, 16 + c2 * 8:16 + (c2 + 1) * 8, None],
                            in_=prod[:], axis=AX)
                    ptg = psml.tile([32, 128], f32, space="PSUM", tag="sml")
                    nc.tensor.transpose(out=ptg[:], in_=gcmb[:],
                                        identity=ident[:])
                    nc.vector.tensor_copy(out=xfm[0:32, csl], in_=ptg[:])

                # broadcast u row -> [128, kwin] bf16
                ubc = bgp.tile([128, kwin], bf16)
                for ch in range(NCH):
                    c0 = ch * 512
                    c1 = min(kwin, c0 + 512)
                    psu = pmlp.tile([128, 512], f32, space="PSUM", tag="mlp")
                    nc.tensor.matmul(out=psu[:, :c1 - c0], lhsT=ones_bf[:],
                                     rhs=ufm[:, c0:c1],
                                     start=True, stop=True)
                    nc.vector.tensor_copy(out=ubc[:, c0:c1],
                                          in_=psu[:, :c1 - c0])

                # ---- edge MLP: x0 = u*silu(e1(silu(e0(bes,attrs)))) ----
                x0 = bgp.tile([128, 2, kwin], bf16)
                th = bgp.tile([128, 2, kwin], bf16)
                for ch in range(NCH):
                    c0 = ch * 512
                    c1 = min(kwin, c0 + 512)
                    cw = c1 - c0
                    for hc in range(2):
                        ps = pmlp.tile([128, 512], f32, space="PSUM", tag="mlp")
                        nc.tensor.matmul(
                            out=ps[:, :cw],
                            lhsT=wb[0:40, OFF_WE0 + hc * 128:
                                    OFF_WE0 + (hc + 1) * 128],
                            rhs=xfm[:, c0:c1], start=True, stop=True)
                        silu_act(th[:, hc, c0:c1], ps[:, :cw], bias(0, hc))
                for ch in range(NCH):
                    c0 = ch * 512
                    c1 = min(kwin, c0 + 512)
                    cw = c1 - c0
                    for hc in range(2):
                        ps = pmlp.tile([128, 512], f32, space="PSUM", tag="mlp")
                        for kc in range(2):
                            nc.tensor.matmul(
                                out=ps[:, :cw],
                                lhsT=wb[:, OFF_WE1 + kc * 256 + hc * 128:
                                        OFF_WE1 + kc * 256 + (hc + 1) * 128],
                                rhs=th[:, kc, c0:c1],
                                start=(kc == 0), stop=(kc == 1))
                        silu_act(x0[:, hc, c0:c1], ps[:, :cw], bias(1, hc))
                for hc in range(2):
                    nc.vector.tensor_mul(out=x0[:, hc, :], in0=x0[:, hc, :],
                                          in1=ubc[:])

                # ---- xv, w0 (edge-major [128,16] per tile) ----
                xv = wnp.tile([128, T_W, MUL], f32)
                w0 = wnp.tile([128, T_W, MUL], bf16)
                for t in range(T_W):
                    tsl = slice(t * 128, (t + 1) * 128)
                    p12 = psml.tile([128, 32], f32, space="PSUM", tag="sm2")
                    for kc in range(2):
                        nc.tensor.matmul(
                            out=p12[:, 0:16], lhsT=x0[:, kc, tsl],
                            rhs=wb[:, OFF_WV0 + kc * 16:OFF_WV0 + (kc + 1) * 16],
                            start=(kc == 0), stop=(kc == 1))
                    for kc in range(2):
                        nc.tensor.matmul(
                            out=p12[:, 16:32], lhsT=x0[:, kc, tsl],
                            rhs=wb[:, OFF_WLW0 + kc * 16:
                                    OFF_WLW0 + (kc + 1) * 16],
                            start=(kc == 0), stop=(kc == 1))
                    nc.vector.tensor_copy(out=xv[:, t, :], in_=p12[:, 0:16])
                    nc.vector.tensor_copy(out=w0[:, t, :], in_=p12[:, 16:32])

                # ---- layer-0 scatter: wY[n, m*16+i] ----
                ps_acc = pacc.tile([128, 256], f32, space="PSUM", tag="acc")
                for t in range(T_W):
                    v2 = sp.tile([128, MUL, 16], bf16, tag="v2")
                    nc.vector.tensor_mul(
                        out=v2[:],
                        in0=w0[:, t, :, None].to_broadcast([128, MUL, 16]),
                        in1=Yt[:, t0 + t, None, :].to_broadcast(
                            [128, MUL, 16]))
                    nc.tensor.matmul(
                        out=ps_acc[:],
                        lhsT=ohs[:, t, :],
                        rhs=v2[:].rearrange("p a b -> p (a b)"),
                        start=(t == 0), stop=(t == T_W - 1))
                wY = wnp.tile([128, 256], bf16)
                nc.vector.tensor_copy(out=wY[:], in_=ps_acc[:])

                # ---- gather + Ytil contraction + feedback ----
                # fbfm lives at partitions 64..79 so its matmul shares the
                # base partition of the packed wly1fb_0 weights
                V10 = wnp.tile([128, T_W, MUL], f32)
                fbfm = wnp.tile([80, kwin], bf16)
                prod = wnp.tile([128, MUL, 16], f32)
                ytil = wnp.tile([128, MUL], f32)
                Ssb = wnp.tile([128, MUL], f32)
                fb = wnp.tile([128, MUL], f32)
                for t in range(T_W):
                    pgf = pgth.tile([128, 512], f32, space="PSUM", tag="gth")
                    pg = pgf[:, 0:256]
                    nc.tensor.matmul(out=pg, lhsT=ohg[:, t, :], rhs=wY[:],
                                     start=True, stop=True)
                    pg3 = pg.rearrange("p (a b) -> p a b", b=16)
                    nc.vector.tensor_mul(out=ytil[:], in0=Yt[:, t0 + t, :],
                                          in1=wcol)
                    nc.vector.tensor_mul(
                        out=prod[:], in0=pg3,
                        in1=ytil[:, None, :].to_broadcast([128, MUL, 16]))
                    nc.vector.reduce_sum(out=Ssb[:, :, None], in_=prod[:],
                                         axis=AX)
                    nc.vector.tensor_mul(out=V10[:, t, :], in0=Ssb[:],
                                          in1=xv[:, t, :])
                    nc.vector.tensor_mul(out=fb[:], in0=pg3[:, :, 0],
                                          in1=xv[:, t, :])
                    pst = psml.tile([32, 128], f32, space="PSUM", tag="sml")
                    nc.tensor.transpose(out=pst[0:16, :], in_=fb[:],
                                        identity=ident[:])
                    nc.vector.tensor_copy(
                        out=fbfm[64:80, t * 128:(t + 1) * 128],
                        in_=pst[0:16, :])

                # ---- layer-0 ly1/ly2 + residual -> x1 ----
                x1 = bgp.tile([128, 2, kwin], bf16)

                def mlp_block(xin, xout, l, fbrow, resid_sq2):
                    b1 = OFF_WLY1[l]
                    # wly1fb: layer 0 at rows 64..79 of the WE0 cols,
                    # layer 1 at rows 0..15 of its own FB1 cols
                    fbp, fbc = (64, OFF_WE0) if l == 0 else (0, OFF_FB1)
                    b2 = OFF_WLY2[l]
                    ty = bgp.tile([128, 2, kwin], bf16)
                    for ch in range(NCH):
                        c0 = ch * 512
                        c1 = min(kwin, c0 + 512)
                        cw = c1 - c0
                        for hc in range(2):
                            hs = slice(hc * 128, (hc + 1) * 128)
                            ps = pmlp.tile([128, 512], f32, space="PSUM",
                                           tag="mlp")
                            for kc in range(2):
                                nc.tensor.matmul(
                                    out=ps[:, :cw],
                                    lhsT=wb[:, b1 + kc * 256 + hc * 128:
                                            b1 + kc * 256 + (hc + 1) * 128],
                                    rhs=xin[:, kc, c0:c1],
                                    start=(kc == 0), stop=False)
                            nc.tensor.matmul(
                                out=ps[:, :cw],
                                lhsT=wb[fbp:fbp + 16, fbc + hc * 128:
                                        fbc + (hc + 1) * 128],
                                rhs=fbrow[fbp:fbp + 16, c0:c1],
                                start=False, stop=True)
                            silu_act(ty[:, hc, c0:c1], ps[:, :cw],
                                     bias(2 + l, hc))
                    ty2 = bgp.tile([128, 2, kwin], bf16)
                    for ch in range(NCH):
                        c0 = ch * 512
                        c1 = min(kwin, c0 + 512)
                        cw = c1 - c0
                        for hc in range(2):
                            ps = pmlp.tile([128, 512], f32, space="PSUM",
                                           tag="mlp")
                            for kc in range(2):
                                nc.tensor.matmul(
                                    out=ps[:, :cw],
                                    lhsT=wb[:, b2 + kc * 256 + hc * 128:
                                            b2 + kc * 256 + (hc + 1) * 128],
                                    rhs=ty[:, kc, c0:c1],
                                    start=(kc == 0), stop=(kc == 1))
                            silu_act(ty2[:, hc, c0:c1], ps[:, :cw],
                                     bias(4 + l, hc))
                    # x_out' = x_in' + s * u * y   (s = 1 or sqrt(2))
                    for hc in range(2):
                        nc.vector.tensor_mul(out=ty2[:, hc, :],
                                              in0=ty2[:, hc, :], in1=ubc[:])
                        if resid_sq2:
                            ts(out=ty2[:, hc, :], in0=ty2[:, hc, :],
                               scalar1=math.sqrt(2.0), scalar2=None,
                               op0=OP.mult)
                        nc.vector.tensor_add(out=xout[:, hc, :],
                                             in0=xin[:, hc, :],
                                             in1=ty2[:, hc, :])

                mlp_block(x0, x1, 0, fbfm, False)

                # ---- layer 1: w1, 16-wide scatter/gather, feedback ----
                w1 = wnp.tile([128, T_W, MUL], bf16)
                for t in range(T_W):
                    tsl = slice(t * 128, (t + 1) * 128)
                    p1 = psml.tile([128, 32], f32, space="PSUM", tag="sm2")
                    for kc in range(2):
                        nc.tensor.matmul(
                            out=p1[:, 0:MUL], lhsT=x1[:, kc, tsl],
                            rhs=wb[:, OFF_WLW1 + kc * 16:
                                    OFF_WLW1 + (kc + 1) * 16],
                            start=(kc == 0), stop=(kc == 1))
                    nc.vector.tensor_copy(out=w1[:, t, :], in_=p1[:, 0:MUL])
                ps_a1 = pacc.tile([128, 256], f32, space="PSUM", tag="acc")
                for t in range(T_W):
                    nc.tensor.matmul(out=ps_a1[:, 0:MUL], lhsT=ohs[:, t, :],
                                     rhs=w1[:, t, :],
                                     start=(t == 0), stop=(t == T_W - 1))
                wY1 = wnp.tile([128, MUL], bf16)
                nc.vector.tensor_copy(out=wY1[:], in_=ps_a1[:, 0:MUL])
                fbfm1 = wnp.tile([MUL, kwin], bf16)
                fb1 = wnp.tile([128, MUL], f32)
                for t in range(T_W):
                    pg = pgth.tile([128, 512], f32, space="PSUM", tag="gth")
                    nc.tensor.matmul(out=pg[:, 0:MUL], lhsT=ohg[:, t, :],
                                     rhs=wY1[:], start=True, stop=True)
                    nc.vector.tensor_mul(out=fb1[:], in0=pg[:, 0:MUL],
                                          in1=V10[:, t, :])
                    pst = psml.tile([32, 128], f32, space="PSUM", tag="sml")
                    nc.tensor.transpose(out=pst[0:16, :], in_=fb1[:],
                                        identity=ident[:])
                    nc.vector.tensor_copy(
                        out=fbfm1[:, t * 128:(t + 1) * 128],
                        in_=pst[0:16, :])

                # ---- layer-1 ly1/ly2 + residual -> x2 ----
                x2 = bgp.tile([128, 2, kwin], bf16)
                mlp_block(x1, x2, 1, fbfm1, True)

                # ---- edge out + receiver scatter ----
                eo = wnp.tile([128, 1], f32)
                mt = wnp.tile([128, RWIN], bf16)
                for t in range(T_W):
                    tsl = slice(t * 128, (t + 1) * 128)
                    p1 = psml.tile([128, 32], f32, space="PSUM", tag="sm2")
                    for kc in range(2):
                        nc.tensor.matmul(
                            out=p1[:, 0:1], lhsT=x2[:, kc, tsl],
                            rhs=wb[:, OFF_WOUT + kc:OFF_WOUT + kc + 1],
                            start=(kc == 0), stop=(kc == 1))
                    nc.vector.tensor_mul(out=eo[:], in0=p1[:, 0:1],
                                          in1=u_pl[:, t0 + t, None])
                    nc.vector.tensor_mul(
                        out=mt[:], in0=rwt[:, t, :],
                        in1=eo[:].to_broadcast([128, RWIN]))
                    nc.tensor.matmul(out=ps_rcv[:], lhsT=rqs[:, t, :],
                                     rhs=mt[:],
                                     start=(w == 0 and t == 0),
                                     stop=(w == NW - 1 and t == T_W - 1))

            out_sb = gp.tile([128, RWIN], f32)
            nc.vector.tensor_copy(out=out_sb[:], in_=ps_rcv[:])
            nc.sync.dma_start(out=d_out[:], in_=out_sb[:])

    ET = mybir.EngineType
    eng_map = {ET.DVE: nc.vector, ET.Activation: nc.scalar,
               ET.Pool: nc.gpsimd, ET.PE: nc.tensor, ET.SP: nc.sync}

    def mk_carrier(eng):
        be = eng_map.get(eng)
        if be is None:
            return None
        w = be.wait_ge(carrier_sem, 0)
        ci = w.ins if hasattr(w, "ins") else w
        for bb in nc.m.functions[0].blocks:
            il = list(bb.instructions)
            if any(x is ci for x in il):
                bb.instructions = [x for x in il if x is not ci]
                break
        return ci

    made = _split_waits(nc, mybir, mk_carrier)
    print(f"split_waits: carriers={made}", flush=True)
    return nc


def kernel(**inputs):
    inputs = {k: np.asarray(v) for k, v in inputs.items()}
    kwin, in_maps, _ = make_in_maps(inputs)
    nc = build_graph(kwin)
    from concourse.bass_utils import run_bass_kernel_spmd
    res = run_bass_kernel_spmd(nc, in_maps, core_ids=list(range(NC)))
    out = np.zeros((128, RWIN), np.float64)
    for r in res.results:
        out += np.asarray(r["out"], np.float64)
    # node n = hi*128 + lo stored at [lo, hi]
    return np.ascontiguousarray(out.T.reshape(N, 1)).astype(np.float32)


# revision 43
# speedup vs baseline: 6.1321x; 1.0421x over previous
"""Allegro-style GNN message passing on 8 TRN2 NeuronCores.

Strategy (v2 — minimal host->device bytes):
- Host: shard edges by SENDER node range (1024 nodes/core) -> sender
  segment-sums are fully core-local (no cross-core collectives).
  Within a core, group edges by 128-node sender windows; pad each
  (core, window) group to a common K_WIN with dummy edges (d=2 -> u=0 ->
  zero contribution).
- Inputs per core are just 3 packed blobs (~1.4 MB total): u8 index
  planes (sender-local / receiver-lo / receiver-hi), an f32 blob
  (edge vectors + biases + wcol), and a bf16 blob (node table +
  weights). One-hot scatter/gather matrices and endpoint-attribute
  gathers are built ON DEVICE (iota + is_equal + PE transposes +
  one-hot matmuls) instead of being shipped from the host -- the axon
  PJRT tunnel moves ~40 MB/s, so the previous 17.7 MB/core of host-
  built one-hots dominated wall time.
- Layer algebra: Y[:,0] == 1, so layer-1 only needs a 16-wide
  segment-sum of w1; W_lsh[1] output is dead; V1 is only needed at
  component 0 => contraction with Ytil = Y * W_lsh[0][:,0].
- Receiver scatter: node id = hi*128+lo; per edge-tile matmul with lo
  one-hot lhsT and (hi one-hot * edge_out) rhs accumulates [128,64]
  partials in PSUM; host sums the 8 per-core partials (the unshard).
- 1/sqrt(AVG_NEIGH) and the 1/sqrt(2) residual scales are folded into
  weights on the host.
"""
import math
import sys

import numpy as np

sys.path.insert(0, "/opt/trn_rl_repo")

import ml_dtypes  # noqa: E402

try:
    import jax
    jax.config.update("jax_compilation_cache_dir", "/tmp/jax_pcache")
    jax.config.update("jax_persistent_cache_min_entry_size_bytes", -1)
    jax.config.update("jax_persistent_cache_min_compile_time_secs", 0.0)
except Exception:
    pass

BF16 = ml_dtypes.bfloat16
SIM_SILU = False   # CoreSim lacks Silu; emulate with Sigmoid*z when set

N, E, MUL, H, F = 8192, 131072, 16, 256, 16
NB = 8
INV = 1.0 / math.sqrt(16.0)
NC = 8
NPC = N // NC          # nodes per core
WIN = 128
NW = NPC // WIN        # windows per core
RWIN = N // WIN        # 64 receiver windows
SQ = math.sqrt(0.5)

# ---- bf16 weight-blob column layout [128, CB] (replicated) ----
OFF_WE0 = 0                      # we0 [40, 256] rows 0..39; wly1fb_0
#   shares these cols at rows 64..79
OFF_WE1 = OFF_WE0 + 256          # we1 2 x [128, 256]
OFF_WV0 = OFF_WE1 + 512          # wv0 2 x [128, 16]
OFF_WLW0 = OFF_WV0 + 32
OFF_WLW1 = OFF_WLW0 + 32
OFF_WLY1 = (OFF_WLW1 + 32, OFF_WLW1 + 32 + 512)
OFF_WLY2 = (OFF_WLY1[1] + 512, OFF_WLY1[1] + 512 + 512)
OFF_WOUT = OFF_WLY2[1] + 512     # wout 2 x [128, 1]
OFF_FB1 = OFF_WOUT + 2           # wly1fb_1 [16, 256] (rows 0..15)
CB = OFF_FB1 + 256
# node table rides in the int8 blob: nat [128,1024] cols f*64+hi, then
# snat [128,128] cols w*16+f; dequantized on device by nat_scale
NAT8 = 1024 + 128
VSCALE = 2.0 ** -14              # int16 fixed-point scale for vectors


def _host_shard(vectors, senders, receivers):
    """Group edges by (core, sender-window); pad to common K_WIN."""
    core = senders // NPC
    win = (senders % NPC) // WIN
    key = core * NW + win
    order = np.argsort(key, kind="stable")
    counts = np.bincount(key, minlength=NC * NW)
    kwin = int(((counts.max() + 127) // 128) * 128)
    starts = np.zeros(NC * NW + 1, np.int64)
    np.cumsum(counts, out=starts[1:])

    EP = NW * kwin
    shards = []
    for c in range(NC):
        v16 = np.zeros((EP, 3), np.int16)
        v16[:, 0] = 24576              # dummy edge: d = 1.5 -> u = 0
        sl = np.zeros(EP, np.int8)     # sender local-in-window
        rlo = np.zeros(EP, np.int8)
        rhi = np.zeros(EP, np.int8)
        for w in range(NW):
            g = c * NW + w
            eid = order[starts[g]:starts[g + 1]]
            o = w * kwin
            n_e = len(eid)
            v16[o:o + n_e] = np.round(vectors[eid] / VSCALE).astype(np.int16)
            sl[o:o + n_e] = (senders[eid] - (c * NPC + w * WIN)).astype(np.int8)
            rlo[o:o + n_e] = (receivers[eid] % 128).astype(np.int8)
            rhi[o:o + n_e] = (receivers[eid] // 128).astype(np.int8)
        shards.append((v16, sl, rlo, rhi))
    return kwin, shards


def _plane(a, T_ALL):
    """[EP] or [EP, k] -> plane layout [128, T_ALL*(k)] with e = t*128+p."""
    if a.ndim == 1:
        return np.ascontiguousarray(a.reshape(T_ALL, 128).T)
    # [EP, k] -> [128, k*T_ALL] with component-major column groups
    k = a.shape[1]
    p = a.reshape(T_ALL, 128, k).transpose(2, 1, 0)     # [k, 128, T_ALL]
    return np.ascontiguousarray(p.reshape(k * 128, T_ALL)).reshape(k, 128, T_ALL)


def _prep_weights(i):
    """Fold INV and residual 1/sqrt(2) scales into weights (f32)."""
    w = {}
    w["we0"] = i["W_e0"]                                       # [40,256]
    w["we1"] = i["W_e1"]
    w["wv0"] = i["W_v0"]
    w["wlw0"] = i["W_lw"][0] * INV
    w["wlw1"] = i["W_lw"][1] * INV * SQ                        # x1 = sq*x1'
    wly1_1 = i["W_ly1"][1].copy()
    wly1_1[:H] *= SQ                                           # x rows scaled
    w["wly1_0"] = i["W_ly1"][0]
    w["wly1_1"] = wly1_1
    w["wly2_0"] = i["W_ly2"][0]
    w["wly2_1"] = i["W_ly2"][1]
    w["wout"] = i["W_out"] * INV * 0.5                         # x2 = .5*x2'
    return w


def _pack_blobw(i):
    """Replicated bf16 weight blob [128, CB]."""
    w = _prep_weights(i)
    blob = np.zeros((128, CB), np.float32)
    # rhs row order is [snd attrs(16), rcv attrs(16), bessel(8)] so the
    # on-device copies land on legal partition offsets (0 and 32)
    blob[0:40, OFF_WE0:OFF_WE0 + 256] = np.vstack([w["we0"][8:40],
                                                   w["we0"][0:8]])
    blob[64:80, OFF_WE0:OFF_WE0 + 256] = w["wly1_0"][256:272]
    blob[0:16, OFF_FB1:OFF_FB1 + 256] = w["wly1_1"][256:272]
    for kc in range(2):
        s = slice(kc * 128, (kc + 1) * 128)
        blob[:, OFF_WE1 + kc * 256:OFF_WE1 + (kc + 1) * 256] = w["we1"][s]
        blob[:, OFF_WV0 + kc * 16:OFF_WV0 + (kc + 1) * 16] = w["wv0"][s]
        blob[:, OFF_WLW0 + kc * 16:OFF_WLW0 + (kc + 1) * 16] = w["wlw0"][s]
        blob[:, OFF_WLW1 + kc * 16:OFF_WLW1 + (kc + 1) * 16] = w["wlw1"][s]
        blob[:, OFF_WOUT + kc:OFF_WOUT + kc + 1] = w["wout"][s]
    for l in range(2):
        m = w[f"wly1_{l}"]
        for kc in range(2):
            s = slice(kc * 128, (kc + 1) * 128)
            blob[:, OFF_WLY1[l] + kc * 256:OFF_WLY1[l] + (kc + 1) * 256] = m[s]
            blob[:, OFF_WLY2[l] + kc * 256:OFF_WLY2[l] + (kc + 1) * 256] = \
                w[f"wly2_{l}"][s]
    return blob.astype(BF16)


def make_in_maps(inputs):
    kwin, shards = _host_shard(inputs["vectors"], inputs["senders"],
                               inputs["receivers"])
    EP = NW * kwin
    T_ALL = EP // 128
    bias_list = [inputs["b_e0"], inputs["b_e1"],
                 inputs["b_ly1"][0], inputs["b_ly1"][1],
                 inputs["b_ly2"][0], inputs["b_ly2"][1]]
    wcol = inputs["W_lsh"][0][:, 0]                            # [16]
    blobw = _pack_blobw(inputs)
    na = inputs["node_attrs"]                                  # [N, F]
    nat_scale = float(np.abs(na).max() / 127.0)
    naq = np.round(na / nat_scale).clip(-127, 127).astype(np.int8)
    # nat8[lo, f*64+hi] = naq[hi*128+lo, f]
    nat = naq.reshape(RWIN, 128, F).transpose(1, 2, 0).reshape(128, 1024)
    bf = np.zeros((128, 28), np.float32)
    for i, b in enumerate(bias_list):
        bf[:, 2 * i] = b[0:128]
        bf[:, 2 * i + 1] = b[128:256]
    bf[:, 12:28] = np.tile(wcol.reshape(1, 16), (128, 1))
    in_maps = []
    dbg = []
    for c in range(NC):
        v16, sl, rlo, rhi = shards[c]
        b8 = np.empty((128, 3 * T_ALL + NAT8), np.int8)
        b8[:, 0:T_ALL] = _plane(sl, T_ALL)
        b8[:, T_ALL:2 * T_ALL] = _plane(rlo, T_ALL)
        b8[:, 2 * T_ALL:3 * T_ALL] = _plane(rhi, T_ALL)
        b8[:, 3 * T_ALL:3 * T_ALL + 1024] = nat
        # snat[lo, w*16+f] = naq[(c*8+w)*128+lo, f]
        sn = naq.reshape(RWIN, 128, F)[c * NW:(c + 1) * NW]    # [w, lo, f]
        b8[:, 3 * T_ALL + 1024:] = sn.transpose(1, 0, 2).reshape(128, 128)
        b16 = np.empty((128, 3 * T_ALL), np.int16)
        vp = _plane(v16, T_ALL)                                # [3,128,T]
        b16[:, 0:T_ALL] = vp[0]
        b16[:, T_ALL:2 * T_ALL] = vp[1]
        b16[:, 2 * T_ALL:3 * T_ALL] = vp[2]
        in_maps.append({"blob8": b8, "blob16": b16, "blobf": bf,
                        "blobw": blobw})
        dbg.append(dict(vec=v16.astype(np.float32) * VSCALE,
                        sl=sl, rlo=rlo, rhi=rhi))
    return kwin, nat_scale, in_maps, dbg


_CAP_SKIP = {"InstEventSemaphore", "InstBranch", "InstNop",
             "InstCollectiveCompute"}
_CAP_LIMITS = {}


def _split_waits(nc, mybir, mk_carrier, limit=1):
    """Walrus codegen allows only 1 embedded sem-wait on compute
    instructions.  For each instruction with more, strip the extras onto
    freshly created same-engine carrier instructions inserted directly
    before it (engines are in-order, so this preserves semantics)."""
    f = nc.m.functions[0]
    made = 0
    for bb in f.blocks:
        insts = list(bb.instructions)
        plan = []          # (index, [carrier insts])
        for i, inst in enumerate(insts):
            tname = type(inst).__name__
            si = inst.sync_info
            nwait = len(si.on_wait) if (si and si.on_wait) else 0
            lim = _CAP_LIMITS.get(tname, limit)
            if tname in _CAP_SKIP or nwait <= lim:
                continue
            waits = list(si.on_wait)
            extras, keep = waits[:-lim], waits[-lim:]
            carriers = []
            for wt in extras:
                ci = mk_carrier(inst.engine)
                if ci is None:
                    keep.insert(0, wt)
                    continue
                ci.sync_info = mybir.SyncInfo(on_wait=[wt], on_update=[])
                carriers.append(ci)
                made += 1
            inst.sync_info = mybir.SyncInfo(on_wait=keep,
                                            on_update=si.on_update)
            if carriers:
                plan.append((i, carriers))
        if plan:
            new = []
            pmap = dict(plan)
            for i, inst in enumerate(insts):
                if i in pmap:
                    new.extend(pmap[i])
                new.append(inst)
            bb.instructions = new
    return made


def build_graph(kwin, nat_scale):
    from concourse import bass, mybir
    from concourse.masks import make_identity
    from concourse.tile import TileContext

    EP = NW * kwin
    T_ALL = EP // 128
    T_W = kwin // 128
    NCH = (kwin + 511) // 512      # free chunks per window

    f32 = mybir.dt.float32
    bf16 = mybir.dt.bfloat16
    i32 = mybir.dt.int32
    i8 = mybir.dt.int8
    i16 = mybir.dt.int16
    AX = mybir.AxisListType.X
    OP = mybir.AluOpType
    AF = mybir.ActivationFunctionType

    nc = bass.Bass()
    carrier_sem_cm = nc.semaphore("carrier_sem")
    carrier_sem = carrier_sem_cm.__enter__()
    dp = nc.declare_dram_parameter
    d_b8 = dp("blob8", [128, 3 * T_ALL + NAT8], i8, isOutput=False)
    d_b16 = dp("blob16", [128, 3 * T_ALL], i16, isOutput=False)
    d_bf = dp("blobf", [128, 28], f32, isOutput=False)
    d_bw = dp("blobw", [128, CB], bf16, isOutput=False)
    d_out = dp("out", [128, RWIN], f32, isOutput=True)

    with TileContext(nc) as tc:
        with (
            tc.tile_pool(name="glob", bufs=1) as gp,
            tc.tile_pool(name="wgt", bufs=1) as wp,
            tc.tile_pool(name="win", bufs=2) as wnp,
            tc.tile_pool(name="big", bufs=1) as bgp,
            tc.tile_pool(name="sml", bufs=3) as sp,
            tc.tile_pool(name="ps_mlp", bufs=2, space="PSUM") as pmlp,
            tc.tile_pool(name="ps_acc", bufs=1, space="PSUM") as pacc,
            tc.tile_pool(name="ps_gth", bufs=1, space="PSUM") as pgth,
            tc.tile_pool(name="ps_sml", bufs=1, space="PSUM") as psml,
            tc.tile_pool(name="ps_rcv", bufs=1, space="PSUM") as prcv,
        ):
            # ---------------- blobs to SBUF ----------------
            wb = wp.tile([128, CB], bf16, tag="wb")
            nc.sync.dma_start(out=wb[:], in_=d_bw[:])
            fbt = wp.tile([128, 28], f32, tag="fbt")
            nc.sync.dma_start(out=fbt[:], in_=d_bf[:])
            i8t = wp.tile([128, 3 * T_ALL + NAT8], i8, tag="i8t")
            nc.sync.dma_start(out=i8t[:], in_=d_b8[:])
            b16t = wp.tile([128, 3 * T_ALL], i16, tag="b16t")
            nc.sync.dma_start(out=b16t[:], in_=d_b16[:])
            slf = wp.tile([128, T_ALL], f32, tag="slf")
            rlof = wp.tile([128, T_ALL], f32, tag="rlof")
            rhif = wp.tile([128, T_ALL], f32, tag="rhif")
            nc.vector.tensor_copy(out=slf[:], in_=i8t[:, 0:T_ALL])
            nc.vector.tensor_copy(out=rlof[:], in_=i8t[:, T_ALL:2 * T_ALL])
            nc.vector.tensor_copy(out=rhif[:], in_=i8t[:, 2 * T_ALL:3 * T_ALL])
            # dequantized node table (nat 1024 cols + snat 128 cols)
            natbf = wp.tile([128, NAT8], bf16, tag="natbf")
            nc.vector.tensor_scalar(
                out=natbf[:], in0=i8t[:, 3 * T_ALL:3 * T_ALL + NAT8],
                scalar1=float(nat_scale), scalar2=None, op0=OP.mult)

            ident = wp.tile([128, 128], f32, tag="ident")
            make_identity(nc, ident[:])
            identb = wp.tile([128, 128], bf16, tag="identb")
            make_identity(nc, identb[:])
            it32 = wp.tile([128, 128], i32, tag="it32")
            nc.gpsimd.iota(out=it32[:], pattern=[[1, 128]], base=0,
                           channel_multiplier=0)
            iof = wp.tile([128, 128], f32, tag="iof")
            nc.vector.tensor_copy(out=iof[:], in_=it32[:])
            ones_bf = wp.tile([1, 128], bf16, tag="ones")
            nc.gpsimd.memset(ones_bf[:], 1.0)

            # views into the blobs
            def bias(i, hc):
                return fbt[:, 2 * i + hc, None]
            wcol = fbt[:, 12:28]
            natv = natbf[:, 0:1024]

            # ---------------- edge-scalar stage (planes [128,T_ALL]) ----
            ta = gp.tile([128, T_ALL], f32)
            tb = gp.tile([128, T_ALL], f32)
            ts = nc.vector.tensor_scalar
            act = nc.scalar.activation
            # dequantize int16 fixed-point vectors
            vx = gp.tile([128, T_ALL], f32)
            vy = gp.tile([128, T_ALL], f32)
            vz = gp.tile([128, T_ALL], f32)
            ts(out=vx[:], in0=b16t[:, 0:T_ALL], scalar1=VSCALE,
               scalar2=None, op0=OP.mult)
            ts(out=vy[:], in0=b16t[:, T_ALL:2 * T_ALL], scalar1=VSCALE,
               scalar2=None, op0=OP.mult)
            ts(out=vz[:], in0=b16t[:, 2 * T_ALL:3 * T_ALL], scalar1=VSCALE,
               scalar2=None, op0=OP.mult)

            def silu_act(out, ps_in, bias_ap):
                if not SIM_SILU:
                    act(out=out, in_=ps_in, func=AF.Silu, bias=bias_ap)
                else:
                    pp = ps_in.shape[0]
                    sg = bgp.tile([128, 512], f32, tag="simsilu")
                    zz_ = bgp.tile([128, 512], f32, tag="simsilu2")
                    cw_ = ps_in.shape[-1]
                    act(out=sg[:pp, :cw_], in_=ps_in, func=AF.Sigmoid,
                        bias=bias_ap)
                    nc.vector.tensor_scalar(out=zz_[:pp, :cw_], in0=ps_in,
                                            scalar1=bias_ap, scalar2=None,
                                            op0=OP.add)
                    nc.vector.tensor_mul(out=out, in0=sg[:pp, :cw_],
                                         in1=zz_[:pp, :cw_])
            d_pl = gp.tile([128, T_ALL], f32)
            nc.vector.tensor_mul(out=ta[:], in0=vx[:], in1=vx[:])
            nc.vector.tensor_mul(out=tb[:], in0=vy[:], in1=vy[:])
            nc.vector.tensor_add(out=ta[:], in0=ta[:], in1=tb[:])
            nc.vector.tensor_mul(out=tb[:], in0=vz[:], in1=vz[:])
            nc.vector.tensor_add(out=ta[:], in0=ta[:], in1=tb[:])
            act(out=d_pl[:], in_=ta[:], func=AF.Sqrt)
            rinv = gp.tile([128, T_ALL], f32)
            nc.vector.reciprocal(out=rinv[:], in_=d_pl[:])
            ux = gp.tile([128, T_ALL], f32)
            uy = gp.tile([128, T_ALL], f32)
            uz = gp.tile([128, T_ALL], f32)
            nc.vector.tensor_mul(out=ux[:], in0=vx[:], in1=rinv[:])
            nc.vector.tensor_mul(out=uy[:], in0=vy[:], in1=rinv[:])
            nc.vector.tensor_mul(out=uz[:], in0=vz[:], in1=rinv[:])

            # envelope u = 1 + d^6*(-28 + 48d - 21d^2), zero for d >= 1
            u_pl = gp.tile([128, T_ALL], f32)
            nc.vector.tensor_mul(out=ta[:], in0=d_pl[:], in1=d_pl[:])   # d2
            nc.vector.tensor_mul(out=tb[:], in0=ta[:], in1=d_pl[:])     # d3
            nc.vector.tensor_mul(out=tb[:], in0=tb[:], in1=tb[:])       # d6
            ts(out=ta[:], in0=ta[:], scalar1=-21.0, scalar2=None, op0=OP.mult)
            tc_q = gp.tile([128, T_ALL], f32)
            ts(out=tc_q[:], in0=d_pl[:], scalar1=48.0, scalar2=-28.0,
               op0=OP.mult, op1=OP.add)
            nc.vector.tensor_add(out=ta[:], in0=ta[:], in1=tc_q[:])
            nc.vector.tensor_mul(out=tb[:], in0=tb[:], in1=ta[:])
            ts(out=tb[:], in0=tb[:], scalar1=1.0, scalar2=None, op0=OP.add)
            ts(out=ta[:], in0=d_pl[:], scalar1=1.0, scalar2=None,
               op0=OP.is_lt)
            nc.vector.tensor_mul(out=u_pl[:], in0=tb[:], in1=ta[:])

            # spherical harmonics Y [128, T_ALL, 16] f32
            Yt = gp.tile([128, T_ALL, 16], f32)
            s3 = 3.0 ** 0.5; s5 = 5.0 ** 0.5; s15 = 15.0 ** 0.5
            s7 = 7.0 ** 0.5
            c33 = (35.0 / 8.0) ** 0.5; c32 = 105.0 ** 0.5
            c31 = (21.0 / 8.0) ** 0.5
            xx = gp.tile([128, T_ALL], f32)
            yy = gp.tile([128, T_ALL], f32)
            zz = gp.tile([128, T_ALL], f32)
            xy = gp.tile([128, T_ALL], f32)
            nc.vector.tensor_mul(out=xx[:], in0=ux[:], in1=ux[:])
            nc.vector.tensor_mul(out=yy[:], in0=uy[:], in1=uy[:])
            nc.vector.tensor_mul(out=zz[:], in0=uz[:], in1=uz[:])
            nc.vector.tensor_mul(out=xy[:], in0=ux[:], in1=uy[:])
            ts(out=Yt[:, :, 0], in0=ux[:], scalar1=0.0, scalar2=1.0,
               op0=OP.mult, op1=OP.add)
            ts(out=Yt[:, :, 1], in0=ux[:], scalar1=s3, scalar2=None,
               op0=OP.mult)
            ts(out=Yt[:, :, 2], in0=uy[:], scalar1=s3, scalar2=None,
               op0=OP.mult)
            ts(out=Yt[:, :, 3], in0=uz[:], scalar1=s3, scalar2=None,
               op0=OP.mult)
            ts(out=Yt[:, :, 4], in0=xy[:], scalar1=s15, scalar2=None,
               op0=OP.mult)
            nc.vector.tensor_mul(out=ta[:], in0=uy[:], in1=uz[:])
            ts(out=Yt[:, :, 5], in0=ta[:], scalar1=s15, scalar2=None,
               op0=OP.mult)
            ts(out=Yt[:, :, 6], in0=zz[:], scalar1=1.5 * s5,
               scalar2=-0.5 * s5, op0=OP.mult, op1=OP.add)
            nc.vector.tensor_mul(out=tb[:], in0=ux[:], in1=uz[:])
            ts(out=Yt[:, :, 7], in0=tb[:], scalar1=s15, scalar2=None,
               op0=OP.mult)
            xmy = gp.tile([128, T_ALL], f32)
            nc.vector.tensor_sub(out=xmy[:], in0=xx[:], in1=yy[:])
            ts(out=Yt[:, :, 8], in0=xmy[:], scalar1=0.5 * s15, scalar2=None,
               op0=OP.mult)
            ts(out=ta[:], in0=xx[:], scalar1=3.0, scalar2=None, op0=OP.mult)
            nc.vector.tensor_sub(out=ta[:], in0=ta[:], in1=yy[:])
            nc.vector.tensor_mul(out=ta[:], in0=ta[:], in1=uy[:])
            ts(out=Yt[:, :, 9], in0=ta[:], scalar1=c33, scalar2=None,
               op0=OP.mult)
            nc.vector.tensor_mul(out=ta[:], in0=xy[:], in1=uz[:])
            ts(out=Yt[:, :, 10], in0=ta[:], scalar1=c32, scalar2=None,
               op0=OP.mult)
            ts(out=ta[:], in0=zz[:], scalar1=5.0, scalar2=-1.0,
               op0=OP.mult, op1=OP.add)
            nc.vector.tensor_mul(out=tb[:], in0=ta[:], in1=uy[:])
            ts(out=Yt[:, :, 11], in0=tb[:], scalar1=c31, scalar2=None,
               op0=OP.mult)
            nc.vector.tensor_mul(out=tb[:], in0=ta[:], in1=ux[:])
            ts(out=Yt[:, :, 13], in0=tb[:], scalar1=c31, scalar2=None,
               op0=OP.mult)
            nc.vector.tensor_mul(out=ta[:], in0=zz[:], in1=uz[:])
            ts(out=ta[:], in0=ta[:], scalar1=2.5 * s7, scalar2=None,
               op0=OP.mult)
            ts(out=tb[:], in0=uz[:], scalar1=1.5 * s7, scalar2=None,
               op0=OP.mult)
            nc.vector.tensor_sub(out=Yt[:, :, 12], in0=ta[:], in1=tb[:])
            nc.vector.tensor_mul(out=ta[:], in0=xmy[:], in1=uz[:])
            ts(out=Yt[:, :, 14], in0=ta[:], scalar1=0.5 * c32, scalar2=None,
               op0=OP.mult)
            ts(out=ta[:], in0=yy[:], scalar1=3.0, scalar2=None, op0=OP.mult)
            nc.vector.tensor_sub(out=ta[:], in0=xx[:], in1=ta[:])
            nc.vector.tensor_mul(out=ta[:], in0=ta[:], in1=ux[:])
            ts(out=Yt[:, :, 15], in0=ta[:], scalar1=c33, scalar2=None,
               op0=OP.mult)

            # bessel (range-reduced): besu [128, T_ALL, 8]
            besu = gp.tile([128, T_ALL, 8], f32)
            rs = gp.tile([128, T_ALL], f32)
            ts(out=rs[:], in0=rinv[:], scalar1=math.sqrt(2.0), scalar2=None,
               op0=OP.mult)
            mi = gp.tile([128, T_ALL], mybir.dt.int32)
            for k in range(1, NB + 1):
                ts(out=ta[:], in0=d_pl[:], scalar1=0.5 * k, scalar2=None,
                   op0=OP.mult)
                nc.vector.tensor_copy(out=mi[:], in_=ta[:])
                nc.vector.tensor_copy(out=tb[:], in_=mi[:])
                nc.vector.tensor_sub(out=ta[:], in0=ta[:], in1=tb[:])
                # ta = frac in (-0.5, 1) whether the cast rounds or truncates
                ts(out=tb[:], in0=ta[:], scalar1=0.5, scalar2=None,
                   op0=OP.is_gt)
                nc.vector.tensor_sub(out=ta[:], in0=ta[:], in1=tb[:])
                act(out=ta[:], in_=ta[:], func=AF.Sin, scale=2.0 * math.pi)
                nc.vector.tensor_mul(out=besu[:, :, k - 1], in0=ta[:],
                                      in1=rs[:])

            # ---------------- persistent receiver accumulator ----------
            ps_rcv = prcv.tile([128, RWIN], f32, space="PSUM")

            # ---------------- window loop ----------------
            for w in range(NW):
                t0 = w * T_W
                ohs = wnp.tile([128, T_W, 128], bf16)   # [e, n]
                ohg = wnp.tile([128, T_W, 128], bf16)   # [n, e]
                rqs = wnp.tile([128, T_W, 128], bf16)   # [e, lo]
                rqg = wnp.tile([128, T_W, 128], bf16)   # [lo, e]
                rwt = wnp.tile([128, T_W, RWIN], bf16)  # [e, hi]
                xfm = wnp.tile([40, kwin], bf16)        # snd(16)+rcv(16)+bes(8)
                ufm = wnp.tile([1, kwin], bf16)
                tt = nc.vector.tensor_tensor
                tsw = slice(t0, t0 + T_W)
                tt(out=ohs[:],
                   in0=slf[:, tsw, None].to_broadcast([128, T_W, 128]),
                   in1=iof[:, None, :].to_broadcast([128, T_W, 128]),
                   op=OP.is_equal)
                tt(out=rqs[:],
                   in0=rlof[:, tsw, None].to_broadcast([128, T_W, 128]),
                   in1=iof[:, None, :].to_broadcast([128, T_W, 128]),
                   op=OP.is_equal)
                tt(out=rwt[:],
                   in0=rhif[:, tsw, None].to_broadcast([128, T_W, RWIN]),
                   in1=iof[:, None, 0:RWIN].to_broadcast([128, T_W, RWIN]),
                   op=OP.is_equal)
                for t in range(T_W):
                    tg = t0 + t
                    csl = slice(t * 128, (t + 1) * 128)
                    ptr = psml.tile([128, 128], bf16, space="PSUM", tag="trn")
                    nc.tensor.transpose(out=ptr[:], in_=ohs[:, t, :],
                                        identity=identb[:])
                    nc.vector.tensor_copy(out=ohg[:, t, :], in_=ptr[:])
                    ptr2 = psml.tile([128, 128], bf16, space="PSUM", tag="trn")
                    nc.tensor.transpose(out=ptr2[:], in_=rqs[:, t, :],
                                        identity=identb[:])
                    nc.vector.tensor_copy(out=rqg[:, t, :], in_=ptr2[:])
                    # bessel + u feature-major
                    pst = psml.tile([32, 128], f32, space="PSUM", tag="sml")
                    nc.tensor.transpose(out=pst[0:8, :], in_=besu[:, tg, :],
                                        identity=ident[:])
                    nc.vector.tensor_copy(out=xfm[32:40, csl],
                                          in_=pst[0:8, :])
                    psu1 = psml.tile([32, 128], f32, space="PSUM", tag="sml")
                    nc.tensor.transpose(out=psu1[0:1, :],
                                        in_=u_pl[:, tg, None],
                                        identity=ident[:])
                    nc.vector.tensor_copy(out=ufm[:, csl], in_=psu1[0:1, :])
                    # endpoint-attr gather: sender (window-local one-hot)
                    gcmb = sp.tile([128, 32], f32, tag="gcmb")
                    psn = psml.tile([128, 32], f32, space="PSUM", tag="sm2")
                    nc.tensor.matmul(
                        out=psn[:, 0:16], lhsT=ohg[:, t, :],
                        rhs=natbf[:, 1024 + w * 16:1024 + (w + 1) * 16],
                        start=True, stop=True)
                    nc.vector.tensor_copy(out=gcmb[:, 0:16], in_=psn[:, 0:16])
                    # receiver: lo-gather matmul then hi-select
                    for c2 in range(2):
                        prg = pgth.tile([128, 512], f32, space="PSUM",
                                        tag="gth")
                        nc.tensor.matmul(
                            out=prg[:], lhsT=rqg[:, t, :],
                            rhs=natv[:, c2 * 512:(c2 + 1) * 512],
                            start=True, stop=True)
                        prod = sp.tile([128, 8, RWIN], f32, tag="rsel")
                        nc.vector.tensor_mul(
                            out=prod[:],
                            in0=prg[:].rearrange("p (a b) -> p a b", b=RWIN),
                            in1=rwt[:, t, None, :].to_broadcast(
                                [128, 8, RWIN]))
                        nc.vector.reduce_sum(
                            out=gcmb[:, 16 + c2 * 8:16 + (c2 + 1) * 8, None],
                            in_=prod[:], axis=AX)
                    ptg = psml.tile([32, 128], f32, space="PSUM", tag="sml")
                    nc.tensor.transpose(out=ptg[:], in_=gcmb[:],
                                        identity=ident[:])
                    nc.vector.tensor_copy(out=xfm[0:32, csl], in_=ptg[:])

                # broadcast u row -> [128, kwin] bf16
                ubc = bgp.tile([128, kwin], bf16)
                for ch in range(NCH):
                    c0 = ch * 512
                    c1 = min(kwin, c0 + 512)
                    psu = pmlp.tile([128, 512], f32, space="PSUM", tag="mlp")
                    nc.tensor.matmul(out=psu[:, :c1 - c0], lhsT=ones_bf[:],
                                     rhs=ufm[:, c0:c1],
                                     start=True, stop=True)
                    nc.vector.tensor_copy(out=ubc[:, c0:c1],
                                          in_=psu[:, :c1 - c0])

                # ---- edge MLP: x0 = u*silu(e1(silu(e0(bes,attrs)))) ----
                x0 = bgp.tile([128, 2, kwin], bf16)
                th = bgp.tile([128, 2, kwin], bf16)
                for ch in range(NCH):
                    c0 = ch * 512
                    c1 = min(kwin, c0 + 512)
                    cw = c1 - c0
                    for hc in range(2):
                        ps = pmlp.tile([128, 512], f32, space="PSUM", tag="mlp")
                        nc.tensor.matmul(
                            out=ps[:, :cw],
                            lhsT=wb[0:40, OFF_WE0 + hc * 128:
                                    OFF_WE0 + (hc + 1) * 128],
                            rhs=xfm[:, c0:c1], start=True, stop=True)
                        silu_act(th[:, hc, c0:c1], ps[:, :cw], bias(0, hc))
                for ch in range(NCH):
                    c0 = ch * 512
                    c1 = min(kwin, c0 + 512)
                    cw = c1 - c0
                    for hc in range(2):
                        ps = pmlp.tile([128, 512], f32, space="PSUM", tag="mlp")
                        for kc in range(2):
                            nc.tensor.matmul(
                                out=ps[:, :cw],
                                lhsT=wb[:, OFF_WE1 + kc * 256 + hc * 128:
                                        OFF_WE1 + kc * 256 + (hc + 1) * 128],
                                rhs=th[:, kc, c0:c1],
                                start=(kc == 0), stop=(kc == 1))
                        silu_act(x0[:, hc, c0:c1], ps[:, :cw], bias(1, hc))
                for hc in range(2):
                    nc.vector.tensor_mul(out=x0[:, hc, :], in0=x0[:, hc, :],
                                          in1=ubc[:])

                # ---- xv, w0 (edge-major [128,16] per tile) ----
                xv = wnp.tile([128, T_W, MUL], f32)
                w0 = wnp.tile([128, T_W, MUL], bf16)
                for t in range(T_W):
                    tsl = slice(t * 128, (t + 1) * 128)
                    p12 = psml.tile([128, 32], f32, space="PSUM", tag="sm2")
                    for kc in range(2):
                        nc.tensor.matmul(
                            out=p12[:, 0:16], lhsT=x0[:, kc, tsl],
                            rhs=wb[:, OFF_WV0 + kc * 16:OFF_WV0 + (kc + 1) * 16],
                            start=(kc == 0), stop=(kc == 1))
                    for kc in range(2):
                        nc.tensor.matmul(
                            out=p12[:, 16:32], lhsT=x0[:, kc, tsl],
                            rhs=wb[:, OFF_WLW0 + kc * 16:
                                    OFF_WLW0 + (kc + 1) * 16],
                            start=(kc == 0), stop=(kc == 1))
                    nc.vector.tensor_copy(out=xv[:, t, :], in_=p12[:, 0:16])
                    nc.vector.tensor_copy(out=w0[:, t, :], in_=p12[:, 16:32])

                # ---- layer-0 scatter: wY[n, m*16+i] ----
                ps_acc = pacc.tile([128, 256], f32, space="PSUM", tag="acc")
                for t in range(T_W):
                    v2 = sp.tile([128, MUL, 16], bf16, tag="v2")
                    nc.vector.tensor_mul(
                        out=v2[:],
                        in0=w0[:, t, :, None].to_broadcast([128, MUL, 16]),
                        in1=Yt[:, t0 + t, None, :].to_broadcast(
                            [128, MUL, 16]))
                    nc.tensor.matmul(
                        out=ps_acc[:],
                        lhsT=ohs[:, t, :],
                        rhs=v2[:].rearrange("p a b -> p (a b)"),
                        start=(t == 0), stop=(t == T_W - 1))
                wY = wnp.tile([128, 256], bf16)
                nc.vector.tensor_copy(out=wY[:], in_=ps_acc[:])

                # ---- gather + Ytil contraction + feedback ----
                # fbfm lives at partitions 64..79 so its matmul shares the
                # base partition of the packed wly1fb_0 weights
                V10 = wnp.tile([128, T_W, MUL], f32)
                fbfm = wnp.tile([80, kwin], bf16)
                prod = wnp.tile([128, MUL, 16], f32)
                ytil = wnp.tile([128, MUL], f32)
                Ssb = wnp.tile([128, MUL], f32)
                fb = wnp.tile([128, MUL], f32)
                for t in range(T_W):
                    pgf = pgth.tile([128, 512], f32, space="PSUM", tag="gth")
                    pg = pgf[:, 0:256]
                    nc.tensor.matmul(out=pg, lhsT=ohg[:, t, :], rhs=wY[:],
                                     start=True, stop=True)
                    pg3 = pg.rearrange("p (a b) -> p a b", b=16)
                    nc.vector.tensor_mul(out=ytil[:], in0=Yt[:, t0 + t, :],
                                          in1=wcol)
                    nc.vector.tensor_mul(
                        out=prod[:], in0=pg3,
                        in1=ytil[:, None, :].to_broadcast([128, MUL, 16]))
                    nc.vector.reduce_sum(out=Ssb[:, :, None], in_=prod[:],
                                         axis=AX)
                    nc.vector.tensor_mul(out=V10[:, t, :], in0=Ssb[:],
                                          in1=xv[:, t, :])
                    nc.vector.tensor_mul(out=fb[:], in0=pg3[:, :, 0],
                                          in1=xv[:, t, :])
                    pst = psml.tile([32, 128], f32, space="PSUM", tag="sml")
                    nc.tensor.transpose(out=pst[0:16, :], in_=fb[:],
                                        identity=ident[:])
                    nc.vector.tensor_copy(
                        out=fbfm[64:80, t * 128:(t + 1) * 128],
                        in_=pst[0:16, :])

                # ---- layer-0 ly1/ly2 + residual -> x1 ----
                x1 = bgp.tile([128, 2, kwin], bf16)

                def mlp_block(xin, xout, l, fbrow, resid_sq2):
                    b1 = OFF_WLY1[l]
                    # wly1fb: layer 0 at rows 64..79 of the WE0 cols,
                    # layer 1 at rows 0..15 of its own FB1 cols
                    fbp, fbc = (64, OFF_WE0) if l == 0 else (0, OFF_FB1)
                    b2 = OFF_WLY2[l]
                    ty = bgp.tile([128, 2, kwin], bf16)
                    for ch in range(NCH):
                        c0 = ch * 512
                        c1 = min(kwin, c0 + 512)
                        cw = c1 - c0
                        for hc in range(2):
                            hs = slice(hc * 128, (hc + 1) * 128)
                            ps = pmlp.tile([128, 512], f32, space="PSUM",
                                           tag="mlp")
                            for kc in range(2):
                                nc.tensor.matmul(
                                    out=ps[:, :cw],
                                    lhsT=wb[:, b1 + kc * 256 + hc * 128:
                                            b1 + kc * 256 + (hc + 1) * 128],
                                    rhs=xin[:, kc, c0:c1],
                                    start=(kc == 0), stop=False)
                            nc.tensor.matmul(
                                out=ps[:, :cw],
                                lhsT=wb[fbp:fbp + 16, fbc + hc * 128:
                                        fbc + (hc + 1) * 128],
                                rhs=fbrow[fbp:fbp + 16, c0:c1],
                                start=False, stop=True)
                            silu_act(ty[:, hc, c0:c1], ps[:, :cw],
                                     bias(2 + l, hc))
                    ty2 = bgp.tile([128, 2, kwin], bf16)
                    for ch in range(NCH):
                        c0 = ch * 512
                        c1 = min(kwin, c0 + 512)
                        cw = c1 - c0
                        for hc in range(2):
                            ps = pmlp.tile([128, 512], f32, space="PSUM",
                                           tag="mlp")
                            for kc in range(2):
                                nc.tensor.matmul(
                                    out=ps[:, :cw],
                                    lhsT=wb[:, b2 + kc * 256 + hc * 128:
                                            b2 + kc * 256 + (hc + 1) * 128],
                                    rhs=ty[:, kc, c0:c1],
                                    start=(kc == 0), stop=(kc == 1))
                            silu_act(ty2[:, hc, c0:c1], ps[:, :cw],
                                     bias(4 + l, hc))
                    # x_out' = x_in' + s * u * y   (s = 1 or sqrt(2))
                    for hc in range(2):
                        nc.vector.tensor_mul(out=ty2[:, hc, :],
                                              in0=ty2[:, hc, :], in1=ubc[:])
                        if resid_sq2:
                            ts(out=ty2[:, hc, :], in0=ty2[:, hc, :],
                               scalar1=math.sqrt(2.0), scalar2=None,
                               op0=OP.mult)
                        nc.vector.tensor_add(out=xout[:, hc, :],
                                             in0=xin[:, hc, :],
                                             in1=ty2[:, hc, :])

                mlp_block(x0, x1, 0, fbfm, False)

                # ---- layer 1: w1, 16-wide scatter/gather, feedback ----
                w1 = wnp.tile([128, T_W, MUL], bf16)
                for t in range(T_W):
                    tsl = slice(t * 128, (t + 1) * 128)
                    p1 = psml.tile([128, 32], f32, space="PSUM", tag="sm2")
                    for kc in range(2):
                        nc.tensor.matmul(
                            out=p1[:, 0:MUL], lhsT=x1[:, kc, tsl],
                            rhs=wb[:, OFF_WLW1 + kc * 16:
                                    OFF_WLW1 + (kc + 1) * 16],
                            start=(kc == 0), stop=(kc == 1))
                    nc.vector.tensor_copy(out=w1[:, t, :], in_=p1[:, 0:MUL])
                ps_a1 = pacc.tile([128, 256], f32, space="PSUM", tag="acc")
                for t in range(T_W):
                    nc.tensor.matmul(out=ps_a1[:, 0:MUL], lhsT=ohs[:, t, :],
                                     rhs=w1[:, t, :],
                                     start=(t == 0), stop=(t == T_W - 1))
                wY1 = wnp.tile([128, MUL], bf16)
                nc.vector.tensor_copy(out=wY1[:], in_=ps_a1[:, 0:MUL])
                fbfm1 = wnp.tile([MUL, kwin], bf16)
                fb1 = wnp.tile([128, MUL], f32)
                for t in range(T_W):
                    pg = pgth.tile([128, 512], f32, space="PSUM", tag="gth")
                    nc.tensor.matmul(out=pg[:, 0:MUL], lhsT=ohg[:, t, :],
                                     rhs=wY1[:], start=True, stop=True)
                    nc.vector.tensor_mul(out=fb1[:], in0=pg[:, 0:MUL],
                                          in1=V10[:, t, :])
                    pst = psml.tile([32, 128], f32, space="PSUM", tag="sml")
                    nc.tensor.transpose(out=pst[0:16, :], in_=fb1[:],
                                        identity=ident[:])
                    nc.vector.tensor_copy(
                        out=fbfm1[:, t * 128:(t + 1) * 128],
                        in_=pst[0:16, :])

                # ---- layer-1 ly1/ly2 + residual -> x2 ----
                x2 = bgp.tile([128, 2, kwin], bf16)
                mlp_block(x1, x2, 1, fbfm1, True)

                # ---- edge out + receiver scatter ----
                eo = wnp.tile([128, 1], f32)
                mt = wnp.tile([128, RWIN], bf16)
                for t in range(T_W):
                    tsl = slice(t * 128, (t + 1) * 128)
                    p1 = psml.tile([128, 32], f32, space="PSUM", tag="sm2")
                    for kc in range(2):
                        nc.tensor.matmul(
                            out=p1[:, 0:1], lhsT=x2[:, kc, tsl],
                            rhs=wb[:, OFF_WOUT + kc:OFF_WOUT + kc + 1],
                            start=(kc == 0), stop=(kc == 1))
                    nc.vector.tensor_mul(out=eo[:], in0=p1[:, 0:1],
                                          in1=u_pl[:, t0 + t, None])
                    nc.vector.tensor_mul(
                        out=mt[:], in0=rwt[:, t, :],
                        in1=eo[:].to_broadcast([128, RWIN]))
                    nc.tensor.matmul(out=ps_rcv[:], lhsT=rqs[:, t, :],
                                     rhs=mt[:],
                                     start=(w == 0 and t == 0),
                                     stop=(w == NW - 1 and t == T_W - 1))

            out_sb = gp.tile([128, RWIN], f32)
            nc.vector.tensor_copy(out=out_sb[:], in_=ps_rcv[:])
            nc.sync.dma_start(out=d_out[:], in_=out_sb[:])

    ET = mybir.EngineType
    eng_map = {ET.DVE: nc.vector, ET.Activation: nc.scalar,
               ET.Pool: nc.gpsimd, ET.PE: nc.tensor, ET.SP: nc.sync}

    def mk_carrier(eng):
        be = eng_map.get(eng)
        if be is None:
            return None
        w = be.wait_ge(carrier_sem, 0)
        ci = w.ins if hasattr(w, "ins") else w
        for bb in nc.m.functions[0].blocks:
            il = list(bb.instructions)
            if any(x is ci for x in il):
                bb.instructions = [x for x in il if x is not ci]
                break
        return ci

    made = _split_waits(nc, mybir, mk_carrier)
    print(f"split_waits: carriers={made}", flush=True)
    return nc


def kernel(**inputs):
    inputs = {k: np.asarray(v) for k, v in inputs.items()}
    kwin, nat_scale, in_maps, _ = make_in_maps(inputs)
    nc = build_graph(kwin, nat_scale)
    from concourse.bass_utils import run_bass_kernel_spmd
    res = run_bass_kernel_spmd(nc, in_maps, core_ids=list(range(NC)))
    out = np.zeros((128, RWIN), np.float64)
    for r in res.results:
        out += np.asarray(r["out"], np.float64)
    # node n = hi*128 + lo stored at [lo, hi]
    return np.ascontiguousarray(out.T.reshape(N, 1)).astype(np.float32)


# revision 49
# speedup vs baseline: 6.8855x; 1.1229x over previous
"""Allegro-style GNN message passing on 8 TRN2 NeuronCores.

Strategy (v2 — minimal host->device bytes):
- Host: shard edges by SENDER node range (1024 nodes/core) -> sender
  segment-sums are fully core-local (no cross-core collectives).
  Within a core, group edges by 128-node sender windows; pad each
  (core, window) group to a common K_WIN with dummy edges (d=2 -> u=0 ->
  zero contribution).
- Inputs per core are just 3 packed blobs (~1.4 MB total): u8 index
  planes (sender-local / receiver-lo / receiver-hi), an f32 blob
  (edge vectors + biases + wcol), and a bf16 blob (node table +
  weights). One-hot scatter/gather matrices and endpoint-attribute
  gathers are built ON DEVICE (iota + is_equal + PE transposes +
  one-hot matmuls) instead of being shipped from the host -- the axon
  PJRT tunnel moves ~40 MB/s, so the previous 17.7 MB/core of host-
  built one-hots dominated wall time.
- Layer algebra: Y[:,0] == 1, so layer-1 only needs a 16-wide
  segment-sum of w1; W_lsh[1] output is dead; V1 is only needed at
  component 0 => contraction with Ytil = Y * W_lsh[0][:,0].
- Receiver scatter: node id = hi*128+lo; per edge-tile matmul with lo
  one-hot lhsT and (hi one-hot * edge_out) rhs accumulates [128,64]
  partials in PSUM; host sums the 8 per-core partials (the unshard).
- 1/sqrt(AVG_NEIGH) and the 1/sqrt(2) residual scales are folded into
  weights on the host.
"""
import math
import sys

import numpy as np

sys.path.insert(0, "/opt/trn_rl_repo")

import ml_dtypes  # noqa: E402

try:
    import jax
    jax.config.update("jax_compilation_cache_dir", "/tmp/jax_pcache")
    jax.config.update("jax_persistent_cache_min_entry_size_bytes", -1)
    jax.config.update("jax_persistent_cache_min_compile_time_secs", 0.0)
except Exception:
    pass

BF16 = ml_dtypes.bfloat16
SIM_SILU = False   # CoreSim lacks Silu; emulate with Sigmoid*z when set

N, E, MUL, H, F = 8192, 131072, 16, 256, 16
NB = 8
INV = 1.0 / math.sqrt(16.0)
NC = 8
NPC = N // NC          # nodes per core
WIN = 128
NW = NPC // WIN        # windows per core
RWIN = N // WIN        # 64 receiver windows
SQ = math.sqrt(0.5)

# ---- bf16 weight-blob column layout [128, CB] (replicated) ----
OFF_WE0 = 0                      # we0 [40, 256] rows 0..39; wly1fb_0
#   shares these cols at rows 64..79
OFF_WE1 = OFF_WE0 + 256          # we1 2 x [128, 256]
OFF_WV0 = OFF_WE1 + 512          # wv0 2 x [128, 16]
OFF_WLW0 = OFF_WV0 + 32
OFF_WLW1 = OFF_WLW0 + 32
OFF_WLY1 = (OFF_WLW1 + 32, OFF_WLW1 + 32 + 512)
OFF_WLY2 = (OFF_WLY1[1] + 512, OFF_WLY1[1] + 512 + 512)
OFF_WOUT = OFF_WLY2[1] + 512     # wout 2 x [128, 1]
OFF_FB1 = OFF_WOUT + 2           # wly1fb_1 [16, 256] (rows 0..15)
CB = OFF_FB1 + 256
OFF_MISC = CB                    # biases(12)+wcol(16) as bf16 hi then lo
CBX = CB + 56                    # total blobw cols
# node table rides in the int8 blob: nat [128,1024] cols f*64+hi, then
# snat [128,128] cols w*16+f; dequantized on device by nat_scale.
# vectors ride as int16 fixed-point split into hi/lo int8 planes.
NAT8 = 1024 + 128
VSCALE = 2.0 ** -14              # int16 fixed-point scale for vectors


def _host_shard(vectors, senders, receivers):
    """Group edges by (core, sender-window); pad to common K_WIN."""
    core = senders // NPC
    win = (senders % NPC) // WIN
    key = core * NW + win
    order = np.argsort(key, kind="stable")
    counts = np.bincount(key, minlength=NC * NW)
    kwin = int(((counts.max() + 127) // 128) * 128)
    starts = np.zeros(NC * NW + 1, np.int64)
    np.cumsum(counts, out=starts[1:])

    EP = NW * kwin
    shards = []
    for c in range(NC):
        v16 = np.zeros((EP, 3), np.int16)
        v16[:, 0] = 24576              # dummy edge: d = 1.5 -> u = 0
        sl = np.zeros(EP, np.int8)     # sender local-in-window
        rlo = np.zeros(EP, np.int8)
        rhi = np.zeros(EP, np.int8)
        for w in range(NW):
            g = c * NW + w
            eid = order[starts[g]:starts[g + 1]]
            o = w * kwin
            n_e = len(eid)
            v16[o:o + n_e] = np.round(vectors[eid] / VSCALE).astype(np.int16)
            sl[o:o + n_e] = (senders[eid] - (c * NPC + w * WIN)).astype(np.int8)
            rlo[o:o + n_e] = (receivers[eid] % 128).astype(np.int8)
            rhi[o:o + n_e] = (receivers[eid] // 128).astype(np.int8)
        shards.append((v16, sl, rlo, rhi))
    return kwin, shards


def _plane(a, T_ALL):
    """[EP] or [EP, k] -> plane layout [128, T_ALL*(k)] with e = t*128+p."""
    if a.ndim == 1:
        return np.ascontiguousarray(a.reshape(T_ALL, 128).T)
    # [EP, k] -> [128, k*T_ALL] with component-major column groups
    k = a.shape[1]
    p = a.reshape(T_ALL, 128, k).transpose(2, 1, 0)     # [k, 128, T_ALL]
    return np.ascontiguousarray(p.reshape(k * 128, T_ALL)).reshape(k, 128, T_ALL)


def _prep_weights(i):
    """Fold INV and residual 1/sqrt(2) scales into weights (f32)."""
    w = {}
    w["we0"] = i["W_e0"]                                       # [40,256]
    w["we1"] = i["W_e1"]
    w["wv0"] = i["W_v0"]
    w["wlw0"] = i["W_lw"][0] * INV
    w["wlw1"] = i["W_lw"][1] * INV * SQ                        # x1 = sq*x1'
    wly1_1 = i["W_ly1"][1].copy()
    wly1_1[:H] *= SQ                                           # x rows scaled
    w["wly1_0"] = i["W_ly1"][0]
    w["wly1_1"] = wly1_1
    w["wly2_0"] = i["W_ly2"][0]
    w["wly2_1"] = i["W_ly2"][1]
    w["wout"] = i["W_out"] * INV * 0.5                         # x2 = .5*x2'
    return w


def _pack_blobw(i):
    """Replicated bf16 weight blob [128, CB]."""
    w = _prep_weights(i)
    blob = np.zeros((128, CB), np.float32)
    # rhs row order is [snd attrs(16), rcv attrs(16), bessel(8)] so the
    # on-device copies land on legal partition offsets (0 and 32)
    blob[0:40, OFF_WE0:OFF_WE0 + 256] = np.vstack([w["we0"][8:40],
                                                   w["we0"][0:8]])
    blob[64:80, OFF_WE0:OFF_WE0 + 256] = w["wly1_0"][256:272]
    blob[0:16, OFF_FB1:OFF_FB1 + 256] = w["wly1_1"][256:272]
    for kc in range(2):
        s = slice(kc * 128, (kc + 1) * 128)
        blob[:, OFF_WE1 + kc * 256:OFF_WE1 + (kc + 1) * 256] = w["we1"][s]
        blob[:, OFF_WV0 + kc * 16:OFF_WV0 + (kc + 1) * 16] = w["wv0"][s]
        blob[:, OFF_WLW0 + kc * 16:OFF_WLW0 + (kc + 1) * 16] = w["wlw0"][s]
        blob[:, OFF_WLW1 + kc * 16:OFF_WLW1 + (kc + 1) * 16] = w["wlw1"][s]
        blob[:, OFF_WOUT + kc:OFF_WOUT + kc + 1] = w["wout"][s]
    for l in range(2):
        m = w[f"wly1_{l}"]
        for kc in range(2):
            s = slice(kc * 128, (kc + 1) * 128)
            blob[:, OFF_WLY1[l] + kc * 256:OFF_WLY1[l] + (kc + 1) * 256] = m[s]
            blob[:, OFF_WLY2[l] + kc * 256:OFF_WLY2[l] + (kc + 1) * 256] = \
                w[f"wly2_{l}"][s]
    return blob.astype(BF16)


def make_in_maps(inputs):
    kwin, shards = _host_shard(inputs["vectors"], inputs["senders"],
                               inputs["receivers"])
    EP = NW * kwin
    T_ALL = EP // 128
    bias_list = [inputs["b_e0"], inputs["b_e1"],
                 inputs["b_ly1"][0], inputs["b_ly1"][1],
                 inputs["b_ly2"][0], inputs["b_ly2"][1]]
    wcol = inputs["W_lsh"][0][:, 0]                            # [16]
    blobw = _pack_blobw(inputs)
    na = inputs["node_attrs"]                                  # [N, F]
    nat_scale = float(np.abs(na).max() / 127.0)
    naq = np.round(na / nat_scale).clip(-127, 127).astype(np.int8)
    # nat8[lo, f*64+hi] = naq[hi*128+lo, f]
    nat = naq.reshape(RWIN, 128, F).transpose(1, 2, 0).reshape(128, 1024)
    misc = np.zeros((128, 28), np.float32)
    for i, b in enumerate(bias_list):
        misc[:, 2 * i] = b[0:128]
        misc[:, 2 * i + 1] = b[128:256]
    misc[:, 12:28] = np.tile(wcol.reshape(1, 16), (128, 1))
    # f32 -> bf16 hi + bf16 lo pair (reconstructed by one add on device)
    mhi = misc.astype(BF16)
    mlo = (misc - mhi.astype(np.float32)).astype(BF16)
    blobx = np.zeros((128, CBX), BF16)
    blobx[:, 0:CB] = blobw
    blobx[:, OFF_MISC:OFF_MISC + 28] = mhi
    blobx[:, OFF_MISC + 28:OFF_MISC + 56] = mlo
    in_maps = []
    dbg = []
    for c in range(NC):
        v16, sl, rlo, rhi = shards[c]
        vhi = (v16 >> 8).astype(np.int8)
        vlo = ((v16 & 255) - 128).astype(np.int8)
        b8 = np.empty((128, 9 * T_ALL + NAT8), np.int8)
        b8[:, 0:T_ALL] = _plane(sl, T_ALL)
        b8[:, T_ALL:2 * T_ALL] = _plane(rlo, T_ALL)
        b8[:, 2 * T_ALL:3 * T_ALL] = _plane(rhi, T_ALL)
        b8[:, 3 * T_ALL:3 * T_ALL + 1024] = nat
        # snat[lo, w*16+f] = naq[(c*8+w)*128+lo, f]
        sn = naq.reshape(RWIN, 128, F)[c * NW:(c + 1) * NW]    # [w, lo, f]
        b8[:, 3 * T_ALL + 1024:3 * T_ALL + NAT8] = \
            sn.transpose(1, 0, 2).reshape(128, 128)
        o5 = 3 * T_ALL + NAT8
        o6 = o5 + 3 * T_ALL
        vph = _plane(vhi, T_ALL)                               # [3,128,T]
        vpl = _plane(vlo, T_ALL)
        for j in range(3):
            b8[:, o5 + j * T_ALL:o5 + (j + 1) * T_ALL] = vph[j]
            b8[:, o6 + j * T_ALL:o6 + (j + 1) * T_ALL] = vpl[j]
        in_maps.append({"blob8": b8, "blobw": blobx})
        dbg.append(dict(vec=v16.astype(np.float32) * VSCALE,
                        sl=sl, rlo=rlo, rhi=rhi))
    return kwin, nat_scale, in_maps, dbg


_CAP_SKIP = {"InstEventSemaphore", "InstBranch", "InstNop",
             "InstCollectiveCompute"}
_CAP_LIMITS = {}


def _split_waits(nc, mybir, mk_carrier, limit=1):
    """Walrus codegen allows only 1 embedded sem-wait on compute
    instructions.  For each instruction with more, strip the extras onto
    freshly created same-engine carrier instructions inserted directly
    before it (engines are in-order, so this preserves semantics)."""
    f = nc.m.functions[0]
    made = 0
    for bb in f.blocks:
        insts = list(bb.instructions)
        plan = []          # (index, [carrier insts])
        for i, inst in enumerate(insts):
            tname = type(inst).__name__
            si = inst.sync_info
            nwait = len(si.on_wait) if (si and si.on_wait) else 0
            lim = _CAP_LIMITS.get(tname, limit)
            if tname in _CAP_SKIP or nwait <= lim:
                continue
            waits = list(si.on_wait)
            extras, keep = waits[:-lim], waits[-lim:]
            carriers = []
            for wt in extras:
                ci = mk_carrier(inst.engine)
                if ci is None:
                    keep.insert(0, wt)
                    continue
                ci.sync_info = mybir.SyncInfo(on_wait=[wt], on_update=[])
                carriers.append(ci)
                made += 1
            inst.sync_info = mybir.SyncInfo(on_wait=keep,
                                            on_update=si.on_update)
            if carriers:
                plan.append((i, carriers))
        if plan:
            new = []
            pmap = dict(plan)
            for i, inst in enumerate(insts):
                if i in pmap:
                    new.extend(pmap[i])
                new.append(inst)
            bb.instructions = new
    return made


def build_graph(kwin, nat_scale):
    from concourse import bass, mybir
    from concourse.masks import make_identity
    from concourse.tile import TileContext

    EP = NW * kwin
    T_ALL = EP // 128
    T_W = kwin // 128
    NCH = (kwin + 511) // 512      # free chunks per window

    f32 = mybir.dt.float32
    bf16 = mybir.dt.bfloat16
    i32 = mybir.dt.int32
    i8 = mybir.dt.int8
    i16 = mybir.dt.int16
    AX = mybir.AxisListType.X
    OP = mybir.AluOpType
    AF = mybir.ActivationFunctionType

    nc = bass.Bass()
    carrier_sem_cm = nc.semaphore("carrier_sem")
    carrier_sem = carrier_sem_cm.__enter__()
    dp = nc.declare_dram_parameter
    d_b8 = dp("blob8", [128, 9 * T_ALL + NAT8], i8, isOutput=False)
    d_bw = dp("blobw", [128, CBX], bf16, isOutput=False)
    d_out = dp("out", [128, RWIN], f32, isOutput=True)

    with TileContext(nc) as tc:
        with (
            tc.tile_pool(name="glob", bufs=1) as gp,
            tc.tile_pool(name="wgt", bufs=1) as wp,
            tc.tile_pool(name="win", bufs=2) as wnp,
            tc.tile_pool(name="big", bufs=1) as bgp,
            tc.tile_pool(name="sml", bufs=3) as sp,
            tc.tile_pool(name="ps_mlp", bufs=2, space="PSUM") as pmlp,
            tc.tile_pool(name="ps_acc", bufs=1, space="PSUM") as pacc,
            tc.tile_pool(name="ps_gth", bufs=1, space="PSUM") as pgth,
            tc.tile_pool(name="ps_sml", bufs=1, space="PSUM") as psml,
            tc.tile_pool(name="ps_rcv", bufs=1, space="PSUM") as prcv,
        ):
            # ---------------- blobs to SBUF ----------------
            wb = wp.tile([128, CBX], bf16, tag="wb")
            nc.sync.dma_start(out=wb[:], in_=d_bw[:])
            i8t = wp.tile([128, 9 * T_ALL + NAT8], i8, tag="i8t")
            nc.sync.dma_start(out=i8t[:], in_=d_b8[:])
            # biases + wcol: f32 = bf16 hi + bf16 lo
            fbt = wp.tile([128, 28], f32, tag="fbt")
            nc.vector.tensor_add(out=fbt[:],
                                 in0=wb[:, OFF_MISC:OFF_MISC + 28],
                                 in1=wb[:, OFF_MISC + 28:OFF_MISC + 56])
            slf = wp.tile([128, T_ALL], f32, tag="slf")
            rlof = wp.tile([128, T_ALL], f32, tag="rlof")
            rhif = wp.tile([128, T_ALL], f32, tag="rhif")
            nc.vector.tensor_copy(out=slf[:], in_=i8t[:, 0:T_ALL])
            nc.vector.tensor_copy(out=rlof[:], in_=i8t[:, T_ALL:2 * T_ALL])
            nc.vector.tensor_copy(out=rhif[:], in_=i8t[:, 2 * T_ALL:3 * T_ALL])
            # dequantized node table (nat 1024 cols + snat 128 cols)
            natbf = wp.tile([128, NAT8], bf16, tag="natbf")
            nc.vector.tensor_scalar(
                out=natbf[:], in0=i8t[:, 3 * T_ALL:3 * T_ALL + NAT8],
                scalar1=float(nat_scale), scalar2=None, op0=OP.mult)

            ident = wp.tile([128, 128], f32, tag="ident")
            make_identity(nc, ident[:])
            identb = wp.tile([128, 128], bf16, tag="identb")
            make_identity(nc, identb[:])
            it32 = wp.tile([128, 128], i32, tag="it32")
            nc.gpsimd.iota(out=it32[:], pattern=[[1, 128]], base=0,
                           channel_multiplier=0)
            iof = wp.tile([128, 128], f32, tag="iof")
            nc.vector.tensor_copy(out=iof[:], in_=it32[:])
            ones_bf = wp.tile([1, 128], bf16, tag="ones")
            nc.gpsimd.memset(ones_bf[:], 1.0)

            # views into the blobs
            def bias(i, hc):
                return fbt[:, 2 * i + hc, None]
            wcol = fbt[:, 12:28]
            natv = natbf[:, 0:1024]

            # ---------------- edge-scalar stage (planes [128,T_ALL]) ----
            ta = gp.tile([128, T_ALL], f32)
            tb = gp.tile([128, T_ALL], f32)
            ts = nc.vector.tensor_scalar
            act = nc.scalar.activation
            # dequantize int16 fixed-point vectors from hi/lo int8 planes:
            # v = (hi*256 + lo + 128) * VSCALE
            vx = gp.tile([128, T_ALL], f32)
            vy = gp.tile([128, T_ALL], f32)
            vz = gp.tile([128, T_ALL], f32)
            o5 = 3 * T_ALL + NAT8
            o6 = o5 + 3 * T_ALL
            for j, vv in enumerate((vx, vy, vz)):
                ts(out=vv[:], in0=i8t[:, o5 + j * T_ALL:o5 + (j + 1) * T_ALL],
                   scalar1=256.0 * VSCALE, scalar2=None, op0=OP.mult)
                ts(out=ta[:], in0=i8t[:, o6 + j * T_ALL:o6 + (j + 1) * T_ALL],
                   scalar1=VSCALE, scalar2=128.0 * VSCALE,
                   op0=OP.mult, op1=OP.add)
                nc.vector.tensor_add(out=vv[:], in0=vv[:], in1=ta[:])

            def silu_act(out, ps_in, bias_ap):
                if not SIM_SILU:
                    act(out=out, in_=ps_in, func=AF.Silu, bias=bias_ap)
                else:
                    pp = ps_in.shape[0]
                    sg = bgp.tile([128, 512], f32, tag="simsilu")
                    zz_ = bgp.tile([128, 512], f32, tag="simsilu2")
                    cw_ = ps_in.shape[-1]
                    act(out=sg[:pp, :cw_], in_=ps_in, func=AF.Sigmoid,
                        bias=bias_ap)
                    nc.vector.tensor_scalar(out=zz_[:pp, :cw_], in0=ps_in,
                                            scalar1=bias_ap, scalar2=None,
                                            op0=OP.add)
                    nc.vector.tensor_mul(out=out, in0=sg[:pp, :cw_],
                                         in1=zz_[:pp, :cw_])
            d_pl = gp.tile([128, T_ALL], f32)
            nc.vector.tensor_mul(out=ta[:], in0=vx[:], in1=vx[:])
            nc.vector.tensor_mul(out=tb[:], in0=vy[:], in1=vy[:])
            nc.vector.tensor_add(out=ta[:], in0=ta[:], in1=tb[:])
            nc.vector.tensor_mul(out=tb[:], in0=vz[:], in1=vz[:])
            nc.vector.tensor_add(out=ta[:], in0=ta[:], in1=tb[:])
            act(out=d_pl[:], in_=ta[:], func=AF.Sqrt)
            rinv = gp.tile([128, T_ALL], f32)
            nc.vector.reciprocal(out=rinv[:], in_=d_pl[:])
            ux = gp.tile([128, T_ALL], f32)
            uy = gp.tile([128, T_ALL], f32)
            uz = gp.tile([128, T_ALL], f32)
            nc.vector.tensor_mul(out=ux[:], in0=vx[:], in1=rinv[:])
            nc.vector.tensor_mul(out=uy[:], in0=vy[:], in1=rinv[:])
            nc.vector.tensor_mul(out=uz[:], in0=vz[:], in1=rinv[:])

            # envelope u = 1 + d^6*(-28 + 48d - 21d^2), zero for d >= 1
            u_pl = gp.tile([128, T_ALL], f32)
            nc.vector.tensor_mul(out=ta[:], in0=d_pl[:], in1=d_pl[:])   # d2
            nc.vector.tensor_mul(out=tb[:], in0=ta[:], in1=d_pl[:])     # d3
            nc.vector.tensor_mul(out=tb[:], in0=tb[:], in1=tb[:])       # d6
            ts(out=ta[:], in0=ta[:], scalar1=-21.0, scalar2=None, op0=OP.mult)
            tc_q = gp.tile([128, T_ALL], f32)
            ts(out=tc_q[:], in0=d_pl[:], scalar1=48.0, scalar2=-28.0,
               op0=OP.mult, op1=OP.add)
            nc.vector.tensor_add(out=ta[:], in0=ta[:], in1=tc_q[:])
            nc.vector.tensor_mul(out=tb[:], in0=tb[:], in1=ta[:])
            ts(out=tb[:], in0=tb[:], scalar1=1.0, scalar2=None, op0=OP.add)
            ts(out=ta[:], in0=d_pl[:], scalar1=1.0, scalar2=None,
               op0=OP.is_lt)
            nc.vector.tensor_mul(out=u_pl[:], in0=tb[:], in1=ta[:])

            # spherical harmonics Y [128, T_ALL, 16] f32
            Yt = gp.tile([128, T_ALL, 16], f32)
            s3 = 3.0 ** 0.5; s5 = 5.0 ** 0.5; s15 = 15.0 ** 0.5
            s7 = 7.0 ** 0.5
            c33 = (35.0 / 8.0) ** 0.5; c32 = 105.0 ** 0.5
            c31 = (21.0 / 8.0) ** 0.5
            xx = gp.tile([128, T_ALL], f32)
            yy = gp.tile([128, T_ALL], f32)
            zz = gp.tile([128, T_ALL], f32)
            xy = gp.tile([128, T_ALL], f32)
            nc.vector.tensor_mul(out=xx[:], in0=ux[:], in1=ux[:])
            nc.vector.tensor_mul(out=yy[:], in0=uy[:], in1=uy[:])
            nc.vector.tensor_mul(out=zz[:], in0=uz[:], in1=uz[:])
            nc.vector.tensor_mul(out=xy[:], in0=ux[:], in1=uy[:])
            ts(out=Yt[:, :, 0], in0=ux[:], scalar1=0.0, scalar2=1.0,
               op0=OP.mult, op1=OP.add)
            ts(out=Yt[:, :, 1], in0=ux[:], scalar1=s3, scalar2=None,
               op0=OP.mult)
            ts(out=Yt[:, :, 2], in0=uy[:], scalar1=s3, scalar2=None,
               op0=OP.mult)
            ts(out=Yt[:, :, 3], in0=uz[:], scalar1=s3, scalar2=None,
               op0=OP.mult)
            ts(out=Yt[:, :, 4], in0=xy[:], scalar1=s15, scalar2=None,
               op0=OP.mult)
            nc.vector.tensor_mul(out=ta[:], in0=uy[:], in1=uz[:])
            ts(out=Yt[:, :, 5], in0=ta[:], scalar1=s15, scalar2=None,
               op0=OP.mult)
            ts(out=Yt[:, :, 6], in0=zz[:], scalar1=1.5 * s5,
               scalar2=-0.5 * s5, op0=OP.mult, op1=OP.add)
            nc.vector.tensor_mul(out=tb[:], in0=ux[:], in1=uz[:])
            ts(out=Yt[:, :, 7], in0=tb[:], scalar1=s15, scalar2=None,
               op0=OP.mult)
            xmy = gp.tile([128, T_ALL], f32)
            nc.vector.tensor_sub(out=xmy[:], in0=xx[:], in1=yy[:])
            ts(out=Yt[:, :, 8], in0=xmy[:], scalar1=0.5 * s15, scalar2=None,
               op0=OP.mult)
            ts(out=ta[:], in0=xx[:], scalar1=3.0, scalar2=None, op0=OP.mult)
            nc.vector.tensor_sub(out=ta[:], in0=ta[:], in1=yy[:])
            nc.vector.tensor_mul(out=ta[:], in0=ta[:], in1=uy[:])
            ts(out=Yt[:, :, 9], in0=ta[:], scalar1=c33, scalar2=None,
               op0=OP.mult)
            nc.vector.tensor_mul(out=ta[:], in0=xy[:], in1=uz[:])
            ts(out=Yt[:, :, 10], in0=ta[:], scalar1=c32, scalar2=None,
               op0=OP.mult)
            ts(out=ta[:], in0=zz[:], scalar1=5.0, scalar2=-1.0,
               op0=OP.mult, op1=OP.add)
            nc.vector.tensor_mul(out=tb[:], in0=ta[:], in1=uy[:])
            ts(out=Yt[:, :, 11], in0=tb[:], scalar1=c31, scalar2=None,
               op0=OP.mult)
            nc.vector.tensor_mul(out=tb[:], in0=ta[:], in1=ux[:])
            ts(out=Yt[:, :, 13], in0=tb[:], scalar1=c31, scalar2=None,
               op0=OP.mult)
            nc.vector.tensor_mul(out=ta[:], in0=zz[:], in1=uz[:])
            ts(out=ta[:], in0=ta[:], scalar1=2.5 * s7, scalar2=None,
               op0=OP.mult)
            ts(out=tb[:], in0=uz[:], scalar1=1.5 * s7, scalar2=None,
               op0=OP.mult)
            nc.vector.tensor_sub(out=Yt[:, :, 12], in0=ta[:], in1=tb[:])
            nc.vector.tensor_mul(out=ta[:], in0=xmy[:], in1=uz[:])
            ts(out=Yt[:, :, 14], in0=ta[:], scalar1=0.5 * c32, scalar2=None,
               op0=OP.mult)
            ts(out=ta[:], in0=yy[:], scalar1=3.0, scalar2=None, op0=OP.mult)
            nc.vector.tensor_sub(out=ta[:], in0=xx[:], in1=ta[:])
            nc.vector.tensor_mul(out=ta[:], in0=ta[:], in1=ux[:])
            ts(out=Yt[:, :, 15], in0=ta[:], scalar1=c33, scalar2=None,
               op0=OP.mult)

            # bessel (range-reduced): besu [128, T_ALL, 8]
            besu = gp.tile([128, T_ALL, 8], f32)
            rs = gp.tile([128, T_ALL], f32)
            ts(out=rs[:], in0=rinv[:], scalar1=math.sqrt(2.0), scalar2=None,
               op0=OP.mult)
            mi = gp.tile([128, T_ALL], mybir.dt.int32)
            for k in range(1, NB + 1):
                ts(out=ta[:], in0=d_pl[:], scalar1=0.5 * k, scalar2=None,
                   op0=OP.mult)
                nc.vector.tensor_copy(out=mi[:], in_=ta[:])
                nc.vector.tensor_copy(out=tb[:], in_=mi[:])
                nc.vector.tensor_sub(out=ta[:], in0=ta[:], in1=tb[:])
                # ta = frac in (-0.5, 1) whether the cast rounds or truncates
                ts(out=tb[:], in0=ta[:], scalar1=0.5, scalar2=None,
                   op0=OP.is_gt)
                nc.vector.tensor_sub(out=ta[:], in0=ta[:], in1=tb[:])
                act(out=ta[:], in_=ta[:], func=AF.Sin, scale=2.0 * math.pi)
                nc.vector.tensor_mul(out=besu[:, :, k - 1], in0=ta[:],
                                      in1=rs[:])

            # ---------------- persistent receiver accumulator ----------
            ps_rcv = prcv.tile([128, RWIN], f32, space="PSUM")

            # ---------------- window loop ----------------
            for w in range(NW):
                t0 = w * T_W
                ohs = wnp.tile([128, T_W, 128], bf16)   # [e, n]
                ohg = wnp.tile([128, T_W, 128], bf16)   # [n, e]
                rqs = wnp.tile([128, T_W, 128], bf16)   # [e, lo]
                rqg = wnp.tile([128, T_W, 128], bf16)   # [lo, e]
                rwt = wnp.tile([128, T_W, RWIN], bf16)  # [e, hi]
                xfm = wnp.tile([40, kwin], bf16)        # snd(16)+rcv(16)+bes(8)
                ufm = wnp.tile([1, kwin], bf16)
                tt = nc.vector.tensor_tensor
                tsw = slice(t0, t0 + T_W)
                tt(out=ohs[:],
                   in0=slf[:, tsw, None].to_broadcast([128, T_W, 128]),
                   in1=iof[:, None, :].to_broadcast([128, T_W, 128]),
                   op=OP.is_equal)
                tt(out=rqs[:],
                   in0=rlof[:, tsw, None].to_broadcast([128, T_W, 128]),
                   in1=iof[:, None, :].to_broadcast([128, T_W, 128]),
                   op=OP.is_equal)
                tt(out=rwt[:],
                   in0=rhif[:, tsw, None].to_broadcast([128, T_W, RWIN]),
                   in1=iof[:, None, 0:RWIN].to_broadcast([128, T_W, RWIN]),
                   op=OP.is_equal)
                for t in range(T_W):
                    tg = t0 + t
                    csl = slice(t * 128, (t + 1) * 128)
                    ptr = psml.tile([128, 128], bf16, space="PSUM", tag="trn")
                    nc.tensor.transpose(out=ptr[:], in_=ohs[:, t, :],
                                        identity=identb[:])
                    nc.vector.tensor_copy(out=ohg[:, t, :], in_=ptr[:])
                    ptr2 = psml.tile([128, 128], bf16, space="PSUM", tag="trn")
                    nc.tensor.transpose(out=ptr2[:], in_=rqs[:, t, :],
                                        identity=identb[:])
                    nc.vector.tensor_copy(out=rqg[:, t, :], in_=ptr2[:])
                    # bessel + u feature-major
                    pst = psml.tile([32, 128], f32, space="PSUM", tag="sml")
                    nc.tensor.transpose(out=pst[0:8, :], in_=besu[:, tg, :],
                                        identity=ident[:])
                    nc.vector.tensor_copy(out=xfm[32:40, csl],
                                          in_=pst[0:8, :])
                    psu1 = psml.tile([32, 128], f32, space="PSUM", tag="sml")
                    nc.tensor.transpose(out=psu1[0:1, :],
                                        in_=u_pl[:, tg, None],
                                        identity=ident[:])
                    nc.vector.tensor_copy(out=ufm[:, csl], in_=psu1[0:1, :])
                    # endpoint-attr gather: sender (window-local one-hot)
                    gcmb = sp.tile([128, 32], f32, tag="gcmb")
                    psn = psml.tile([128, 32], f32, space="PSUM", tag="sm2")
                    nc.tensor.matmul(
                        out=psn[:, 0:16], lhsT=ohg[:, t, :],
                        rhs=natbf[:, 1024 + w * 16:1024 + (w + 1) * 16],
                        start=True, stop=True)
                    nc.vector.tensor_copy(out=gcmb[:, 0:16], in_=psn[:, 0:16])
                    # receiver: lo-gather matmul then hi-select
                    for c2 in range(2):
                        prg = pgth.tile([128, 512], f32, space="PSUM",
                                        tag="gth")
                        nc.tensor.matmul(
                            out=prg[:], lhsT=rqg[:, t, :],
                            rhs=natv[:, c2 * 512:(c2 + 1) * 512],
                            start=True, stop=True)
                        prod = sp.tile([128, 8, RWIN], f32, tag="rsel")
                        nc.vector.tensor_mul(
                            out=prod[:],
                            in0=prg[:].rearrange("p (a b) -> p a b", b=RWIN),
                            in1=rwt[:, t, None, :].to_broadcast(
                                [128, 8, RWIN]))
                        nc.vector.reduce_sum(
                            out=gcmb[:, 16 + c2 * 8:16 + (c2 + 1) * 8, None],
                            in_=prod[:], axis=AX)
                    ptg = psml.tile([32, 128], f32, space="PSUM", tag="sml")
                    nc.tensor.transpose(out=ptg[:], in_=gcmb[:],
                                        identity=ident[:])
                    nc.vector.tensor_copy(out=xfm[0:32, csl], in_=ptg[:])

                # broadcast u row -> [128, kwin] bf16
                ubc = bgp.tile([128, kwin], bf16)
                for ch in range(NCH):
                    c0 = ch * 512
                    c1 = min(kwin, c0 + 512)
                    psu = pmlp.tile([128, 512], f32, space="PSUM", tag="mlp")
                    nc.tensor.matmul(out=psu[:, :c1 - c0], lhsT=ones_bf[:],
                                     rhs=ufm[:, c0:c1],
                                     start=True, stop=True)
                    nc.vector.tensor_copy(out=ubc[:, c0:c1],
                                          in_=psu[:, :c1 - c0])

                # ---- edge MLP: x0 = u*silu(e1(silu(e0(bes,attrs)))) ----
                x0 = bgp.tile([128, 2, kwin], bf16)
                th = bgp.tile([128, 2, kwin], bf16)
                for ch in range(NCH):
                    c0 = ch * 512
                    c1 = min(kwin, c0 + 512)
                    cw = c1 - c0
                    for hc in range(2):
                        ps = pmlp.tile([128, 512], f32, space="PSUM", tag="mlp")
                        nc.tensor.matmul(
                            out=ps[:, :cw],
                            lhsT=wb[0:40, OFF_WE0 + hc * 128:
                                    OFF_WE0 + (hc + 1) * 128],
                            rhs=xfm[:, c0:c1], start=True, stop=True)
                        silu_act(th[:, hc, c0:c1], ps[:, :cw], bias(0, hc))
                for ch in range(NCH):
                    c0 = ch * 512
                    c1 = min(kwin, c0 + 512)
                    cw = c1 - c0
                    for hc in range(2):
                        ps = pmlp.tile([128, 512], f32, space="PSUM", tag="mlp")
                        for kc in range(2):
                            nc.tensor.matmul(
                                out=ps[:, :cw],
                                lhsT=wb[:, OFF_WE1 + kc * 256 + hc * 128:
                                        OFF_WE1 + kc * 256 + (hc + 1) * 128],
                                rhs=th[:, kc, c0:c1],
                                start=(kc == 0), stop=(kc == 1))
                        silu_act(x0[:, hc, c0:c1], ps[:, :cw], bias(1, hc))
                for hc in range(2):
                    nc.vector.tensor_mul(out=x0[:, hc, :], in0=x0[:, hc, :],
                                          in1=ubc[:])

                # ---- xv, w0 (edge-major [128,16] per tile) ----
                xv = wnp.tile([128, T_W, MUL], f32)
                w0 = wnp.tile([128, T_W, MUL], bf16)
                for t in range(T_W):
                    tsl = slice(t * 128, (t + 1) * 128)
                    p12 = psml.tile([128, 32], f32, space="PSUM", tag="sm2")
                    for kc in range(2):
                        nc.tensor.matmul(
                            out=p12[:, 0:16], lhsT=x0[:, kc, tsl],
                            rhs=wb[:, OFF_WV0 + kc * 16:OFF_WV0 + (kc + 1) * 16],
                            start=(kc == 0), stop=(kc == 1))
                    for kc in range(2):
                        nc.tensor.matmul(
                            out=p12[:, 16:32], lhsT=x0[:, kc, tsl],
                            rhs=wb[:, OFF_WLW0 + kc * 16:
                                    OFF_WLW0 + (kc + 1) * 16],
                            start=(kc == 0), stop=(kc == 1))
                    nc.vector.tensor_copy(out=xv[:, t, :], in_=p12[:, 0:16])
                    nc.vector.tensor_copy(out=w0[:, t, :], in_=p12[:, 16:32])

                # ---- layer-0 scatter: wY[n, m*16+i] ----
                ps_acc = pacc.tile([128, 256], f32, space="PSUM", tag="acc")
                for t in range(T_W):
                    v2 = sp.tile([128, MUL, 16], bf16, tag="v2")
                    nc.vector.tensor_mul(
                        out=v2[:],
                        in0=w0[:, t, :, None].to_broadcast([128, MUL, 16]),
                        in1=Yt[:, t0 + t, None, :].to_broadcast(
                            [128, MUL, 16]))
                    nc.tensor.matmul(
                        out=ps_acc[:],
                        lhsT=ohs[:, t, :],
                        rhs=v2[:].rearrange("p a b -> p (a b)"),
                        start=(t == 0), stop=(t == T_W - 1))
                wY = wnp.tile([128, 256], bf16)
                nc.vector.tensor_copy(out=wY[:], in_=ps_acc[:])

                # ---- gather + Ytil contraction + feedback ----
                # fbfm lives at partitions 64..79 so its matmul shares the
                # base partition of the packed wly1fb_0 weights
                V10 = wnp.tile([128, T_W, MUL], f32)
                fbfm = wnp.tile([80, kwin], bf16)
                prod = wnp.tile([128, MUL, 16], f32)
                ytil = wnp.tile([128, MUL], f32)
                Ssb = wnp.tile([128, MUL], f32)
                fb = wnp.tile([128, MUL], f32)
                for t in range(T_W):
                    pgf = pgth.tile([128, 512], f32, space="PSUM", tag="gth")
                    pg = pgf[:, 0:256]
                    nc.tensor.matmul(out=pg, lhsT=ohg[:, t, :], rhs=wY[:],
                                     start=True, stop=True)
                    pg3 = pg.rearrange("p (a b) -> p a b", b=16)
                    nc.vector.tensor_mul(out=ytil[:], in0=Yt[:, t0 + t, :],
                                          in1=wcol)
                    nc.vector.tensor_mul(
                        out=prod[:], in0=pg3,
                        in1=ytil[:, None, :].to_broadcast([128, MUL, 16]))
                    nc.vector.reduce_sum(out=Ssb[:, :, None], in_=prod[:],
                                         axis=AX)
                    nc.vector.tensor_mul(out=V10[:, t, :], in0=Ssb[:],
                                          in1=xv[:, t, :])
                    nc.vector.tensor_mul(out=fb[:], in0=pg3[:, :, 0],
                                          in1=xv[:, t, :])
                    pst = psml.tile([32, 128], f32, space="PSUM", tag="sml")
                    nc.tensor.transpose(out=pst[0:16, :], in_=fb[:],
                                        identity=ident[:])
                    nc.vector.tensor_copy(
                        out=fbfm[64:80, t * 128:(t + 1) * 128],
                        in_=pst[0:16, :])

                # ---- layer-0 ly1/ly2 + residual -> x1 ----
                x1 = bgp.tile([128, 2, kwin], bf16)

                def mlp_block(xin, xout, l, fbrow, resid_sq2):
                    b1 = OFF_WLY1[l]
                    # wly1fb: layer 0 at rows 64..79 of the WE0 cols,
                    # layer 1 at rows 0..15 of its own FB1 cols
                    fbp, fbc = (64, OFF_WE0) if l == 0 else (0, OFF_FB1)
                    b2 = OFF_WLY2[l]
                    ty = bgp.tile([128, 2, kwin], bf16)
                    for ch in range(NCH):
                        c0 = ch * 512
                        c1 = min(kwin, c0 + 512)
                        cw = c1 - c0
                        for hc in range(2):
                            hs = slice(hc * 128, (hc + 1) * 128)
                            ps = pmlp.tile([128, 512], f32, space="PSUM",
                                           tag="mlp")
                            for kc in range(2):
                                nc.tensor.matmul(
                                    out=ps[:, :cw],
                                    lhsT=wb[:, b1 + kc * 256 + hc * 128:
                                            b1 + kc * 256 + (hc + 1) * 128],
                                    rhs=xin[:, kc, c0:c1],
                                    start=(kc == 0), stop=False)
                            nc.tensor.matmul(
                                out=ps[:, :cw],
                                lhsT=wb[fbp:fbp + 16, fbc + hc * 128:
                                        fbc + (hc + 1) * 128],
                                rhs=fbrow[fbp:fbp + 16, c0:c1],
                                start=False, stop=True)
                            silu_act(ty[:, hc, c0:c1], ps[:, :cw],
                                     bias(2 + l, hc))
                    ty2 = bgp.tile([128, 2, kwin], bf16)
                    for ch in range(NCH):
                        c0 = ch * 512
                        c1 = min(kwin, c0 + 512)
                        cw = c1 - c0
                        for hc in range(2):
                            ps = pmlp.tile([128, 512], f32, space="PSUM",
                                           tag="mlp")
                            for kc in range(2):
                                nc.tensor.matmul(
                                    out=ps[:, :cw],
                                    lhsT=wb[:, b2 + kc * 256 + hc * 128:
                                            b2 + kc * 256 + (hc + 1) * 128],
                                    rhs=ty[:, kc, c0:c1],
                                    start=(kc == 0), stop=(kc == 1))
                            silu_act(ty2[:, hc, c0:c1], ps[:, :cw],
                                     bias(4 + l, hc))
                    # x_out' = x_in' + s * u * y   (s = 1 or sqrt(2))
                    for hc in range(2):
                        nc.vector.tensor_mul(out=ty2[:, hc, :],
                                              in0=ty2[:, hc, :], in1=ubc[:])
                        if resid_sq2:
                            ts(out=ty2[:, hc, :], in0=ty2[:, hc, :],
                               scalar1=math.sqrt(2.0), scalar2=None,
                               op0=OP.mult)
                        nc.vector.tensor_add(out=xout[:, hc, :],
                                             in0=xin[:, hc, :],
                                             in1=ty2[:, hc, :])

                mlp_block(x0, x1, 0, fbfm, False)

                # ---- layer 1: w1, 16-wide scatter/gather, feedback ----
                w1 = wnp.tile([128, T_W, MUL], bf16)
                for t in range(T_W):
                    tsl = slice(t * 128, (t + 1) * 128)
                    p1 = psml.tile([128, 32], f32, space="PSUM", tag="sm2")
                    for kc in range(2):
                        nc.tensor.matmul(
                            out=p1[:, 0:MUL], lhsT=x1[:, kc, tsl],
                            rhs=wb[:, OFF_WLW1 + kc * 16:
                                    OFF_WLW1 + (kc + 1) * 16],
                            start=(kc == 0), stop=(kc == 1))
                    nc.vector.tensor_copy(out=w1[:, t, :], in_=p1[:, 0:MUL])
                ps_a1 = pacc.tile([128, 256], f32, space="PSUM", tag="acc")
                for t in range(T_W):
                    nc.tensor.matmul(out=ps_a1[:, 0:MUL], lhsT=ohs[:, t, :],
                                     rhs=w1[:, t, :],
                                     start=(t == 0), stop=(t == T_W - 1))
                wY1 = wnp.tile([128, MUL], bf16)
                nc.vector.tensor_copy(out=wY1[:], in_=ps_a1[:, 0:MUL])
                fbfm1 = wnp.tile([MUL, kwin], bf16)
                fb1 = wnp.tile([128, MUL], f32)
                for t in range(T_W):
                    pg = pgth.tile([128, 512], f32, space="PSUM", tag="gth")
                    nc.tensor.matmul(out=pg[:, 0:MUL], lhsT=ohg[:, t, :],
                                     rhs=wY1[:], start=True, stop=True)
                    nc.vector.tensor_mul(out=fb1[:], in0=pg[:, 0:MUL],
                                          in1=V10[:, t, :])
                    pst = psml.tile([32, 128], f32, space="PSUM", tag="sml")
                    nc.tensor.transpose(out=pst[0:16, :], in_=fb1[:],
                                        identity=ident[:])
                    nc.vector.tensor_copy(
                        out=fbfm1[:, t * 128:(t + 1) * 128],
                        in_=pst[0:16, :])

                # ---- layer-1 ly1/ly2 + residual -> x2 ----
                x2 = bgp.tile([128, 2, kwin], bf16)
                mlp_block(x1, x2, 1, fbfm1, True)

                # ---- edge out + receiver scatter ----
                eo = wnp.tile([128, 1], f32)
                mt = wnp.tile([128, RWIN], bf16)
                for t in range(T_W):
                    tsl = slice(t * 128, (t + 1) * 128)
                    p1 = psml.tile([128, 32], f32, space="PSUM", tag="sm2")
                    for kc in range(2):
                        nc.tensor.matmul(
                            out=p1[:, 0:1], lhsT=x2[:, kc, tsl],
                            rhs=wb[:, OFF_WOUT + kc:OFF_WOUT + kc + 1],
                            start=(kc == 0), stop=(kc == 1))
                    nc.vector.tensor_mul(out=eo[:], in0=p1[:, 0:1],
                                          in1=u_pl[:, t0 + t, None])
                    nc.vector.tensor_mul(
                        out=mt[:], in0=rwt[:, t, :],
                        in1=eo[:].to_broadcast([128, RWIN]))
                    nc.tensor.matmul(out=ps_rcv[:], lhsT=rqs[:, t, :],
                                     rhs=mt[:],
                                     start=(w == 0 and t == 0),
                                     stop=(w == NW - 1 and t == T_W - 1))

            out_sb = gp.tile([128, RWIN], f32)
            nc.vector.tensor_copy(out=out_sb[:], in_=ps_rcv[:])
            nc.sync.dma_start(out=d_out[:], in_=out_sb[:])

    ET = mybir.EngineType
    eng_map = {ET.DVE: nc.vector, ET.Activation: nc.scalar,
               ET.Pool: nc.gpsimd, ET.PE: nc.tensor, ET.SP: nc.sync}

    def mk_carrier(eng):
        be = eng_map.get(eng)
        if be is None:
            return None
        w = be.wait_ge(carrier_sem, 0)
        ci = w.ins if hasattr(w, "ins") else w
        for bb in nc.m.functions[0].blocks:
            il = list(bb.instructions)
            if any(x is ci for x in il):
                bb.instructions = [x for x in il if x is not ci]
                break
        return ci

    made = _split_waits(nc, mybir, mk_carrier)
    print(f"split_waits: carriers={made}", flush=True)
    return nc


def kernel(**inputs):
    inputs = {k: np.asarray(v) for k, v in inputs.items()}
    kwin, nat_scale, in_maps, _ = make_in_maps(inputs)
    nc = build_graph(kwin, nat_scale)
    from concourse.bass_utils import run_bass_kernel_spmd
    res = run_bass_kernel_spmd(nc, in_maps, core_ids=list(range(NC)))
    out = np.zeros((128, RWIN), np.float64)
    for r in res.results:
        out += np.asarray(r["out"], np.float64)
    # node n = hi*128 + lo stored at [lo, hi]
    return np.ascontiguousarray(out.T.reshape(N, 1)).astype(np.float32)
